# revision 1
# baseline (speedup 1.0000x reference)
"""BrainNetGAT Bass/Tile kernel for 8 Trainium2 NeuronCores.

Graph-level data parallelism: 16 graphs per core, processed as 8 pairs with
two concurrent PE column-tiled streams. Edge message passing is dense
augmented matmuls over each graph's 116x116 edge grid:
  moving tile T = [ea^T (5 rows); one-hot src-index (116 rows)]  [121, 13456]
  stationary   = [We (5 rows); Xsrc (116 rows)]                  [121, 64]
so one matmul yields ea@We + x_src[src] for every edge; a second matmul with
a broadcast-AP identity adds x_dst[dst]. GAT attention scores are computed by
a 4-stream block-diagonal att matmul, bounced through DRAM, reloaded as
[src, dst] matrices; exp() of the raw scores becomes the stationary of an
alpha-matmul whose rhs carries [xl + bias | ones] so the softmax denominator
comes out as column 64. BatchNorm stats and the global pooling softmax sum
use two small AllReduces.
"""
import contextlib

import numpy as np
import ml_dtypes

import concourse.bacc as bacc
import concourse.mybir as mybir
import concourse.tile as tile

F32 = mybir.dt.float32
BF16 = mybir.dt.bfloat16
AF = mybir.ActivationFunctionType
OP = mybir.AluOpType
AX = mybir.AxisListType

N_ROI = 116
HID = 64
EDIM = 5
N_GRAPHS = 128
N_CORES = 8
G_C = N_GRAPHS // N_CORES          # 16 graphs per core
PAIRS = G_C // 2
N_C = G_C * N_ROI                  # 1856 nodes per core
E_G = N_ROI * N_ROI                # 13456 edges per graph
E_C = G_C * E_G
N_TOTAL = N_GRAPHS * N_ROI         # 14848
KAUG = EDIM + N_ROI + 2            # 123: +2 exact hi/lo offset rows
CH = 4 * N_ROI                     # 464-col edge chunk (4 src blocks)
NCH = E_G // CH                    # 29
GRP = 3                            # z-chunks per psum group
NODE_CH = 4 * N_ROI                # 464 node cols (4 graphs)


def build_nc(n_cores=N_CORES):
    nc = bacc.Bacc()
    d = {}

    def inp(name, shape, dt):
        d[name] = nc.declare_dram_parameter(name, list(shape), dt,
                                            isOutput=False)

    inp("xT", (N_ROI, N_C), BF16)
    inp("eaT", (EDIM, E_C), BF16)
    inp("embT", (16, N_ROI), BF16)
    inp("I116", (N_ROI, N_ROI), BF16)
    inp("encW1", (N_ROI, HID), BF16)
    inp("encW2", (16, HID), BF16)
    inp("enc_b", (HID, 1), F32)
    inp("bn_g", (HID, 1), F32)
    inp("bn_b", (HID, 1), F32)
    inp("gineWe", (EDIM, HID), BF16)
    inp("gine_be", (HID, 1), F32)
    inp("gineW1", (HID, HID), F32)
    inp("gine_b1", (HID, 1), F32)
    inp("gineW2", (HID, HID), F32)
    inp("gine_b2", (HID, 1), F32)
    for l in range(2):
        inp(f"gatWl{l}", (HID, HID), F32)
        inp(f"gat_bl{l}", (HID, 1), F32)
        inp(f"gatWr{l}", (HID, HID), F32)
        inp(f"gat_br{l}", (HID, 1), F32)
        inp(f"gat_blb{l}", (HID, 1), F32)
        inp(f"gatWe{l}", (EDIM, HID), BF16)
        inp(f"att{l}", (128, 1), F32)
    inp("poolW1", (HID, HID), F32)
    inp("pool_b1", (HID, 1), F32)
    inp("pool_w2", (HID, 1), BF16)
    inp("lin1W", (HID, N_ROI), BF16)
    inp("lin1_b", (N_ROI, 1), F32)
    inp("lin2W", (N_ROI, 2), BF16)
    inp("lin2_b", (2, 1), F32)
    inp("ones2", (2, E_G), BF16)
    outT = nc.declare_dram_parameter("outT", [2, G_C], F32, isOutput=True)

    with tile.TileContext(nc) as tc:
        _body(nc, tc, d, outT, n_cores)
    nc.finalize()
    return nc


def _body(nc, tc, d, outT, n_cores=N_CORES):
    ctx = contextlib.ExitStack()
    with ctx:
        wpool = ctx.enter_context(tc.tile_pool(name="weights", bufs=1))
        state = ctx.enter_context(tc.tile_pool(name="state", bufs=1))
        tpool = ctx.enter_context(tc.tile_pool(name="tmoving", bufs=1))
        upool = ctx.enter_context(tc.tile_pool(name="u", bufs=1))
        spool = ctx.enter_context(tc.tile_pool(name="smalls", bufs=3))
        station = ctx.enter_context(tc.tile_pool(name="station", bufs=2))
        psA = ctx.enter_context(tc.tile_pool(name="psA", bufs=2, space="PSUM"))
        psB = ctx.enter_context(tc.tile_pool(name="psB", bufs=2, space="PSUM"))
        dpool = ctx.enter_context(tc.tile_pool(name="dram", bufs=2,
                                               space="DRAM"))
        scAB = dpool.tile([2, E_G], F32, tag="scAB")
        psc_dram = dpool.tile([1, N_C], F32, tag="psc_dram", bufs=1)
        bn_in = dpool.tile([HID, 2], F32, tag="bn_in", bufs=1)
        bn_out = dpool.tile([HID, 2], F32, tag="bn_out", bufs=1)
        s_in = dpool.tile([1, 1], F32, tag="s_in", bufs=1)
        s_out = dpool.tile([1, 1], F32, tag="s_out", bufs=1)

        # ---------- weights / constants ----------
        W = {}
        for name, h in d.items():
            if name in ("eaT", "ones2"):
                continue
            W[name] = wpool.tile(list(h.shape), h.dtype, tag=name, name=name)
            nc.gpsimd.dma_start(out=W[name][:], in_=h[:])

        ident = wpool.tile([128, 128], BF16, tag="ident")
        nc.vector.memset(ident[:], 0.0)
        nc.gpsimd.affine_select(out=ident[:], in_=ident[:],
                                compare_op=OP.not_equal, fill=1.0, base=0,
                                pattern=[[-1, 128]], channel_multiplier=1)
        identF = wpool.tile([128, 128], F32, tag="identF")
        nc.vector.memset(identF[:], 0.0)
        nc.gpsimd.affine_select(out=identF[:], in_=identF[:],
                                compare_op=OP.not_equal, fill=1.0, base=0,
                                pattern=[[-1, 128]], channel_multiplier=1)
        alpha02 = wpool.tile([128, 1], F32, tag="alpha02")
        nc.vector.memset(alpha02[:], 0.2)
        eps6 = wpool.tile([N_ROI, 1], F32, tag="eps6")
        nc.vector.memset(eps6[:], 1e-6)
        eps5 = wpool.tile([HID, 1], F32, tag="eps5")
        nc.vector.memset(eps5[:], 1e-5)
        ones116 = wpool.tile([N_ROI, 1], BF16, tag="ones116")
        nc.vector.memset(ones116[:], 1.0)

        attd = []
        for l in range(2):
            t = wpool.tile([128, 32], F32, tag=f"attd{l}")
            nc.vector.memset(t[:], 0.0)
            nc.vector.tensor_copy(t[0:HID, 0:1], W[f"att{l}"][0:HID, :])
            nc.vector.tensor_copy(t[HID:128, 1:2], W[f"att{l}"][HID:128, :])
            attd.append(t)

        Bbe = wpool.tile([HID, 1], F32, tag="Bbe")

        # persistent moving tiles: rows 0:5 = ea (per pair), 5:121 = s-ind
        T0 = tpool.tile([KAUG, E_G], BF16, tag="T0")
        T1 = tpool.tile([KAUG, E_G], BF16, tag="T1")
        sind_src = W["I116"][:, :].unsqueeze(2).broadcast_to(
            [N_ROI, N_ROI, N_ROI])
        for T in (T0, T1):
            nc.vector.tensor_copy(
                T[0:N_ROI, :].rearrange("p (s dd) -> p s dd", dd=N_ROI),
                sind_src)
            nc.gpsimd.dma_start(out=T[N_ROI + EDIM:KAUG, :],
                                in_=d["ones2"][:])
        dind = None  # built per use from I116

        hT = state.tile([HID, N_C], F32, tag="hT")
        hbeT = state.tile([HID, N_C], BF16, tag="hbeT")
        h0T = state.tile([HID, N_C], BF16, tag="h0T")
        tanT = state.tile([HID, N_C], BF16, tag="tanT")
        eMat = state.tile([N_ROI, G_C], F32, tag="eMat")
        hnew = []
        for g in range(G_C):
            hn_t = state.tile([N_ROI, HID], BF16, tag=f"hnew{g}",
                              name=f"hnew{g}")
            hnew.append(hn_t)

        # ---------- phase A: encoder + BN ----------
        emb_b = W["embT"][:, :].unsqueeze(1).broadcast_to([16, 4, N_ROI])
        for k in range(N_C // NODE_CH):
            sl = slice(k * NODE_CH, (k + 1) * NODE_CH)
            ps = psB.tile([HID, NODE_CH], F32, tag="mm_ps")
            nc.tensor.matmul(ps[:], W["encW1"][:], W["xT"][:, sl],
                             start=True, stop=False)
            nc.tensor.matmul(ps[:], W["encW2"][:], emb_b,
                             start=False, stop=True)
            nc.scalar.activation(h0T[:, sl], ps[:], AF.Relu,
                                 bias=W["enc_b"][:])

        st = spool.tile([HID, 2], F32, tag="bn_st")
        sq = upool.tile([HID, N_C], BF16, tag="sq")
        nc.vector.tensor_reduce(st[:, 0:1], h0T[:, :], axis=AX.X, op=OP.add)
        nc.vector.tensor_tensor(sq[:], h0T[:], h0T[:], op=OP.mult)
        nc.vector.tensor_reduce(st[:, 1:2], sq[:, :], axis=AX.X, op=OP.add)
        nc.gpsimd.dma_start(out=bn_in[:], in_=st[:])
        nc.gpsimd.collective_compute(
            "AllReduce", OP.add, replica_groups=[list(range(n_cores))],
            ins=[bn_in[:]], outs=[bn_out[:]])
        stg = spool.tile([HID, 2], F32, tag="bn_stg")
        nc.gpsimd.dma_start(out=stg[:], in_=bn_out[:])

        mu = spool.tile([HID, 1], F32, tag="mu")
        var = spool.tile([HID, 1], F32, tag="var")
        sd = spool.tile([HID, 1], F32, tag="sd")
        A = spool.tile([HID, 1], F32, tag="A")
        B = spool.tile([HID, 1], F32, tag="B")
        t3 = spool.tile([HID, 1], F32, tag="t3")
        nc.vector.tensor_scalar_mul(mu[:], stg[:, 0:1], 1.0 / N_TOTAL)
        nc.vector.tensor_scalar_mul(var[:], stg[:, 1:2], 1.0 / N_TOTAL)
        nc.vector.tensor_tensor(t3[:], mu[:], mu[:], op=OP.mult)
        nc.vector.tensor_tensor(var[:], var[:], t3[:], op=OP.subtract)
        nc.scalar.activation(sd[:], var[:], AF.Sqrt, bias=eps5[:])
        nc.vector.reciprocal(sd[:], sd[:])
        nc.vector.tensor_tensor(A[:], sd[:], W["bn_g"][:], op=OP.mult)
        nc.vector.tensor_tensor(t3[:], mu[:], A[:], op=OP.mult)
        nc.vector.tensor_tensor(B[:], W["bn_b"][:], t3[:], op=OP.subtract)
        nc.vector.tensor_tensor(Bbe[:], B[:], W["gine_be"][:], op=OP.add)
        nc.scalar.activation(hT[:, :], h0T[:, :], AF.Identity,
                             bias=B[:], scale=A[:])
        nc.scalar.activation(hbeT[:, :], h0T[:, :], AF.Identity,
                             bias=Bbe[:], scale=A[:])

        # ---------- phase B: per-pair ----------
        for p in range(PAIRS):
            gA, gB = 2 * p, 2 * p + 1
            cA = slice(gA * N_ROI, (gA + 1) * N_ROI)
            cB = slice(gB * N_ROI, (gB + 1) * N_ROI)
            cP = slice(gA * N_ROI, (gB + 1) * N_ROI)
            lA = slice(0, N_ROI)
            lB = slice(N_ROI, 2 * N_ROI)
            nc.gpsimd.dma_start(out=T0[N_ROI:N_ROI + EDIM, :],
                                in_=d["eaT"][:, gA * E_G:(gA + 1) * E_G])
            nc.gpsimd.dma_start(out=T1[N_ROI:N_ROI + EDIM, :],
                                in_=d["eaT"][:, gB * E_G:(gB + 1) * E_G])

            # ===== GINE =====
            SA = station.tile([KAUG - 2, HID], BF16, tag="SA")
            SB = station.tile([KAUG - 2, HID], BF16, tag="SB")
            nc.gpsimd.dma_start(out=SA[N_ROI:N_ROI + EDIM, :],
                                in_=d["gineWe"][:])
            nc.gpsimd.dma_start(out=SB[N_ROI:N_ROI + EDIM, :],
                                in_=d["gineWe"][:])
            for (S, cg) in ((SA, cA), (SB, cB)):
                trp = psB.tile([N_ROI, HID], BF16, tag="mm_ps")
                nc.tensor.transpose(trp[:], hbeT[:, cg], ident[0:HID, 0:HID])
                nc.vector.tensor_copy(S[0:N_ROI, :], trp[:])

            u1 = upool.tile([128, E_G], BF16, tag="u", bufs=1)
            for g0 in range(0, NCH, GRP):
                ng = min(GRP, NCH - g0)
                zps = psA.tile([128, GRP * 512], F32, tag="zps")
                for j in range(ng):
                    ch = slice((g0 + j) * CH, (g0 + j + 1) * CH)
                    pj = slice(j * 512, j * 512 + CH)
                    nc.tensor.matmul(zps[0:HID, pj], SA,
                                     T0[0:KAUG - 2, ch],
                                     start=True, stop=True)
                for j in range(ng):
                    ch = slice((g0 + j) * CH, (g0 + j + 1) * CH)
                    pj = slice(j * 512, j * 512 + CH)
                    nc.tensor.matmul(zps[HID:128, pj], SB,
                                     T1[0:KAUG - 2, ch],
                                     start=True, stop=True,
                                     tile_position=(0, 64))
                src = zps[:, :].rearrange("p (g c) -> p g c",
                                          c=512)[:, 0:ng, 0:CH]
                dst = u1[:, g0 * CH:(g0 + ng) * CH].rearrange(
                    "p (g c) -> p g c", c=CH)
                nc.scalar.activation(dst, src, AF.Relu)

            agg = spool.tile([128, N_ROI], F32, tag="agg")
            u3 = u1[:, :].rearrange("p (s dd) -> p s dd", dd=N_ROI)
            nc.vector.tensor_reduce(agg[:], u3.transpose([0, 2, 1]),
                                    axis=AX.X, op=OP.add)
            nc.vector.tensor_tensor(hT[:, cA], hT[:, cA], agg[0:HID, :],
                                    op=OP.add)
            aggB = spool.tile([HID, N_ROI], F32, tag="aggB")
            nc.gpsimd.dma_start(out=aggB[:], in_=agg[HID:128, :])
            nc.vector.tensor_tensor(hT[:, cB], hT[:, cB], aggB[:],
                                    op=OP.add)
            mp1 = psB.tile([HID, 2 * N_ROI], F32, tag="mm_ps")
            nc.tensor.matmul(mp1[:], W["gineW1"][:], hT[:, cP],
                             start=True, stop=True)
            mt = spool.tile([HID, 2 * N_ROI], F32, tag="mt")
            nc.scalar.activation(mt[:], mp1[:], AF.Relu, bias=W["gine_b1"][:])
            mp2 = psB.tile([HID, 2 * N_ROI], F32, tag="mm_ps")
            nc.tensor.matmul(mp2[:], W["gineW2"][:], mt[:],
                             start=True, stop=True)
            nc.scalar.activation(hT[:, cP], mp2[:], AF.Relu,
                                 bias=W["gine_b2"][:])

            # ===== GAT layers =====
            for l in range(2):
                xps = psB.tile([HID, 2 * N_ROI], F32, tag="mm_ps")
                nc.tensor.matmul(xps[:], W[f"gatWl{l}"][:], hT[:, cP],
                                 start=True, stop=True)
                xlT = spool.tile([HID, 2 * N_ROI], F32, tag="xlT")
                xlbT = spool.tile([HID, 2 * N_ROI], F32, tag="xlbT")
                nc.scalar.activation(xlT[:], xps[:], AF.Identity,
                                     bias=W[f"gat_bl{l}"][:])
                nc.scalar.activation(xlbT[:], xps[:], AF.Identity,
                                     bias=W[f"gat_blb{l}"][:])
                xps2 = psB.tile([HID, 2 * N_ROI], F32, tag="mm_ps")
                nc.tensor.matmul(xps2[:], W[f"gatWr{l}"][:], hT[:, cP],
                                 start=True, stop=True)
                xrT = spool.tile([HID, 2 * N_ROI], F32, tag="xrT")
                nc.scalar.activation(xrT[:], xps2[:], AF.Identity,
                                     bias=W[f"gat_br{l}"][:])

                SA2 = station.tile([KAUG, HID], BF16, tag="SA2")
                SB2 = station.tile([KAUG, HID], BF16, tag="SB2")
                XrA = station.tile([N_ROI, HID], BF16, tag="XrA")
                XrB = station.tile([N_ROI, HID], BF16, tag="XrB")
                XlbA = station.tile([N_ROI, HID], F32, tag="XlbA")
                XlbB = station.tile([N_ROI, HID], F32, tag="XlbB")
                nc.gpsimd.dma_start(out=SA2[N_ROI:N_ROI + EDIM, :],
                                    in_=d[f"gatWe{l}"][:])
                nc.gpsimd.dma_start(out=SB2[N_ROI:N_ROI + EDIM, :],
                                    in_=d[f"gatWe{l}"][:])
                for (S, Xr, Xlb, lg) in ((SA2, XrA, XlbA, lA),
                                         (SB2, XrB, XlbB, lB)):
                    # per-graph centering of xl/xr; exact offset via 2 rows
                    mL = spool.tile([HID, 1], F32, tag="mL")
                    mR = spool.tile([HID, 1], F32, tag="mR")
                    nc.vector.tensor_reduce(mL[:], xlT[:, lg], axis=AX.X,
                                            op=OP.add)
                    nc.vector.tensor_scalar_mul(mL[:], mL[:], 1.0 / N_ROI)
                    nc.vector.tensor_reduce(mR[:], xrT[:, lg], axis=AX.X,
                                            op=OP.add)
                    nc.vector.tensor_scalar_mul(mR[:], mR[:], 1.0 / N_ROI)
                    xlc = spool.tile([HID, N_ROI], BF16, tag="xlc")
                    xrc = spool.tile([HID, N_ROI], BF16, tag="xrc")
                    nc.vector.tensor_scalar(xlc[:], xlT[:, lg], mL[:],
                                            scalar2=None,
                                            op0=OP.subtract)
                    nc.vector.tensor_scalar(xrc[:], xrT[:, lg], mR[:],
                                            scalar2=None,
                                            op0=OP.subtract)
                    Ksum = spool.tile([HID, 1], F32, tag="Ksum")
                    nc.vector.tensor_tensor(Ksum[:], mL[:], mR[:], op=OP.add)
                    K2 = spool.tile([HID, 2], BF16, tag="K2")
                    nc.vector.tensor_copy(K2[:, 0:1], Ksum[:])
                    Klo = spool.tile([HID, 1], F32, tag="Klo")
                    nc.vector.tensor_tensor(Klo[:], Ksum[:], K2[:, 0:1],
                                            op=OP.subtract)
                    nc.vector.tensor_copy(K2[:, 1:2], Klo[:])
                    k2p = psB.tile([2, HID], BF16, tag="mm_ps")
                    nc.tensor.transpose(k2p[:], K2[:], ident[0:HID, 0:HID])
                    k2s = spool.tile([2, HID], BF16, tag="k2s")
                    nc.vector.tensor_copy(k2s[:], k2p[:])
                    nc.gpsimd.dma_start(out=S[N_ROI + EDIM:KAUG, :],
                                        in_=k2s[:])
                    t1p = psB.tile([N_ROI, HID], BF16, tag="mm_ps")
                    nc.tensor.transpose(t1p[:], xlc[:], ident[0:HID, 0:HID])
                    nc.vector.tensor_copy(S[0:N_ROI, :], t1p[:])
                    t2p = psB.tile([N_ROI, HID], BF16, tag="mm_ps")
                    nc.tensor.transpose(t2p[:], xrc[:], ident[0:HID, 0:HID])
                    nc.vector.tensor_copy(Xr[:], t2p[:])
                    t3p = psB.tile([N_ROI, HID], F32, tag="mm_ps")
                    nc.tensor.transpose(t3p[:], xlbT[:, lg],
                                        identF[0:HID, 0:HID])
                    nc.vector.tensor_copy(Xlb[:], t3p[:])

                dind = W["I116"][:, :].unsqueeze(1).broadcast_to(
                    [N_ROI, 4, N_ROI])
                u2 = upool.tile([128, E_G], F32, tag="u", bufs=1)
                for g0 in range(0, NCH, GRP):
                    ng = min(GRP, NCH - g0)
                    zps = psA.tile([128, GRP * 512], F32, tag="zps")
                    for j in range(ng):
                        ch = slice((g0 + j) * CH, (g0 + j + 1) * CH)
                        pj = slice(j * 512, j * 512 + CH)
                        nc.tensor.matmul(zps[0:HID, pj], SA2, T0[:, ch],
                                         start=True, stop=False)
                    for j in range(ng):
                        ch = slice((g0 + j) * CH, (g0 + j + 1) * CH)
                        pj = slice(j * 512, j * 512 + CH)
                        nc.tensor.matmul(zps[0:HID, pj], XrA, dind,
                                         start=False, stop=True)
                    for j in range(ng):
                        ch = slice((g0 + j) * CH, (g0 + j + 1) * CH)
                        pj = slice(j * 512, j * 512 + CH)
                        nc.tensor.matmul(zps[HID:128, pj], SB2, T1[:, ch],
                                         start=True, stop=False,
                                         tile_position=(0, 64))
                    for j in range(ng):
                        ch = slice((g0 + j) * CH, (g0 + j + 1) * CH)
                        pj = slice(j * 512, j * 512 + CH)
                        nc.tensor.matmul(zps[HID:128, pj], XrB, dind,
                                         start=False, stop=True,
                                         tile_position=(0, 64))
                    src = zps[:, :].rearrange("p (g c) -> p g c",
                                              c=512)[:, 0:ng, 0:CH]
                    dst = u2[:, g0 * CH:(g0 + ng) * CH].rearrange(
                        "p (g c) -> p g c", c=CH)
                    nc.scalar.activation(dst, src, AF.Prelu,
                                         alpha=alpha02[:])

                # attention scores: 4 col-tiled streams -> rows 0,32,64,96
                scAB_c = scAB[:, :].rearrange("r (cc c) -> r cc c", c=CH)
                for base0 in range(0, NCH, 12):
                    n = min(12, NCH - base0)
                    npad = (n + 3) // 4 * 4
                    nslot = npad // 4
                    sps = psA.tile([128, GRP * 512], F32, tag="zps")
                    for idx in range(npad):
                        c = base0 + min(idx, n - 1)
                        k, j = idx % 4, idx // 4
                        nc.tensor.matmul(
                            sps[32 * k:32 * k + 32, j * 512:j * 512 + CH],
                            attd[l], u2[:, c * CH:(c + 1) * CH],
                            start=True, stop=True,
                            tile_position=(0, 32 * k))
                    scc = spool.tile([128, GRP * 512], F32, tag="scc", bufs=2)
                    ssrc = sps[:, :].rearrange("p (j c) -> p j c",
                                               c=512)[:, 0:nslot, 0:CH]
                    sdst = scc[:, 0:nslot * CH].rearrange(
                        "p (j c) -> p j c", c=CH)
                    nc.scalar.activation(sdst, ssrc, AF.Copy)
                    for k in range(4):
                        nk = len([i for i in range(n) if i % 4 == k])
                        if nk == 0:
                            continue
                        src3 = scc[32 * k:32 * k + 2, 0:nslot * CH].rearrange(
                            "p (j c) -> p j c", c=CH)[:, 0:nk, :]
                        dst3 = scAB_c[:, base0 + k:base0 + n:4, :]
                        nc.gpsimd.dma_start(out=dst3, in_=src3)

                scAB_m = scAB[:, :].rearrange("r (s dd) -> r s dd",
                                              dd=N_ROI)
                for (g, rr, Xlb, lg, cg) in (
                        (gA, 0, XlbA, lA, cA),
                        (gB, 1, XlbB, lB, cB)):
                    ep = spool.tile([N_ROI, N_ROI], F32, tag="ep")
                    nc.gpsimd.dma_start(out=ep[:], in_=scAB_m[rr])
                    # softmax over src done in [dst, src] orientation
                    ept_ps = psB.tile([N_ROI, N_ROI], F32, tag="mm_ps")
                    nc.tensor.transpose(ept_ps[:], ep[:],
                                        identF[0:N_ROI, 0:N_ROI])
                    epT = spool.tile([N_ROI, N_ROI], F32, tag="epT")
                    nc.vector.tensor_copy(epT[:], ept_ps[:])
                    mrow = spool.tile([N_ROI, 1], F32, tag="mrow")
                    nc.vector.tensor_reduce(mrow[:], epT[:, :], axis=AX.X,
                                            op=OP.max)
                    mneg = spool.tile([N_ROI, 1], F32, tag="mneg")
                    nc.vector.tensor_scalar_mul(mneg[:], mrow[:], -1.0)
                    ehT = spool.tile([N_ROI, N_ROI], F32, tag="ehT")
                    nc.scalar.activation(ehT[:], epT[:], AF.Exp,
                                         bias=mneg[:])
                    srow = spool.tile([N_ROI, 1], F32, tag="srow")
                    nc.vector.tensor_reduce(srow[:], ehT[:, :], axis=AX.X,
                                            op=OP.add)
                    rrow = spool.tile([N_ROI, 1], F32, tag="rrow")
                    nc.vector.reciprocal(rrow[:], srow[:])
                    adT = spool.tile([N_ROI, N_ROI], F32, tag="adT")
                    nc.vector.tensor_scalar_mul(adT[:], ehT[:], rrow[:])
                    as_ps = psB.tile([N_ROI, N_ROI], F32, tag="mm_ps")
                    nc.tensor.transpose(as_ps[:], adT[:],
                                        identF[0:N_ROI, 0:N_ROI])
                    aS = spool.tile([N_ROI, N_ROI], F32, tag="aS")
                    nc.vector.tensor_copy(aS[:], as_ps[:])
                    am = psB.tile([N_ROI, HID], F32, tag="mm_ps")
                    nc.tensor.matmul(am[:], aS[:], Xlb[:],
                                     start=True, stop=True)
                    hnF = spool.tile([N_ROI, HID], F32, tag="hnF")
                    nc.scalar.activation(hnF[:], am[:], AF.Relu)
                    nc.vector.tensor_copy(hnew[g][:], hnF[:])
                    htp = psB.tile([HID, N_ROI], F32, tag="mm_ps")
                    nc.tensor.transpose(htp[:], hnF[:],
                                        identF[0:N_ROI, 0:N_ROI])
                    nc.vector.tensor_copy(hT[:, cg], htp[:])

            # ===== pool scores =====
            pps = psB.tile([HID, 2 * N_ROI], F32, tag="mm_ps")
            nc.tensor.matmul(pps[:], W["poolW1"][:], hT[:, cP],
                             start=True, stop=True)
            nc.scalar.activation(tanT[:, cP], pps[:], AF.Tanh,
                                 bias=W["pool_b1"][:])
            scp = psB.tile([1, 2 * N_ROI], F32, tag="mm_ps")
            nc.tensor.matmul(scp[:], W["pool_w2"][:], tanT[:, cP],
                             start=True, stop=True)
            scs = spool.tile([1, 2 * N_ROI], F32, tag="scs")
            nc.vector.tensor_copy(scs[:], scp[:])
            nc.gpsimd.dma_start(
                out=psc_dram[0, p * 2 * N_ROI:(p + 1) * 2 * N_ROI],
                in_=scs[:])

        # ---------- phase C: pooling + head ----------
        nc.gpsimd.dma_start(
            out=eMat[:], in_=psc_dram[0, :].rearrange("(g r) -> r g", r=N_ROI))
        eMb = spool.tile([N_ROI, G_C], BF16, tag="eMb")
        nc.scalar.activation(eMb[:], eMat[:], AF.Exp)
        ssum_ps = psB.tile([1, G_C], F32, tag="mm_ps")
        nc.tensor.matmul(ssum_ps[:], ones116[:], eMb[:],
                         start=True, stop=True)
        ssum = spool.tile([1, 1], F32, tag="ssum")
        nc.vector.tensor_reduce(ssum[:], ssum_ps[0:1, :], axis=AX.X,
                                op=OP.add)
        nc.gpsimd.dma_start(out=s_in[:], in_=ssum[:])
        nc.gpsimd.collective_compute(
            "AllReduce", OP.add, replica_groups=[list(range(n_cores))],
            ins=[s_in[:]], outs=[s_out[:]])
        sS64 = spool.tile([HID, 1], F32, tag="sS64")
        nc.gpsimd.dma_start(out=sS64[:], in_=s_out[:].broadcast_to([HID, 1]))
        nc.vector.reciprocal(sS64[:], sS64[:])

        pool_ps = psB.tile([HID, G_C], F32, tag="mm_ps")
        for g in range(G_C):
            nc.tensor.matmul(pool_ps[:, g:g + 1], hnew[g][:],
                             eMb[:, g:g + 1], start=True, stop=True)
        pooledT = spool.tile([HID, G_C], BF16, tag="pooledT")
        nc.scalar.activation(pooledT[:], pool_ps[:], AF.Identity,
                             scale=sS64[:])
        o1ps = psB.tile([N_ROI, G_C], F32, tag="mm_ps")
        nc.tensor.matmul(o1ps[:], W["lin1W"][:], pooledT[:],
                         start=True, stop=True)
        o1 = spool.tile([N_ROI, G_C], BF16, tag="o1")
        nc.scalar.activation(o1[:], o1ps[:], AF.Relu, bias=W["lin1_b"][:])
        o2ps = psB.tile([2, G_C], F32, tag="mm_ps")
        nc.tensor.matmul(o2ps[:], W["lin2W"][:], o1[:], start=True, stop=True)
        oflast = spool.tile([2, G_C], F32, tag="oflast")
        nc.scalar.activation(oflast[:], o2ps[:], AF.Identity,
                             bias=W["lin2_b"][:])
        nc.gpsimd.dma_start(out=outT[:], in_=oflast[:])


# ---------------------------------------------------------------------------
_NC_CACHE = {}


def get_nc():
    if "nc" not in _NC_CACHE:
        _NC_CACHE["nc"] = build_nc()
    return _NC_CACHE["nc"]


def host_prep(x, edge_attr, emb, enc_W, enc_b, bn_g, bn_b,
              gine_We, gine_be, gine_W1, gine_b1, gine_W2, gine_b2,
              gat_Wl, gat_bl, gat_Wr, gat_br, gat_att, gat_We, gat_bias,
              pool_W1, pool_b1, pool_w2, lin1_W, lin1_b, lin2_W, lin2_b,
              group_ids):
    bf = ml_dtypes.bfloat16
    f32 = np.float32

    def col(v):
        return np.ascontiguousarray(np.asarray(v, f32).reshape(-1, 1))

    base = {
        "embT": np.ascontiguousarray(
            np.asarray(emb, f32).T[:, np.asarray(group_ids[:N_ROI])]
        ).astype(bf),
        "I116": np.eye(N_ROI).astype(bf),
        "encW1": np.ascontiguousarray(np.asarray(enc_W, f32)[:N_ROI]
                                      ).astype(bf),
        "encW2": np.ascontiguousarray(np.asarray(enc_W, f32)[N_ROI:]
                                      ).astype(bf),
        "enc_b": col(enc_b), "bn_g": col(bn_g), "bn_b": col(bn_b),
        "gineWe": np.asarray(gine_We, f32).astype(bf),
        "gine_be": col(gine_be),
        "gineW1": np.asarray(gine_W1, f32),
        "gine_b1": col(gine_b1),
        "gineW2": np.asarray(gine_W2, f32),
        "gine_b2": col(gine_b2),
        "poolW1": np.asarray(pool_W1, f32),
        "pool_b1": col(pool_b1),
        "pool_w2": col(pool_w2).astype(bf),
        "lin1W": np.asarray(lin1_W, f32).astype(bf),
        "lin1_b": col(lin1_b),
        "lin2W": np.asarray(lin2_W, f32).astype(bf),
        "lin2_b": col(lin2_b),
        "ones2": np.ones((2, E_G)).astype(bf),
    }
    for l in range(2):
        base[f"gatWl{l}"] = np.asarray(gat_Wl[l], f32)
        base[f"gat_bl{l}"] = col(gat_bl[l])
        base[f"gatWr{l}"] = np.asarray(gat_Wr[l], f32)
        base[f"gat_br{l}"] = col(gat_br[l])
        base[f"gat_blb{l}"] = col(np.asarray(gat_bl[l], f32) +
                                  np.asarray(gat_bias[l], f32))
        base[f"gatWe{l}"] = np.asarray(gat_We[l], f32).astype(bf)
        base[f"att{l}"] = col(np.concatenate([np.asarray(gat_att[l], f32), np.asarray(gat_att[l], f32)]))

    xT = np.ascontiguousarray(np.asarray(x, f32).T).astype(bf)
    eaT = np.ascontiguousarray(np.asarray(edge_attr, f32).T).astype(bf)

    in_maps = []
    for c in range(N_CORES):
        m = dict(base)
        m["xT"] = np.ascontiguousarray(xT[:, c * N_C:(c + 1) * N_C])
        m["eaT"] = np.ascontiguousarray(eaT[:, c * E_C:(c + 1) * E_C])
        in_maps.append(m)
    return in_maps


def assemble_out(results):
    return np.concatenate([np.asarray(r["outT"], np.float32).T
                           for r in results], axis=0)


# ===========================================================================
# SPMD runner (replicates bass2jax.run_bass_via_pjrt, but reusable + timeable)
# ===========================================================================
def _make_runner(nc, n_cores=N_CORES):
    import jax
    import jax.numpy as jnp
    from jax.sharding import Mesh, PartitionSpec
    from jax.experimental.shard_map import shard_map
    from concourse import bass2jax
    from concourse.bass2jax import _bass_exec_p, partition_id_tensor
    import concourse.mybir as mb

    bass2jax.install_neuronx_cc_hook()
    partition_name = (nc.partition_id_tensor.name
                      if nc.partition_id_tensor else None)
    in_names, out_names, out_avals, zero_outs = [], [], [], []
    for alloc in nc.m.functions[0].allocations:
        if not isinstance(alloc, mb.MemoryLocationSet):
            continue
        name = alloc.memorylocations[0].name
        if alloc.kind == "ExternalInput":
            if name != partition_name:
                in_names.append(name)
        elif alloc.kind == "ExternalOutput":
            out_names.append(name)
            shape = tuple(alloc.tensor_shape)
            dtype = mb.dt.np(alloc.dtype)
            out_avals.append(jax.core.ShapedArray(shape, dtype))
            zero_outs.append(np.zeros(shape, dtype))
    n_params = len(in_names)
    all_in = in_names + out_names
    if partition_name is not None:
        all_in = all_in + [partition_name]

    def _body(*args):
        operands = list(args)
        if partition_name is not None:
            operands.append(partition_id_tensor())
        outs = _bass_exec_p.bind(
            *operands, out_avals=tuple(out_avals), in_names=tuple(all_in),
            out_names=tuple(out_names), lowering_input_output_aliases=(),
            sim_require_finite=False, sim_require_nnan=False, nc=nc)
        return tuple(outs)

    devices = jax.devices()[:n_cores]
    mesh = Mesh(np.asarray(devices), ("core",))
    nin = n_params + len(zero_outs)
    sharded = jax.jit(shard_map(
        _body, mesh=mesh, in_specs=(PartitionSpec("core"),) * nin,
        out_specs=(PartitionSpec("core"),) * len(out_names),
        check_rep=False), keep_unused=True)

    def run(in_maps):
        per_core = [[np.asarray(m[name]) for name in in_names]
                    for m in in_maps]
        concat_in = [np.concatenate([per_core[c][i] for c in range(n_cores)],
                                    axis=0) for i in range(n_params)]
        concat_zeros = [np.zeros((n_cores * z.shape[0], *z.shape[1:]),
                                 z.dtype) for z in zero_outs]
        out_arrs = sharded(*concat_in, *concat_zeros)
        jax.block_until_ready(out_arrs)
        return [{name: np.asarray(out_arrs[i]).reshape(
                    n_cores, *out_avals[i].shape)[c]
                 for i, name in enumerate(out_names)}
                for c in range(n_cores)]

    def run_device(dev_in, fresh_zero_arrs):
        out_arrs = sharded(*dev_in, *fresh_zero_arrs)
        jax.block_until_ready(out_arrs)
        return out_arrs

    runner = dict(run=run, run_device=run_device, sharded=sharded,
                  in_names=in_names, out_names=out_names,
                  out_avals=out_avals, zero_outs=zero_outs,
                  n_params=n_params, mesh=mesh, n_cores=n_cores)
    return runner


_RUNNER_CACHE = {}


def _get_runner():
    if "r" not in _RUNNER_CACHE:
        _RUNNER_CACHE["r"] = _make_runner(get_nc(), N_CORES)
    return _RUNNER_CACHE["r"]


# ===========================================================================
# structured-input check + numpy fallback
# ===========================================================================
def _is_structured(edge_index, batch, group_ids, num_graphs, N, E):
    ng = int(np.asarray(num_graphs))
    if ng != N_GRAPHS or N != ng * N_ROI or E != ng * E_G:
        return False
    src = np.asarray(edge_index[0])
    dst = np.asarray(edge_index[1])
    idx = np.arange(N_ROI)
    s = np.repeat(idx, N_ROI)
    dd = np.tile(idx, N_ROI)
    off = (np.arange(ng) * N_ROI)[:, None]
    if not np.array_equal(src.reshape(ng, E_G), s[None, :] + off):
        return False
    if not np.array_equal(dst.reshape(ng, E_G), dd[None, :] + off):
        return False
    if not np.array_equal(np.asarray(batch),
                          np.repeat(np.arange(ng), N_ROI)):
        return False
    gi = np.asarray(group_ids)
    if not np.array_equal(gi, np.tile(gi[:N_ROI], ng)):
        return False
    return True


def _numpy_fallback(x, edge_attr, emb, enc_W, enc_b, bn_g, bn_b,
                    gine_We, gine_be, gine_W1, gine_b1, gine_W2, gine_b2,
                    gat_Wl, gat_bl, gat_Wr, gat_br, gat_att, gat_We,
                    gat_bias, pool_W1, pool_b1, pool_w2, lin1_W, lin1_b,
                    lin2_W, lin2_b, edge_index, batch, group_ids,
                    num_graphs):
    f32 = np.float32
    x = np.asarray(x, f32)
    edge_attr = np.asarray(edge_attr, f32)
    src = np.asarray(edge_index[0]).astype(np.int64)
    dst = np.asarray(edge_index[1]).astype(np.int64)
    batch = np.asarray(batch).astype(np.int64)
    ng = int(np.asarray(num_graphs))
    N = x.shape[0]
    h = np.concatenate([x, np.asarray(emb, f32)[np.asarray(group_ids)]], 1)
    h = h @ np.asarray(enc_W, f32) + np.asarray(enc_b, f32)
    h = np.maximum(h, 0)
    mu = h.mean(0)
    var = h.var(0)
    h = (h - mu) / np.sqrt(var + 1e-5) * np.asarray(bn_g, f32) + \
        np.asarray(bn_b, f32)
    e = edge_attr @ np.asarray(gine_We, f32) + np.asarray(gine_be, f32)
    msg = np.maximum(h[src] + e, 0)
    agg = np.zeros_like(h)
    np.add.at(agg, dst, msg)
    h = h + agg
    h = np.maximum(h @ np.asarray(gine_W1, f32) +
                   np.asarray(gine_b1, f32), 0)
    h = h @ np.asarray(gine_W2, f32) + np.asarray(gine_b2, f32)
    h = np.maximum(h, 0)
    for l in range(2):
        xl = h @ np.asarray(gat_Wl, f32)[l] + np.asarray(gat_bl, f32)[l]
        xr = h @ np.asarray(gat_Wr, f32)[l] + np.asarray(gat_br, f32)[l]
        z = xl[src] + xr[dst] + edge_attr @ np.asarray(gat_We, f32)[l]
        z = np.where(z > 0, z, 0.2 * z)
        sc = z @ np.asarray(gat_att, f32)[l]
        m = np.full(N, -np.inf, f32)
        np.maximum.at(m, dst, sc)
        ex = np.exp(sc - m[dst])
        ssum = np.zeros(N, f32)
        np.add.at(ssum, dst, ex)
        alpha = ex / (ssum[dst] + np.float32(1e-16))
        acc = np.zeros_like(h)
        np.add.at(acc, dst, xl[src] * alpha[:, None])
        h = np.maximum(acc + np.asarray(gat_bias, f32)[l], 0)
    sc = np.tanh(h @ np.asarray(pool_W1, f32) + np.asarray(pool_b1, f32))
    sc = sc @ np.asarray(pool_w2, f32)
    ex = np.exp(sc - sc.max())
    w = ex / ex.sum()
    hw = h * w[:, None]
    pooled = np.zeros((ng, HID), f32)
    np.add.at(pooled, batch, hw)
    o = np.maximum(pooled @ np.asarray(lin1_W, f32) +
                   np.asarray(lin1_b, f32), 0)
    return (o @ np.asarray(lin2_W, f32) + np.asarray(lin2_b, f32)).astype(f32)


def kernel(x, edge_attr, emb, enc_W, enc_b, bn_g, bn_b,
           gine_We, gine_be, gine_W1, gine_b1, gine_W2, gine_b2,
           gat_Wl, gat_bl, gat_Wr, gat_br, gat_att, gat_We, gat_bias,
           pool_W1, pool_b1, pool_w2, lin1_W, lin1_b, lin2_W, lin2_b,
           edge_index, batch, group_ids, num_graphs):
    N = np.asarray(x).shape[0]
    E = np.asarray(edge_attr).shape[0]
    if not _is_structured(edge_index, batch, group_ids, num_graphs, N, E):
        return _numpy_fallback(
            x, edge_attr, emb, enc_W, enc_b, bn_g, bn_b, gine_We, gine_be,
            gine_W1, gine_b1, gine_W2, gine_b2, gat_Wl, gat_bl, gat_Wr,
            gat_br, gat_att, gat_We, gat_bias, pool_W1, pool_b1, pool_w2,
            lin1_W, lin1_b, lin2_W, lin2_b, edge_index, batch, group_ids,
            num_graphs)
    in_maps = host_prep(x, edge_attr, emb, enc_W, enc_b, bn_g, bn_b,
                        gine_We, gine_be, gine_W1, gine_b1, gine_W2,
                        gine_b2, gat_Wl, gat_bl, gat_Wr, gat_br, gat_att,
                        gat_We, gat_bias, pool_W1, pool_b1, pool_w2,
                        lin1_W, lin1_b, lin2_W, lin2_b, group_ids)
    runner = _get_runner()
    results = runner["run"](in_maps)
    return assemble_out(results)



# revision 36
# speedup vs baseline: 1.9012x; 1.9012x over previous
"""BrainNetGAT Bass/Tile kernel for 8 Trainium2 NeuronCores.

Graph-level data parallelism: 16 graphs per core, processed as 8 pairs with
two concurrent PE column-tiled streams. Edge message passing is dense
augmented matmuls over each graph's 116x116 edge grid, with edges in
DST-MAJOR order (edge = dst*116 + src):
  moving tile T = [one-hot src-index (116); ea^T (5); ones (2)]  [123, 13456]
  stationary   = [Xsrc (116); We (5); K (2)]                     [123, 64]
so one matmul yields ea@We + x_src[src] for every edge; a second matmul with
a per-chunk sliced broadcast-AP identity adds x_dst[dst]. Dst-major makes
the GINE segment-sum a contiguous-axis DVE reduce and makes the attention
score matrix load back from DRAM directly as [dst, src] with no transposes.
GAT attention scores are computed by a 4-stream block-diagonal att matmul
(bf16), bounced through DRAM. GINE relu runs on the Vector engine to keep
the Scalar/ACT engine for the GAT leaky-relu. Most small DMAs are issued
from the otherwise-idle Sync engine. BatchNorm stats and the global pooling
softmax sum use two small AllReduces.
"""
import contextlib

import numpy as np
import ml_dtypes

import concourse.bacc as bacc
import concourse.mybir as mybir
import concourse.tile as tile

F32 = mybir.dt.float32
BF16 = mybir.dt.bfloat16
AF = mybir.ActivationFunctionType
OP = mybir.AluOpType
AX = mybir.AxisListType

N_ROI = 116
HID = 64
EDIM = 5
N_GRAPHS = 128
N_CORES = 8
G_C = N_GRAPHS // N_CORES          # 16 graphs per core
PAIRS = G_C // 2
N_C = G_C * N_ROI                  # 1856 nodes per core
E_G = N_ROI * N_ROI                # 13456 edges per graph
E_C = G_C * E_G
N_TOTAL = N_GRAPHS * N_ROI         # 14848
# shared moving tile rows: 0:116 src-onehot, 116:121 eaA, 121:126 eaB,
# 126:128 ones (for the per-graph bf16-centering K rows)
EA_A = N_ROI                       # 116
EA_B = N_ROI + EDIM                # 121
ONES_R = N_ROI + 2 * EDIM          # 126
KAUG = ONES_R + 2                  # 128
KGINE = ONES_R                     # 126 rows for the GINE matmuls
CH = 4 * N_ROI                     # 464-col edge chunk (4 dst blocks)
NCH = E_G // CH                    # 29
GRP = 3                            # z-chunks per psum group
NODE_CH = 4 * N_ROI                # 464 node cols (4 graphs)


def build_nc(n_cores=N_CORES):
    nc = bacc.Bacc()
    d = {}

    def inp(name, shape, dt):
        d[name] = nc.declare_dram_parameter(name, list(shape), dt,
                                            isOutput=False)

    inp("xT", (N_ROI, N_C), BF16)
    inp("eaT", (EDIM, E_C), BF16)
    inp("embT", (16, N_ROI), BF16)
    inp("I116", (N_ROI, N_ROI), BF16)
    inp("encW1", (N_ROI, HID), BF16)
    inp("encW2", (16, HID), BF16)
    inp("enc_b", (HID, 1), F32)
    inp("bn_g", (HID, 1), F32)
    inp("bn_b", (HID, 1), F32)
    inp("gineWeZA", (2 * EDIM, HID), BF16)
    inp("gineWeZB", (2 * EDIM, HID), BF16)
    inp("gine_be", (HID, 1), F32)
    inp("gineW1", (HID, HID), F32)
    inp("gine_b1", (HID, 1), F32)
    inp("gineW2", (HID, HID), F32)
    inp("gine_b2", (HID, 1), F32)
    for l in range(2):
        inp(f"gatWl{l}", (HID, HID), F32)
        inp(f"gat_bl{l}", (HID, 1), F32)
        inp(f"gatWr{l}", (HID, HID), F32)
        inp(f"gat_br{l}", (HID, 1), F32)
        inp(f"gat_blb{l}", (HID, 1), F32)
        inp(f"gatWeZA{l}", (2 * EDIM, HID), BF16)
        inp(f"gatWeZB{l}", (2 * EDIM, HID), BF16)
        inp(f"att{l}", (128, 1), F32)
    inp("poolW1", (HID, HID), F32)
    inp("pool_b1", (HID, 1), F32)
    inp("pool_w2", (HID, 1), BF16)
    inp("lin1W", (HID, N_ROI), BF16)
    inp("lin1_b", (N_ROI, 1), F32)
    inp("lin2W", (N_ROI, 2), BF16)
    inp("lin2_b", (2, 1), F32)
    inp("ones2", (2, E_G), BF16)
    outT = nc.declare_dram_parameter("outT", [2, G_C], F32, isOutput=True)

    with tile.TileContext(nc) as tc:
        _body(nc, tc, d, outT, n_cores)
    nc.finalize()
    return nc


def _body(nc, tc, d, outT, n_cores=N_CORES):
    ctx = contextlib.ExitStack()
    with ctx:
        wpool = ctx.enter_context(tc.tile_pool(name="weights", bufs=1))
        state = ctx.enter_context(tc.tile_pool(name="state", bufs=1))
        tpool = ctx.enter_context(tc.tile_pool(name="tmoving", bufs=1))
        upool = ctx.enter_context(tc.tile_pool(name="u", bufs=1))
        spool = ctx.enter_context(tc.tile_pool(name="smalls", bufs=3))
        station = ctx.enter_context(tc.tile_pool(name="station", bufs=2))
        psA = ctx.enter_context(tc.tile_pool(name="psA", bufs=2, space="PSUM"))
        psB = ctx.enter_context(tc.tile_pool(name="psB", bufs=2, space="PSUM"))
        dpool = ctx.enter_context(tc.tile_pool(name="dram", bufs=2,
                                               space="DRAM"))
        bn_in = dpool.tile([HID, 2], F32, tag="bn_in", bufs=1)
        bn_out = dpool.tile([HID, 2], F32, tag="bn_out", bufs=1)
        s_in = dpool.tile([1, 1], F32, tag="s_in", bufs=1)
        s_out = dpool.tile([1, 1], F32, tag="s_out", bufs=1)
        psc_row = dpool.tile([1, N_C], F32, tag="psc_row", bufs=1)
        scAB = dpool.tile([2, E_G], F32, tag="scAB")

        # ---------- weights / constants ----------
        W = {}
        for name, h in d.items():
            if name in ("eaT", "ones2"):
                continue
            W[name] = wpool.tile(list(h.shape), h.dtype, tag=name, name=name)
            nc.gpsimd.dma_start(out=W[name][:], in_=h[:])

        ident = wpool.tile([128, 128], BF16, tag="ident")
        nc.vector.memset(ident[:], 0.0)
        nc.gpsimd.affine_select(out=ident[:], in_=ident[:],
                                compare_op=OP.not_equal, fill=1.0, base=0,
                                pattern=[[-1, 128]], channel_multiplier=1)
        identF = wpool.tile([128, 128], F32, tag="identF")
        nc.vector.memset(identF[:], 0.0)
        nc.gpsimd.affine_select(out=identF[:], in_=identF[:],
                                compare_op=OP.not_equal, fill=1.0, base=0,
                                pattern=[[-1, 128]], channel_multiplier=1)
        alpha02 = wpool.tile([128, 1], F32, tag="alpha02")
        nc.vector.memset(alpha02[:], 0.2)
        eps6 = wpool.tile([N_ROI, 1], F32, tag="eps6")
        nc.vector.memset(eps6[:], 1e-6)
        eps5 = wpool.tile([HID, 1], F32, tag="eps5")
        nc.vector.memset(eps5[:], 1e-5)
        ones116 = wpool.tile([N_ROI, 1], BF16, tag="ones116")
        nc.vector.memset(ones116[:], 1.0)

        attd = []
        for l in range(2):
            t = wpool.tile([128, 32], BF16, tag=f"attd{l}")
            nc.vector.memset(t[:], 0.0)
            nc.vector.tensor_copy(t[0:HID, 0:1], W[f"att{l}"][0:HID, :])
            nc.vector.tensor_copy(t[HID:128, 1:2], W[f"att{l}"][HID:128, :])
            attd.append(t)

        Bbe = wpool.tile([HID, 1], F32, tag="Bbe")

        # one shared moving tile (dst-major edges) for BOTH graphs of a pair:
        # rows 0:116 = s-onehot, 116:121 = eaA, 121:126 = eaB, 126:128 = ones
        T = tpool.tile([KAUG, E_G], BF16, tag="T")
        sind_src = W["I116"][:, :].unsqueeze(1).broadcast_to(
            [N_ROI, N_ROI, N_ROI])
        nc.vector.tensor_copy(
            T[0:N_ROI, :].rearrange("p (dd s) -> p dd s", s=N_ROI),
            sind_src)
        nc.gpsimd.dma_start(out=T[ONES_R:KAUG, :], in_=d["ones2"][:])

        hT = state.tile([HID, N_C], F32, tag="hT")
        hbeT = state.tile([HID, N_C], BF16, tag="hbeT")
        h0T = state.tile([HID, N_C], BF16, tag="h0T")
        tanT = state.tile([HID, N_C], BF16, tag="tanT")
        eMat = state.tile([N_ROI, G_C], F32, tag="eMat")
        hnew = []
        for g in range(G_C):
            hn_t = state.tile([N_ROI, HID], BF16, tag=f"hnew{g}",
                              name=f"hnew{g}")
            hnew.append(hn_t)

        # ---------- phase A: encoder + BN ----------
        emb_b = W["embT"][:, :].unsqueeze(1).broadcast_to([16, 4, N_ROI])
        for k in range(N_C // NODE_CH):
            sl = slice(k * NODE_CH, (k + 1) * NODE_CH)
            ps = psB.tile([HID, NODE_CH], F32, tag="mm_ps")
            nc.tensor.matmul(ps[:], W["encW1"][:], W["xT"][:, sl],
                             start=True, stop=False)
            nc.tensor.matmul(ps[:], W["encW2"][:], emb_b,
                             start=False, stop=True)
            nc.scalar.activation(h0T[:, sl], ps[:], AF.Relu,
                                 bias=W["enc_b"][:])

        st = spool.tile([HID, 2], F32, tag="bn_st")
        sq = upool.tile([HID, N_C], BF16, tag="sq")
        nc.vector.tensor_reduce(st[:, 0:1], h0T[:, :], axis=AX.X, op=OP.add)
        nc.vector.tensor_tensor(sq[:], h0T[:], h0T[:], op=OP.mult)
        nc.vector.tensor_reduce(st[:, 1:2], sq[:, :], axis=AX.X, op=OP.add)
        nc.gpsimd.dma_start(out=bn_in[:], in_=st[:])
        nc.gpsimd.collective_compute(
            "AllReduce", OP.add, replica_groups=[list(range(n_cores))],
            ins=[bn_in[:]], outs=[bn_out[:]])
        stg = spool.tile([HID, 2], F32, tag="bn_stg")
        nc.gpsimd.dma_start(out=stg[:], in_=bn_out[:])

        mu = spool.tile([HID, 1], F32, tag="mu")
        var = spool.tile([HID, 1], F32, tag="var")
        sd = spool.tile([HID, 1], F32, tag="sd")
        A = spool.tile([HID, 1], F32, tag="A")
        B = spool.tile([HID, 1], F32, tag="B")
        t3 = spool.tile([HID, 1], F32, tag="t3")
        nc.vector.tensor_scalar_mul(mu[:], stg[:, 0:1], 1.0 / N_TOTAL)
        nc.vector.tensor_scalar_mul(var[:], stg[:, 1:2], 1.0 / N_TOTAL)
        nc.vector.tensor_tensor(t3[:], mu[:], mu[:], op=OP.mult)
        nc.vector.tensor_tensor(var[:], var[:], t3[:], op=OP.subtract)
        nc.scalar.activation(sd[:], var[:], AF.Sqrt, bias=eps5[:])
        nc.vector.reciprocal(sd[:], sd[:])
        nc.vector.tensor_tensor(A[:], sd[:], W["bn_g"][:], op=OP.mult)
        nc.vector.tensor_tensor(t3[:], mu[:], A[:], op=OP.mult)
        nc.vector.tensor_tensor(B[:], W["bn_b"][:], t3[:], op=OP.subtract)
        nc.vector.tensor_tensor(Bbe[:], B[:], W["gine_be"][:], op=OP.add)
        nc.scalar.activation(hT[:, :], h0T[:, :], AF.Identity,
                             bias=B[:], scale=A[:])
        nc.scalar.activation(hbeT[:, :], h0T[:, :], AF.Identity,
                             bias=Bbe[:], scale=A[:])

        # ---------- phase B: per-pair ----------
        for p in range(PAIRS):
            gA, gB = 2 * p, 2 * p + 1
            cA = slice(gA * N_ROI, (gA + 1) * N_ROI)
            cB = slice(gB * N_ROI, (gB + 1) * N_ROI)
            cP = slice(gA * N_ROI, (gB + 1) * N_ROI)
            lA = slice(0, N_ROI)
            lB = slice(N_ROI, 2 * N_ROI)
            nc.sync.dma_start(out=T[EA_A:EA_A + EDIM, :],
                              in_=d["eaT"][:, gA * E_G:(gA + 1) * E_G])
            nc.sync.dma_start(out=T[EA_B:EA_B + EDIM, :],
                              in_=d["eaT"][:, gB * E_G:(gB + 1) * E_G])

            # ===== GINE =====
            SA = station.tile([KGINE, HID], BF16, tag="SA")
            SB = station.tile([KGINE, HID], BF16, tag="SB")
            nc.gpsimd.dma_start(out=SA[EA_A:KGINE, :],
                                in_=d["gineWeZA"][:])
            nc.gpsimd.dma_start(out=SB[EA_A:KGINE, :],
                                in_=d["gineWeZB"][:])
            for (S, cg) in ((SA, cA), (SB, cB)):
                trp = psB.tile([N_ROI, HID], BF16, tag="mm_ps")
                nc.tensor.transpose(trp[:], hbeT[:, cg], ident[0:HID, 0:HID])
                nc.vector.tensor_copy(S[0:N_ROI, :], trp[:])

            agg = spool.tile([128, N_ROI], F32, tag="agg")
            for g0 in range(0, NCH, GRP):
                ng = min(GRP, NCH - g0)
                zps = psA.tile([128, GRP * 512], F32, tag="zps")
                for j in range(ng):
                    ch = slice((g0 + j) * CH, (g0 + j + 1) * CH)
                    pj = slice(j * 512, j * 512 + CH)
                    nc.tensor.matmul(zps[0:HID, pj], SA,
                                     T[0:KGINE, ch],
                                     start=True, stop=True)
                for j in range(ng):
                    ch = slice((g0 + j) * CH, (g0 + j + 1) * CH)
                    pj = slice(j * 512, j * 512 + CH)
                    nc.tensor.matmul(zps[HID:128, pj], SB,
                                     T[0:KGINE, ch],
                                     start=True, stop=True,
                                     tile_position=(0, 64))
                # relu into a small scratch, then contiguous segment-sum
                u1g = spool.tile([128, GRP * CH], BF16, tag="u1g")
                srcv = zps[:, :].rearrange("p (g c) -> p g c",
                                           c=512)[:, 0:ng, 0:CH]
                dstv = u1g[:, 0:ng * CH].rearrange("p (g c) -> p g c", c=CH)
                nc.vector.tensor_scalar_max(dstv, srcv, 0.0)
                rv = u1g[:, 0:ng * CH].rearrange("p (dd s) -> p dd s",
                                                 s=N_ROI)
                nc.vector.tensor_reduce(agg[:, 4 * g0:4 * (g0 + ng)], rv,
                                        axis=AX.X, op=OP.add)
            nc.vector.tensor_tensor(hT[:, cA], hT[:, cA], agg[0:HID, :],
                                    op=OP.add)
            aggB = spool.tile([HID, N_ROI], F32, tag="aggB")
            nc.sync.dma_start(out=aggB[:], in_=agg[HID:128, :])
            nc.vector.tensor_tensor(hT[:, cB], hT[:, cB], aggB[:],
                                    op=OP.add)
            mp1 = psB.tile([HID, 2 * N_ROI], F32, tag="mm_ps")
            nc.tensor.matmul(mp1[:], W["gineW1"][:], hT[:, cP],
                             start=True, stop=True)
            mt = spool.tile([HID, 2 * N_ROI], F32, tag="mt")
            nc.scalar.activation(mt[:], mp1[:], AF.Relu, bias=W["gine_b1"][:])
            mp2 = psB.tile([HID, 2 * N_ROI], F32, tag="mm_ps")
            nc.tensor.matmul(mp2[:], W["gineW2"][:], mt[:],
                             start=True, stop=True)
            nc.scalar.activation(hT[:, cP], mp2[:], AF.Relu,
                                 bias=W["gine_b2"][:])

            # ===== GAT layers =====
            for l in range(2):
                xps = psB.tile([HID, 2 * N_ROI], F32, tag="mm_ps")
                nc.tensor.matmul(xps[:], W[f"gatWl{l}"][:], hT[:, cP],
                                 start=True, stop=True)
                xlT = spool.tile([HID, 2 * N_ROI], F32, tag="xlT")
                xlbT = spool.tile([HID, 2 * N_ROI], F32, tag="xlbT")
                nc.scalar.activation(xlT[:], xps[:], AF.Identity,
                                     bias=W[f"gat_bl{l}"][:])
                nc.scalar.activation(xlbT[:], xps[:], AF.Identity,
                                     bias=W[f"gat_blb{l}"][:])
                xps2 = psB.tile([HID, 2 * N_ROI], F32, tag="mm_ps")
                nc.tensor.matmul(xps2[:], W[f"gatWr{l}"][:], hT[:, cP],
                                 start=True, stop=True)
                xrT = spool.tile([HID, 2 * N_ROI], F32, tag="xrT")
                nc.scalar.activation(xrT[:], xps2[:], AF.Identity,
                                     bias=W[f"gat_br{l}"][:])

                SA2 = station.tile([KAUG, HID], BF16, tag="SA2")
                SB2 = station.tile([KAUG, HID], BF16, tag="SB2")
                XrA = station.tile([N_ROI, HID], BF16, tag="XrA")
                XrB = station.tile([N_ROI, HID], BF16, tag="XrB")
                XlbA = station.tile([N_ROI, HID], F32, tag="XlbA")
                XlbB = station.tile([N_ROI, HID], F32, tag="XlbB")
                nc.sync.dma_start(out=SA2[EA_A:ONES_R, :],
                                  in_=d[f"gatWeZA{l}"][:])
                nc.sync.dma_start(out=SB2[EA_A:ONES_R, :],
                                  in_=d[f"gatWeZB{l}"][:])
                for (S, Xr, Xlb, lg) in ((SA2, XrA, XlbA, lA),
                                         (SB2, XrB, XlbB, lB)):
                    # per-graph centering of xl/xr; exact offset via 2 rows
                    mL = spool.tile([HID, 1], F32, tag="mL")
                    mR = spool.tile([HID, 1], F32, tag="mR")
                    nc.vector.tensor_reduce(mL[:], xlT[:, lg], axis=AX.X,
                                            op=OP.add)
                    nc.vector.tensor_scalar_mul(mL[:], mL[:], 1.0 / N_ROI)
                    nc.vector.tensor_reduce(mR[:], xrT[:, lg], axis=AX.X,
                                            op=OP.add)
                    nc.vector.tensor_scalar_mul(mR[:], mR[:], 1.0 / N_ROI)
                    xlc = spool.tile([HID, N_ROI], BF16, tag="xlc")
                    xrc = spool.tile([HID, N_ROI], BF16, tag="xrc")
                    nc.vector.tensor_scalar(xlc[:], xlT[:, lg], mL[:],
                                            scalar2=None,
                                            op0=OP.subtract)
                    nc.vector.tensor_scalar(xrc[:], xrT[:, lg], mR[:],
                                            scalar2=None,
                                            op0=OP.subtract)
                    Ksum = spool.tile([HID, 1], F32, tag="Ksum")
                    nc.vector.tensor_tensor(Ksum[:], mL[:], mR[:], op=OP.add)
                    K2 = spool.tile([HID, 2], BF16, tag="K2")
                    nc.vector.tensor_copy(K2[:, 0:1], Ksum[:])
                    Klo = spool.tile([HID, 1], F32, tag="Klo")
                    nc.vector.tensor_tensor(Klo[:], Ksum[:], K2[:, 0:1],
                                            op=OP.subtract)
                    nc.vector.tensor_copy(K2[:, 1:2], Klo[:])
                    k2p = psB.tile([2, HID], BF16, tag="mm_ps")
                    nc.tensor.transpose(k2p[:], K2[:], ident[0:HID, 0:HID])
                    k2s = spool.tile([2, HID], BF16, tag="k2s")
                    nc.vector.tensor_copy(k2s[:], k2p[:])
                    nc.sync.dma_start(out=S[ONES_R:KAUG, :],
                                      in_=k2s[:])
                    t1p = psB.tile([N_ROI, HID], BF16, tag="mm_ps")
                    nc.tensor.transpose(t1p[:], xlc[:], ident[0:HID, 0:HID])
                    nc.vector.tensor_copy(S[0:N_ROI, :], t1p[:])
                    t2p = psB.tile([N_ROI, HID], BF16, tag="mm_ps")
                    nc.tensor.transpose(t2p[:], xrc[:], ident[0:HID, 0:HID])
                    nc.vector.tensor_copy(Xr[:], t2p[:])
                    t3p = psB.tile([N_ROI, HID], F32, tag="mm_ps")
                    nc.tensor.transpose(t3p[:], xlbT[:, lg],
                                        identF[0:HID, 0:HID])
                    nc.vector.tensor_copy(Xlb[:], t3p[:])

                u2 = upool.tile([128, E_G], BF16, tag="u")
                for g0 in range(0, NCH, GRP):
                    ng = min(GRP, NCH - g0)
                    zps = psA.tile([128, GRP * 512], F32, tag="zps")
                    for j in range(ng):
                        ch = slice((g0 + j) * CH, (g0 + j + 1) * CH)
                        pj = slice(j * 512, j * 512 + CH)
                        nc.tensor.matmul(zps[0:HID, pj], SA2, T[:, ch],
                                         start=True, stop=False)
                    for j in range(ng):
                        c4 = slice(4 * (g0 + j), 4 * (g0 + j) + 4)
                        pj = slice(j * 512, j * 512 + CH)
                        dind = W["I116"][:, c4].unsqueeze(2).broadcast_to(
                            [N_ROI, 4, N_ROI])
                        nc.tensor.matmul(zps[0:HID, pj], XrA, dind,
                                         start=False, stop=True)
                    for j in range(ng):
                        ch = slice((g0 + j) * CH, (g0 + j + 1) * CH)
                        pj = slice(j * 512, j * 512 + CH)
                        nc.tensor.matmul(zps[HID:128, pj], SB2, T[:, ch],
                                         start=True, stop=False,
                                         tile_position=(0, 64))
                    for j in range(ng):
                        c4 = slice(4 * (g0 + j), 4 * (g0 + j) + 4)
                        pj = slice(j * 512, j * 512 + CH)
                        dind = W["I116"][:, c4].unsqueeze(2).broadcast_to(
                            [N_ROI, 4, N_ROI])
                        nc.tensor.matmul(zps[HID:128, pj], XrB, dind,
                                         start=False, stop=True,
                                         tile_position=(0, 64))
                    src = zps[:, :].rearrange("p (g c) -> p g c",
                                              c=512)[:, 0:ng, 0:CH]
                    dst = u2[:, g0 * CH:(g0 + ng) * CH].rearrange(
                        "p (g c) -> p g c", c=CH)
                    nc.scalar.activation(dst, src, AF.Prelu,
                                         alpha=alpha02[:])

                # attention scores: 4 col-tiled streams -> rows 0,32,64,96
                scAB_c = scAB[:, :].rearrange("r (cc c) -> r cc c", c=CH)
                for base0 in range(0, NCH, 12):
                    n = min(12, NCH - base0)
                    npad = (n + 3) // 4 * 4
                    nslot = npad // 4
                    sps = psA.tile([128, GRP * 512], F32, tag="zps")
                    for idx in range(npad):
                        c = base0 + min(idx, n - 1)
                        k, j = idx % 4, idx // 4
                        nc.tensor.matmul(
                            sps[32 * k:32 * k + 32, j * 512:j * 512 + CH],
                            attd[l], u2[:, c * CH:(c + 1) * CH],
                            start=True, stop=True,
                            tile_position=(0, 32 * k))
                    scc = spool.tile([128, GRP * 512], F32, tag="scc", bufs=2)
                    ssrc = sps[:, :].rearrange("p (j c) -> p j c",
                                               c=512)[:, 0:nslot, 0:CH]
                    sdst = scc[:, 0:nslot * CH].rearrange(
                        "p (j c) -> p j c", c=CH)
                    nc.scalar.activation(sdst, ssrc, AF.Copy)
                    for k in range(4):
                        nk = len([i for i in range(n) if i % 4 == k])
                        if nk == 0:
                            continue
                        src3 = scc[32 * k:32 * k + 2, 0:nslot * CH].rearrange(
                            "p (j c) -> p j c", c=CH)[:, 0:nk, :]
                        dst3 = scAB_c[:, base0 + k:base0 + n:4, :]
                        nc.sync.dma_start(out=dst3, in_=src3)

                # dst-major: scAB rows reload directly as [dst, src]
                scAB_m = scAB[:, :].rearrange("r (dd s) -> r dd s",
                                              s=N_ROI)
                for (g, rr, Xlb, lg, cg) in (
                        (gA, 0, XlbA, lA, cA),
                        (gB, 1, XlbB, lB, cB)):
                    epT = spool.tile([N_ROI, N_ROI], F32, tag="epT")
                    nc.sync.dma_start(out=epT[:], in_=scAB_m[rr])
                    mrow = spool.tile([N_ROI, 1], F32, tag="mrow")
                    nc.vector.tensor_reduce(mrow[:], epT[:, :], axis=AX.X,
                                            op=OP.max)
                    mneg = spool.tile([N_ROI, 1], F32, tag="mneg")
                    nc.vector.tensor_scalar_mul(mneg[:], mrow[:], -1.0)
                    ehT = spool.tile([N_ROI, N_ROI], F32, tag="ehT")
                    nc.scalar.activation(ehT[:], epT[:], AF.Exp,
                                         bias=mneg[:])
                    srow = spool.tile([N_ROI, 1], F32, tag="srow")
                    nc.vector.tensor_reduce(srow[:], ehT[:, :], axis=AX.X,
                                            op=OP.add)
                    rrow = spool.tile([N_ROI, 1], F32, tag="rrow")
                    nc.vector.reciprocal(rrow[:], srow[:])
                    adT = spool.tile([N_ROI, N_ROI], F32, tag="adT")
                    nc.vector.tensor_scalar_mul(adT[:], ehT[:], rrow[:])
                    as_ps = psB.tile([N_ROI, N_ROI], F32, tag="mm_ps")
                    nc.tensor.transpose(as_ps[:], adT[:],
                                        identF[0:N_ROI, 0:N_ROI])
                    aS = spool.tile([N_ROI, N_ROI], F32, tag="aS")
                    nc.vector.tensor_copy(aS[:], as_ps[:])
                    am = psB.tile([N_ROI, HID], F32, tag="mm_ps")
                    nc.tensor.matmul(am[:], aS[:], Xlb[:],
                                     start=True, stop=True)
                    hnF = spool.tile([N_ROI, HID], F32, tag="hnF")
                    nc.scalar.activation(hnF[:], am[:], AF.Relu)
                    nc.vector.tensor_copy(hnew[g][:], hnF[:])
                    htp = psB.tile([HID, N_ROI], F32, tag="mm_ps")
                    nc.tensor.transpose(htp[:], hnF[:],
                                        identF[0:N_ROI, 0:N_ROI])
                    nc.vector.tensor_copy(hT[:, cg], htp[:])

            # ===== pool scores =====
            pps = psB.tile([HID, 2 * N_ROI], F32, tag="mm_ps")
            nc.tensor.matmul(pps[:], W["poolW1"][:], hT[:, cP],
                             start=True, stop=True)
            nc.scalar.activation(tanT[:, cP], pps[:], AF.Tanh,
                                 bias=W["pool_b1"][:])
            scp = psB.tile([1, 2 * N_ROI], F32, tag="mm_ps")
            nc.tensor.matmul(scp[:], W["pool_w2"][:], tanT[:, cP],
                             start=True, stop=True)
            scs = spool.tile([1, 2 * N_ROI], F32, tag="scs")
            nc.vector.tensor_copy(scs[:], scp[:])
            nc.sync.dma_start(
                out=psc_row[0, p * 2 * N_ROI:(p + 1) * 2 * N_ROI],
                in_=scs[:])

        # ---------- phase C: pooling + head ----------
        nc.gpsimd.dma_start(
            out=eMat[:], in_=psc_row[0, :].rearrange("(g r) -> r g", r=N_ROI))
        eMb = spool.tile([N_ROI, G_C], BF16, tag="eMb")
        nc.scalar.activation(eMb[:], eMat[:], AF.Exp)
        ssum_ps = psB.tile([1, G_C], F32, tag="mm_ps")
        nc.tensor.matmul(ssum_ps[:], ones116[:], eMb[:],
                         start=True, stop=True)
        ssum = spool.tile([1, 1], F32, tag="ssum")
        nc.vector.tensor_reduce(ssum[:], ssum_ps[0:1, :], axis=AX.X,
                                op=OP.add)
        nc.gpsimd.dma_start(out=s_in[:], in_=ssum[:])
        nc.gpsimd.collective_compute(
            "AllReduce", OP.add, replica_groups=[list(range(n_cores))],
            ins=[s_in[:]], outs=[s_out[:]])
        sS64 = spool.tile([HID, 1], F32, tag="sS64")
        nc.gpsimd.dma_start(out=sS64[:], in_=s_out[:].broadcast_to([HID, 1]))
        nc.vector.reciprocal(sS64[:], sS64[:])

        pool_ps = psB.tile([HID, G_C], F32, tag="mm_ps")
        for g in range(G_C):
            nc.tensor.matmul(pool_ps[:, g:g + 1], hnew[g][:],
                             eMb[:, g:g + 1], start=True, stop=True)
        pooledT = spool.tile([HID, G_C], BF16, tag="pooledT")
        nc.scalar.activation(pooledT[:], pool_ps[:], AF.Identity,
                             scale=sS64[:])
        o1ps = psB.tile([N_ROI, G_C], F32, tag="mm_ps")
        nc.tensor.matmul(o1ps[:], W["lin1W"][:], pooledT[:],
                         start=True, stop=True)
        o1 = spool.tile([N_ROI, G_C], BF16, tag="o1")
        nc.scalar.activation(o1[:], o1ps[:], AF.Relu, bias=W["lin1_b"][:])
        o2ps = psB.tile([2, G_C], F32, tag="mm_ps")
        nc.tensor.matmul(o2ps[:], W["lin2W"][:], o1[:], start=True, stop=True)
        oflast = spool.tile([2, G_C], F32, tag="oflast")
        nc.scalar.activation(oflast[:], o2ps[:], AF.Identity,
                             bias=W["lin2_b"][:])
        nc.gpsimd.dma_start(out=outT[:], in_=oflast[:])


# ---------------------------------------------------------------------------
_NC_CACHE = {}


def get_nc():
    if "nc" not in _NC_CACHE:
        _NC_CACHE["nc"] = build_nc()
    return _NC_CACHE["nc"]


def host_prep(x, edge_attr, emb, enc_W, enc_b, bn_g, bn_b,
              gine_We, gine_be, gine_W1, gine_b1, gine_W2, gine_b2,
              gat_Wl, gat_bl, gat_Wr, gat_br, gat_att, gat_We, gat_bias,
              pool_W1, pool_b1, pool_w2, lin1_W, lin1_b, lin2_W, lin2_b,
              group_ids):
    bf = ml_dtypes.bfloat16
    f32 = np.float32

    def col(v):
        return np.ascontiguousarray(np.asarray(v, f32).reshape(-1, 1))

    base = {
        "embT": np.ascontiguousarray(
            np.asarray(emb, f32).T[:, np.asarray(group_ids[:N_ROI])]
        ).astype(bf),
        "I116": np.eye(N_ROI).astype(bf),
        "encW1": np.ascontiguousarray(np.asarray(enc_W, f32)[:N_ROI]
                                      ).astype(bf),
        "encW2": np.ascontiguousarray(np.asarray(enc_W, f32)[N_ROI:]
                                      ).astype(bf),
        "enc_b": col(enc_b), "bn_g": col(bn_g), "bn_b": col(bn_b),
        "gineWeZA": np.vstack([np.asarray(gine_We, f32),
                               np.zeros((EDIM, HID), f32)]).astype(bf),
        "gineWeZB": np.vstack([np.zeros((EDIM, HID), f32),
                               np.asarray(gine_We, f32)]).astype(bf),
        "gine_be": col(gine_be),
        "gineW1": np.asarray(gine_W1, f32),
        "gine_b1": col(gine_b1),
        "gineW2": np.asarray(gine_W2, f32),
        "gine_b2": col(gine_b2),
        "poolW1": np.asarray(pool_W1, f32),
        "pool_b1": col(pool_b1),
        "pool_w2": col(pool_w2).astype(bf),
        "lin1W": np.asarray(lin1_W, f32).astype(bf),
        "lin1_b": col(lin1_b),
        "lin2W": np.asarray(lin2_W, f32).astype(bf),
        "lin2_b": col(lin2_b),
        "ones2": np.ones((2, E_G)).astype(bf),
    }
    for l in range(2):
        base[f"gatWl{l}"] = np.asarray(gat_Wl[l], f32)
        base[f"gat_bl{l}"] = col(gat_bl[l])
        base[f"gatWr{l}"] = np.asarray(gat_Wr[l], f32)
        base[f"gat_br{l}"] = col(gat_br[l])
        base[f"gat_blb{l}"] = col(np.asarray(gat_bl[l], f32) +
                                  np.asarray(gat_bias[l], f32))
        base[f"gatWeZA{l}"] = np.vstack([np.asarray(gat_We[l], f32),
                                         np.zeros((EDIM, HID), f32)
                                         ]).astype(bf)
        base[f"gatWeZB{l}"] = np.vstack([np.zeros((EDIM, HID), f32),
                                         np.asarray(gat_We[l], f32)
                                         ]).astype(bf)
        base[f"att{l}"] = col(np.concatenate([np.asarray(gat_att[l], f32), np.asarray(gat_att[l], f32)]))

    xT = np.ascontiguousarray(np.asarray(x, f32).T).astype(bf)
    # reorder edges to dst-major within each graph: edge = dst*116 + src
    ea4 = np.asarray(edge_attr, f32).reshape(N_GRAPHS, N_ROI, N_ROI, EDIM)
    ea_dm = np.ascontiguousarray(ea4.transpose(0, 2, 1, 3)).reshape(-1, EDIM)
    eaT = np.ascontiguousarray(ea_dm.T).astype(bf)

    in_maps = []
    for c in range(N_CORES):
        m = dict(base)
        m["xT"] = np.ascontiguousarray(xT[:, c * N_C:(c + 1) * N_C])
        m["eaT"] = np.ascontiguousarray(eaT[:, c * E_C:(c + 1) * E_C])
        in_maps.append(m)
    return in_maps


def assemble_out(results):
    return np.concatenate([np.asarray(r["outT"], np.float32).T
                           for r in results], axis=0)


# ===========================================================================
# SPMD runner (replicates bass2jax.run_bass_via_pjrt, but reusable + timeable)
# ===========================================================================
def _make_runner(nc, n_cores=N_CORES):
    import jax
    import jax.numpy as jnp
    from jax.sharding import Mesh, PartitionSpec
    from jax.experimental.shard_map import shard_map
    from concourse import bass2jax
    from concourse.bass2jax import _bass_exec_p, partition_id_tensor
    import concourse.mybir as mb

    bass2jax.install_neuronx_cc_hook()
    partition_name = (nc.partition_id_tensor.name
                      if nc.partition_id_tensor else None)
    in_names, out_names, out_avals, zero_outs = [], [], [], []
    for alloc in nc.m.functions[0].allocations:
        if not isinstance(alloc, mb.MemoryLocationSet):
            continue
        name = alloc.memorylocations[0].name
        if alloc.kind == "ExternalInput":
            if name != partition_name:
                in_names.append(name)
        elif alloc.kind == "ExternalOutput":
            out_names.append(name)
            shape = tuple(alloc.tensor_shape)
            dtype = mb.dt.np(alloc.dtype)
            out_avals.append(jax.core.ShapedArray(shape, dtype))
            zero_outs.append(np.zeros(shape, dtype))
    n_params = len(in_names)
    all_in = in_names + out_names
    if partition_name is not None:
        all_in = all_in + [partition_name]

    def _body(*args):
        operands = list(args)
        if partition_name is not None:
            operands.append(partition_id_tensor())
        outs = _bass_exec_p.bind(
            *operands, out_avals=tuple(out_avals), in_names=tuple(all_in),
            out_names=tuple(out_names), lowering_input_output_aliases=(),
            sim_require_finite=False, sim_require_nnan=False, nc=nc)
        return tuple(outs)

    devices = jax.devices()[:n_cores]
    mesh = Mesh(np.asarray(devices), ("core",))
    nin = n_params + len(zero_outs)
    sharded = jax.jit(shard_map(
        _body, mesh=mesh, in_specs=(PartitionSpec("core"),) * nin,
        out_specs=(PartitionSpec("core"),) * len(out_names),
        check_rep=False), keep_unused=True)

    def run(in_maps):
        per_core = [[np.asarray(m[name]) for name in in_names]
                    for m in in_maps]
        concat_in = [np.concatenate([per_core[c][i] for c in range(n_cores)],
                                    axis=0) for i in range(n_params)]
        concat_zeros = [np.zeros((n_cores * z.shape[0], *z.shape[1:]),
                                 z.dtype) for z in zero_outs]
        out_arrs = sharded(*concat_in, *concat_zeros)
        jax.block_until_ready(out_arrs)
        return [{name: np.asarray(out_arrs[i]).reshape(
                    n_cores, *out_avals[i].shape)[c]
                 for i, name in enumerate(out_names)}
                for c in range(n_cores)]

    def run_device(dev_in, fresh_zero_arrs):
        out_arrs = sharded(*dev_in, *fresh_zero_arrs)
        jax.block_until_ready(out_arrs)
        return out_arrs

    runner = dict(run=run, run_device=run_device, sharded=sharded,
                  in_names=in_names, out_names=out_names,
                  out_avals=out_avals, zero_outs=zero_outs,
                  n_params=n_params, mesh=mesh, n_cores=n_cores)
    return runner


_RUNNER_CACHE = {}


def _get_runner():
    if "r" not in _RUNNER_CACHE:
        _RUNNER_CACHE["r"] = _make_runner(get_nc(), N_CORES)
    return _RUNNER_CACHE["r"]


# ===========================================================================
# structured-input check + numpy fallback
# ===========================================================================
def _is_structured(edge_index, batch, group_ids, num_graphs, N, E):
    ng = int(np.asarray(num_graphs))
    if ng != N_GRAPHS or N != ng * N_ROI or E != ng * E_G:
        return False
    src = np.asarray(edge_index[0])
    dst = np.asarray(edge_index[1])
    idx = np.arange(N_ROI)
    s = np.repeat(idx, N_ROI)
    dd = np.tile(idx, N_ROI)
    off = (np.arange(ng) * N_ROI)[:, None]
    if not np.array_equal(src.reshape(ng, E_G), s[None, :] + off):
        return False
    if not np.array_equal(dst.reshape(ng, E_G), dd[None, :] + off):
        return False
    if not np.array_equal(np.asarray(batch),
                          np.repeat(np.arange(ng), N_ROI)):
        return False
    gi = np.asarray(group_ids)
    if not np.array_equal(gi, np.tile(gi[:N_ROI], ng)):
        return False
    return True


def _numpy_fallback(x, edge_attr, emb, enc_W, enc_b, bn_g, bn_b,
                    gine_We, gine_be, gine_W1, gine_b1, gine_W2, gine_b2,
                    gat_Wl, gat_bl, gat_Wr, gat_br, gat_att, gat_We,
                    gat_bias, pool_W1, pool_b1, pool_w2, lin1_W, lin1_b,
                    lin2_W, lin2_b, edge_index, batch, group_ids,
                    num_graphs):
    f32 = np.float32
    x = np.asarray(x, f32)
    edge_attr = np.asarray(edge_attr, f32)
    src = np.asarray(edge_index[0]).astype(np.int64)
    dst = np.asarray(edge_index[1]).astype(np.int64)
    batch = np.asarray(batch).astype(np.int64)
    ng = int(np.asarray(num_graphs))
    N = x.shape[0]
    h = np.concatenate([x, np.asarray(emb, f32)[np.asarray(group_ids)]], 1)
    h = h @ np.asarray(enc_W, f32) + np.asarray(enc_b, f32)
    h = np.maximum(h, 0)
    mu = h.mean(0)
    var = h.var(0)
    h = (h - mu) / np.sqrt(var + 1e-5) * np.asarray(bn_g, f32) + \
        np.asarray(bn_b, f32)
    e = edge_attr @ np.asarray(gine_We, f32) + np.asarray(gine_be, f32)
    msg = np.maximum(h[src] + e, 0)
    agg = np.zeros_like(h)
    np.add.at(agg, dst, msg)
    h = h + agg
    h = np.maximum(h @ np.asarray(gine_W1, f32) +
                   np.asarray(gine_b1, f32), 0)
    h = h @ np.asarray(gine_W2, f32) + np.asarray(gine_b2, f32)
    h = np.maximum(h, 0)
    for l in range(2):
        xl = h @ np.asarray(gat_Wl, f32)[l] + np.asarray(gat_bl, f32)[l]
        xr = h @ np.asarray(gat_Wr, f32)[l] + np.asarray(gat_br, f32)[l]
        z = xl[src] + xr[dst] + edge_attr @ np.asarray(gat_We, f32)[l]
        z = np.where(z > 0, z, 0.2 * z)
        sc = z @ np.asarray(gat_att, f32)[l]
        m = np.full(N, -np.inf, f32)
        np.maximum.at(m, dst, sc)
        ex = np.exp(sc - m[dst])
        ssum = np.zeros(N, f32)
        np.add.at(ssum, dst, ex)
        alpha = ex / (ssum[dst] + np.float32(1e-16))
        acc = np.zeros_like(h)
        np.add.at(acc, dst, xl[src] * alpha[:, None])
        h = np.maximum(acc + np.asarray(gat_bias, f32)[l], 0)
    sc = np.tanh(h @ np.asarray(pool_W1, f32) + np.asarray(pool_b1, f32))
    sc = sc @ np.asarray(pool_w2, f32)
    ex = np.exp(sc - sc.max())
    w = ex / ex.sum()
    hw = h * w[:, None]
    pooled = np.zeros((ng, HID), f32)
    np.add.at(pooled, batch, hw)
    o = np.maximum(pooled @ np.asarray(lin1_W, f32) +
                   np.asarray(lin1_b, f32), 0)
    return (o @ np.asarray(lin2_W, f32) + np.asarray(lin2_b, f32)).astype(f32)


def kernel(x, edge_attr, emb, enc_W, enc_b, bn_g, bn_b,
           gine_We, gine_be, gine_W1, gine_b1, gine_W2, gine_b2,
           gat_Wl, gat_bl, gat_Wr, gat_br, gat_att, gat_We, gat_bias,
           pool_W1, pool_b1, pool_w2, lin1_W, lin1_b, lin2_W, lin2_b,
           edge_index, batch, group_ids, num_graphs):
    N = np.asarray(x).shape[0]
    E = np.asarray(edge_attr).shape[0]
    if not _is_structured(edge_index, batch, group_ids, num_graphs, N, E):
        return _numpy_fallback(
            x, edge_attr, emb, enc_W, enc_b, bn_g, bn_b, gine_We, gine_be,
            gine_W1, gine_b1, gine_W2, gine_b2, gat_Wl, gat_bl, gat_Wr,
            gat_br, gat_att, gat_We, gat_bias, pool_W1, pool_b1, pool_w2,
            lin1_W, lin1_b, lin2_W, lin2_b, edge_index, batch, group_ids,
            num_graphs)
    in_maps = host_prep(x, edge_attr, emb, enc_W, enc_b, bn_g, bn_b,
                        gine_We, gine_be, gine_W1, gine_b1, gine_W2,
                        gine_b2, gat_Wl, gat_bl, gat_Wr, gat_br, gat_att,
                        gat_We, gat_bias, pool_W1, pool_b1, pool_w2,
                        lin1_W, lin1_b, lin2_W, lin2_b, group_ids)
    runner = _get_runner()
    results = runner["run"](in_maps)
    return assemble_out(results)



# revision 38
# speedup vs baseline: 1.9965x; 1.0502x over previous
"""BrainNetGAT Bass/Tile kernel for 8 Trainium2 NeuronCores.

Graph-level data parallelism: 16 graphs per core, processed as 8 pairs with
two concurrent PE column-tiled streams. Edge message passing is dense
augmented matmuls over each graph's 116x116 edge grid, with edges in
DST-MAJOR order (edge = dst*116 + src):
  moving tile T = [one-hot src-index (116); ea^T (5); ones (2)]  [123, 13456]
  stationary   = [Xsrc (116); We (5); K (2)]                     [123, 64]
so one matmul yields ea@We + x_src[src] for every edge; a second matmul with
a per-chunk sliced broadcast-AP identity adds x_dst[dst]. Dst-major makes
the GINE segment-sum a contiguous-axis DVE reduce and makes the attention
score matrix load back from DRAM directly as [dst, src] with no transposes.
GAT attention scores are computed by a 4-stream block-diagonal att matmul
(bf16), bounced through DRAM. GINE relu runs on the Vector engine to keep
the Scalar/ACT engine for the GAT leaky-relu. Most small DMAs are issued
from the otherwise-idle Sync engine. BatchNorm stats and the global pooling
softmax sum use two small AllReduces.
"""
import contextlib

import numpy as np
import ml_dtypes

import concourse.bacc as bacc
import concourse.mybir as mybir
import concourse.tile as tile

F32 = mybir.dt.float32
BF16 = mybir.dt.bfloat16
AF = mybir.ActivationFunctionType
OP = mybir.AluOpType
AX = mybir.AxisListType

N_ROI = 116
HID = 64
EDIM = 5
N_GRAPHS = 128
N_CORES = 8
G_C = N_GRAPHS // N_CORES          # 16 graphs per core
PAIRS = G_C // 2
N_C = G_C * N_ROI                  # 1856 nodes per core
E_G = N_ROI * N_ROI                # 13456 edges per graph
E_C = G_C * E_G
N_TOTAL = N_GRAPHS * N_ROI         # 14848
# shared moving tile rows: 0:116 src-onehot, 116:121 eaA, 121:126 eaB,
# 126:128 ones (for the per-graph bf16-centering K rows)
EA_A = N_ROI                       # 116
EA_B = N_ROI + EDIM                # 121
ONES_R = N_ROI + 2 * EDIM          # 126
KAUG = ONES_R + 2                  # 128
KGINE = ONES_R                     # 126 rows for the GINE matmuls
CH = 4 * N_ROI                     # 464-col edge chunk (4 dst blocks)
NCH = E_G // CH                    # 29
GRP = 3                            # z-chunks per psum group
NODE_CH = 4 * N_ROI                # 464 node cols (4 graphs)


def build_nc(n_cores=N_CORES):
    nc = bacc.Bacc()
    d = {}

    def inp(name, shape, dt):
        d[name] = nc.declare_dram_parameter(name, list(shape), dt,
                                            isOutput=False)

    inp("xT", (N_ROI, N_C), BF16)
    inp("eaT", (EDIM, E_C), BF16)
    inp("embT", (16, N_ROI), BF16)
    inp("I116", (N_ROI, N_ROI), BF16)
    inp("encW1", (N_ROI, HID), BF16)
    inp("encW2", (16, HID), BF16)
    inp("enc_b", (HID, 1), F32)
    inp("bn_g", (HID, 1), F32)
    inp("bn_b", (HID, 1), F32)
    inp("gineWeZA", (2 * EDIM, HID), BF16)
    inp("gineWeZB", (2 * EDIM, HID), BF16)
    inp("gine_be", (HID, 1), F32)
    inp("gineW1", (HID, HID), F32)
    inp("gine_b1", (HID, 1), F32)
    inp("gineW2", (HID, HID), F32)
    inp("gine_b2", (HID, 1), F32)
    for l in range(2):
        inp(f"gatWl{l}", (HID, HID), F32)
        inp(f"gat_bl{l}", (HID, 1), F32)
        inp(f"gatWr{l}", (HID, HID), F32)
        inp(f"gat_br{l}", (HID, 1), F32)
        inp(f"gat_blb{l}", (HID, 1), F32)
        inp(f"gatWeZA{l}", (2 * EDIM, HID), BF16)
        inp(f"gatWeZB{l}", (2 * EDIM, HID), BF16)
        inp(f"att{l}", (128, 1), F32)
    inp("poolW1", (HID, HID), F32)
    inp("pool_b1", (HID, 1), F32)
    inp("pool_w2", (HID, 1), BF16)
    inp("lin1W", (HID, N_ROI), BF16)
    inp("lin1_b", (N_ROI, 1), F32)
    inp("lin2W", (N_ROI, 2), BF16)
    inp("lin2_b", (2, 1), F32)
    inp("ones2", (2, E_G), BF16)
    outT = nc.declare_dram_parameter("outT", [2, G_C], F32, isOutput=True)

    with tile.TileContext(nc) as tc:
        _body(nc, tc, d, outT, n_cores)
    nc.finalize()
    return nc


def _body(nc, tc, d, outT, n_cores=N_CORES):
    ctx = contextlib.ExitStack()
    with ctx:
        wpool = ctx.enter_context(tc.tile_pool(name="weights", bufs=1))
        state = ctx.enter_context(tc.tile_pool(name="state", bufs=1))
        tpool = ctx.enter_context(tc.tile_pool(name="tmoving", bufs=1))
        upool = ctx.enter_context(tc.tile_pool(name="u", bufs=1))
        spool = ctx.enter_context(tc.tile_pool(name="smalls", bufs=3))
        station = ctx.enter_context(tc.tile_pool(name="station", bufs=2))
        psA = ctx.enter_context(tc.tile_pool(name="psA", bufs=2, space="PSUM"))
        psB = ctx.enter_context(tc.tile_pool(name="psB", bufs=2, space="PSUM"))
        dpool = ctx.enter_context(tc.tile_pool(name="dram", bufs=2,
                                               space="DRAM"))
        bn_in = dpool.tile([HID, 2], F32, tag="bn_in", bufs=1)
        bn_out = dpool.tile([HID, 2], F32, tag="bn_out", bufs=1)
        s_in = dpool.tile([1, 1], F32, tag="s_in", bufs=1)
        s_out = dpool.tile([1, 1], F32, tag="s_out", bufs=1)
        psc_row = dpool.tile([1, N_C], F32, tag="psc_row", bufs=1)
        scAB = dpool.tile([2, E_G], F32, tag="scAB")

        # ---------- weights / constants ----------
        W = {}
        for name, h in d.items():
            if name in ("eaT", "ones2"):
                continue
            W[name] = wpool.tile(list(h.shape), h.dtype, tag=name, name=name)
            nc.gpsimd.dma_start(out=W[name][:], in_=h[:])

        ident = wpool.tile([128, 128], BF16, tag="ident")
        nc.vector.memset(ident[:], 0.0)
        nc.gpsimd.affine_select(out=ident[:], in_=ident[:],
                                compare_op=OP.not_equal, fill=1.0, base=0,
                                pattern=[[-1, 128]], channel_multiplier=1)
        identF = wpool.tile([128, 128], F32, tag="identF")
        nc.vector.memset(identF[:], 0.0)
        nc.gpsimd.affine_select(out=identF[:], in_=identF[:],
                                compare_op=OP.not_equal, fill=1.0, base=0,
                                pattern=[[-1, 128]], channel_multiplier=1)
        alpha02 = wpool.tile([128, 1], F32, tag="alpha02")
        nc.vector.memset(alpha02[:], 0.2)
        eps6 = wpool.tile([N_ROI, 1], F32, tag="eps6")
        nc.vector.memset(eps6[:], 1e-6)
        eps5 = wpool.tile([HID, 1], F32, tag="eps5")
        nc.vector.memset(eps5[:], 1e-5)
        ones116 = wpool.tile([N_ROI, 1], BF16, tag="ones116")
        nc.vector.memset(ones116[:], 1.0)

        attd = []
        for l in range(2):
            t = wpool.tile([128, 32], BF16, tag=f"attd{l}")
            nc.vector.memset(t[:], 0.0)
            nc.vector.tensor_copy(t[0:HID, 0:1], W[f"att{l}"][0:HID, :])
            nc.vector.tensor_copy(t[HID:128, 1:2], W[f"att{l}"][HID:128, :])
            attd.append(t)

        Bbe = wpool.tile([HID, 1], F32, tag="Bbe")

        # one shared moving tile (dst-major edges) for BOTH graphs of a pair:
        # rows 0:116 = s-onehot, 116:121 = eaA, 121:126 = eaB, 126:128 = ones
        T = tpool.tile([KAUG, E_G], BF16, tag="T")
        sind_src = W["I116"][:, :].unsqueeze(1).broadcast_to(
            [N_ROI, N_ROI, N_ROI])
        nc.vector.tensor_copy(
            T[0:N_ROI, :].rearrange("p (dd s) -> p dd s", s=N_ROI),
            sind_src)
        nc.gpsimd.dma_start(out=T[ONES_R:KAUG, :], in_=d["ones2"][:])

        hT = state.tile([HID, N_C], F32, tag="hT")
        hbeT = state.tile([HID, N_C], BF16, tag="hbeT")
        h0T = state.tile([HID, N_C], BF16, tag="h0T")
        tanT = state.tile([HID, N_C], BF16, tag="tanT")
        eMat = state.tile([N_ROI, G_C], F32, tag="eMat")
        hnew = []
        for g in range(G_C):
            hn_t = state.tile([N_ROI, HID], BF16, tag=f"hnew{g}",
                              name=f"hnew{g}")
            hnew.append(hn_t)

        # ---------- phase A: encoder + BN ----------
        emb_b = W["embT"][:, :].unsqueeze(1).broadcast_to([16, 4, N_ROI])
        for k in range(N_C // NODE_CH):
            sl = slice(k * NODE_CH, (k + 1) * NODE_CH)
            ps = psB.tile([HID, NODE_CH], F32, tag="mm_ps")
            nc.tensor.matmul(ps[:], W["encW1"][:], W["xT"][:, sl],
                             start=True, stop=False)
            nc.tensor.matmul(ps[:], W["encW2"][:], emb_b,
                             start=False, stop=True)
            nc.scalar.activation(h0T[:, sl], ps[:], AF.Relu,
                                 bias=W["enc_b"][:])

        st = spool.tile([HID, 2], F32, tag="bn_st")
        sq = upool.tile([HID, N_C], BF16, tag="sq")
        nc.vector.tensor_reduce(st[:, 0:1], h0T[:, :], axis=AX.X, op=OP.add)
        nc.vector.tensor_tensor(sq[:], h0T[:], h0T[:], op=OP.mult)
        nc.vector.tensor_reduce(st[:, 1:2], sq[:, :], axis=AX.X, op=OP.add)
        nc.gpsimd.dma_start(out=bn_in[:], in_=st[:])
        nc.gpsimd.collective_compute(
            "AllReduce", OP.add, replica_groups=[list(range(n_cores))],
            ins=[bn_in[:]], outs=[bn_out[:]])
        stg = spool.tile([HID, 2], F32, tag="bn_stg")
        nc.gpsimd.dma_start(out=stg[:], in_=bn_out[:])

        mu = spool.tile([HID, 1], F32, tag="mu")
        var = spool.tile([HID, 1], F32, tag="var")
        sd = spool.tile([HID, 1], F32, tag="sd")
        A = spool.tile([HID, 1], F32, tag="A")
        B = spool.tile([HID, 1], F32, tag="B")
        t3 = spool.tile([HID, 1], F32, tag="t3")
        nc.vector.tensor_scalar_mul(mu[:], stg[:, 0:1], 1.0 / N_TOTAL)
        nc.vector.tensor_scalar_mul(var[:], stg[:, 1:2], 1.0 / N_TOTAL)
        nc.vector.tensor_tensor(t3[:], mu[:], mu[:], op=OP.mult)
        nc.vector.tensor_tensor(var[:], var[:], t3[:], op=OP.subtract)
        nc.scalar.activation(sd[:], var[:], AF.Sqrt, bias=eps5[:])
        nc.vector.reciprocal(sd[:], sd[:])
        nc.vector.tensor_tensor(A[:], sd[:], W["bn_g"][:], op=OP.mult)
        nc.vector.tensor_tensor(t3[:], mu[:], A[:], op=OP.mult)
        nc.vector.tensor_tensor(B[:], W["bn_b"][:], t3[:], op=OP.subtract)
        nc.vector.tensor_tensor(Bbe[:], B[:], W["gine_be"][:], op=OP.add)
        nc.scalar.activation(hT[:, :], h0T[:, :], AF.Identity,
                             bias=B[:], scale=A[:])
        nc.scalar.activation(hbeT[:, :], h0T[:, :], AF.Identity,
                             bias=Bbe[:], scale=A[:])

        # ---------- phase B: software-pipelined pairs ----------
        def gine_head(p):
            """ea load + GINE stationaries + edge matmuls + relu/segsum."""
            gA, gB = 2 * p, 2 * p + 1
            cA = slice(gA * N_ROI, (gA + 1) * N_ROI)
            cB = slice(gB * N_ROI, (gB + 1) * N_ROI)
            nc.sync.dma_start(out=T[EA_A:EA_A + EDIM, :],
                              in_=d["eaT"][:, gA * E_G:(gA + 1) * E_G])
            nc.sync.dma_start(out=T[EA_B:EA_B + EDIM, :],
                              in_=d["eaT"][:, gB * E_G:(gB + 1) * E_G])
            SA = station.tile([KGINE, HID], BF16, tag="SA")
            SB = station.tile([KGINE, HID], BF16, tag="SB")
            nc.gpsimd.dma_start(out=SA[EA_A:KGINE, :],
                                in_=d["gineWeZA"][:])
            nc.gpsimd.dma_start(out=SB[EA_A:KGINE, :],
                                in_=d["gineWeZB"][:])
            for (S, cg) in ((SA, cA), (SB, cB)):
                trp = psB.tile([N_ROI, HID], BF16, tag="mm_ps")
                nc.tensor.transpose(trp[:], hbeT[:, cg], ident[0:HID, 0:HID])
                nc.vector.tensor_copy(S[0:N_ROI, :], trp[:])

            agg = spool.tile([128, N_ROI], F32, tag="agg",
                             name=f"agg{p}")
            for g0 in range(0, NCH, GRP):
                ng = min(GRP, NCH - g0)
                zps = psA.tile([128, GRP * 512], F32, tag="zps")
                for j in range(ng):
                    ch = slice((g0 + j) * CH, (g0 + j + 1) * CH)
                    pj = slice(j * 512, j * 512 + CH)
                    nc.tensor.matmul(zps[0:HID, pj], SA,
                                     T[0:KGINE, ch],
                                     start=True, stop=True)
                for j in range(ng):
                    ch = slice((g0 + j) * CH, (g0 + j + 1) * CH)
                    pj = slice(j * 512, j * 512 + CH)
                    nc.tensor.matmul(zps[HID:128, pj], SB,
                                     T[0:KGINE, ch],
                                     start=True, stop=True,
                                     tile_position=(0, 64))
                # relu into a small scratch, then contiguous segment-sum
                u1g = spool.tile([128, GRP * CH], BF16, tag="u1g")
                srcv = zps[:, :].rearrange("p (g c) -> p g c",
                                           c=512)[:, 0:ng, 0:CH]
                dstv = u1g[:, 0:ng * CH].rearrange("p (g c) -> p g c", c=CH)
                nc.vector.tensor_scalar_max(dstv, srcv, 0.0)
                rv = u1g[:, 0:ng * CH].rearrange("p (dd s) -> p dd s",
                                                 s=N_ROI)
                nc.vector.tensor_reduce(agg[:, 4 * g0:4 * (g0 + ng)], rv,
                                        axis=AX.X, op=OP.add)
            return agg

        def gine_rest(p, agg):
            gA, gB = 2 * p, 2 * p + 1
            cA = slice(gA * N_ROI, (gA + 1) * N_ROI)
            cB = slice(gB * N_ROI, (gB + 1) * N_ROI)
            cP = slice(gA * N_ROI, (gB + 1) * N_ROI)
            nc.vector.tensor_tensor(hT[:, cA], hT[:, cA], agg[0:HID, :],
                                    op=OP.add)
            aggB = spool.tile([HID, N_ROI], F32, tag="aggB")
            nc.sync.dma_start(out=aggB[:], in_=agg[HID:128, :])
            nc.vector.tensor_tensor(hT[:, cB], hT[:, cB], aggB[:],
                                    op=OP.add)
            mp1 = psB.tile([HID, 2 * N_ROI], F32, tag="mm_ps")
            nc.tensor.matmul(mp1[:], W["gineW1"][:], hT[:, cP],
                             start=True, stop=True)
            mt = spool.tile([HID, 2 * N_ROI], F32, tag="mt")
            nc.scalar.activation(mt[:], mp1[:], AF.Relu, bias=W["gine_b1"][:])
            mp2 = psB.tile([HID, 2 * N_ROI], F32, tag="mm_ps")
            nc.tensor.matmul(mp2[:], W["gineW2"][:], mt[:],
                             start=True, stop=True)
            nc.scalar.activation(hT[:, cP], mp2[:], AF.Relu,
                                 bias=W["gine_b2"][:])

        def gat_edges(p, l):
            """xl/xr projections, stationaries, edge matmuls + prelu,
            attention-score matmuls with PSUM-direct extraction."""
            gA, gB = 2 * p, 2 * p + 1
            cP = slice(gA * N_ROI, (gB + 1) * N_ROI)
            lA = slice(0, N_ROI)
            lB = slice(N_ROI, 2 * N_ROI)
            xps = psB.tile([HID, 2 * N_ROI], F32, tag="mm_ps")
            nc.tensor.matmul(xps[:], W[f"gatWl{l}"][:], hT[:, cP],
                             start=True, stop=True)
            xlT = spool.tile([HID, 2 * N_ROI], F32, tag="xlT")
            xlbT = spool.tile([HID, 2 * N_ROI], F32, tag="xlbT")
            nc.scalar.activation(xlT[:], xps[:], AF.Identity,
                                 bias=W[f"gat_bl{l}"][:])
            nc.scalar.activation(xlbT[:], xps[:], AF.Identity,
                                 bias=W[f"gat_blb{l}"][:])
            xps2 = psB.tile([HID, 2 * N_ROI], F32, tag="mm_ps")
            nc.tensor.matmul(xps2[:], W[f"gatWr{l}"][:], hT[:, cP],
                             start=True, stop=True)
            xrT = spool.tile([HID, 2 * N_ROI], F32, tag="xrT")
            nc.scalar.activation(xrT[:], xps2[:], AF.Identity,
                                 bias=W[f"gat_br{l}"][:])

            SA2 = station.tile([KAUG, HID], BF16, tag="SA2")
            SB2 = station.tile([KAUG, HID], BF16, tag="SB2")
            XrA = station.tile([N_ROI, HID], BF16, tag="XrA")
            XrB = station.tile([N_ROI, HID], BF16, tag="XrB")
            XlbA = station.tile([N_ROI, HID], F32, tag="XlbA")
            XlbB = station.tile([N_ROI, HID], F32, tag="XlbB")
            nc.sync.dma_start(out=SA2[EA_A:ONES_R, :],
                              in_=d[f"gatWeZA{l}"][:])
            nc.sync.dma_start(out=SB2[EA_A:ONES_R, :],
                              in_=d[f"gatWeZB{l}"][:])
            for (S, Xr, Xlb, lg) in ((SA2, XrA, XlbA, lA),
                                     (SB2, XrB, XlbB, lB)):
                # per-graph centering of xl/xr; exact offset via 2 rows
                mL = spool.tile([HID, 1], F32, tag="mL")
                mR = spool.tile([HID, 1], F32, tag="mR")
                nc.vector.tensor_reduce(mL[:], xlT[:, lg], axis=AX.X,
                                        op=OP.add)
                nc.vector.tensor_scalar_mul(mL[:], mL[:], 1.0 / N_ROI)
                nc.vector.tensor_reduce(mR[:], xrT[:, lg], axis=AX.X,
                                        op=OP.add)
                nc.vector.tensor_scalar_mul(mR[:], mR[:], 1.0 / N_ROI)
                xlc = spool.tile([HID, N_ROI], BF16, tag="xlc")
                xrc = spool.tile([HID, N_ROI], BF16, tag="xrc")
                nc.vector.tensor_scalar(xlc[:], xlT[:, lg], mL[:],
                                        scalar2=None,
                                        op0=OP.subtract)
                nc.vector.tensor_scalar(xrc[:], xrT[:, lg], mR[:],
                                        scalar2=None,
                                        op0=OP.subtract)
                Ksum = spool.tile([HID, 1], F32, tag="Ksum")
                nc.vector.tensor_tensor(Ksum[:], mL[:], mR[:], op=OP.add)
                K2 = spool.tile([HID, 2], BF16, tag="K2")
                nc.vector.tensor_copy(K2[:, 0:1], Ksum[:])
                Klo = spool.tile([HID, 1], F32, tag="Klo")
                nc.vector.tensor_tensor(Klo[:], Ksum[:], K2[:, 0:1],
                                        op=OP.subtract)
                nc.vector.tensor_copy(K2[:, 1:2], Klo[:])
                k2p = psB.tile([2, HID], BF16, tag="mm_ps")
                nc.tensor.transpose(k2p[:], K2[:], ident[0:HID, 0:HID])
                k2s = spool.tile([2, HID], BF16, tag="k2s")
                nc.vector.tensor_copy(k2s[:], k2p[:])
                nc.sync.dma_start(out=S[ONES_R:KAUG, :],
                                  in_=k2s[:])
                t1p = psB.tile([N_ROI, HID], BF16, tag="mm_ps")
                nc.tensor.transpose(t1p[:], xlc[:], ident[0:HID, 0:HID])
                nc.vector.tensor_copy(S[0:N_ROI, :], t1p[:])
                t2p = psB.tile([N_ROI, HID], BF16, tag="mm_ps")
                nc.tensor.transpose(t2p[:], xrc[:], ident[0:HID, 0:HID])
                nc.vector.tensor_copy(Xr[:], t2p[:])
                t3p = psB.tile([N_ROI, HID], F32, tag="mm_ps")
                nc.tensor.transpose(t3p[:], xlbT[:, lg],
                                    identF[0:HID, 0:HID])
                nc.vector.tensor_copy(Xlb[:], t3p[:])

            u2 = upool.tile([128, E_G], BF16, tag="u")
            for g0 in range(0, NCH, GRP):
                ng = min(GRP, NCH - g0)
                zps = psA.tile([128, GRP * 512], F32, tag="zps")
                for j in range(ng):
                    ch = slice((g0 + j) * CH, (g0 + j + 1) * CH)
                    pj = slice(j * 512, j * 512 + CH)
                    nc.tensor.matmul(zps[0:HID, pj], SA2, T[:, ch],
                                     start=True, stop=False)
                for j in range(ng):
                    c4 = slice(4 * (g0 + j), 4 * (g0 + j) + 4)
                    pj = slice(j * 512, j * 512 + CH)
                    dind = W["I116"][:, c4].unsqueeze(2).broadcast_to(
                        [N_ROI, 4, N_ROI])
                    nc.tensor.matmul(zps[0:HID, pj], XrA, dind,
                                     start=False, stop=True)
                for j in range(ng):
                    ch = slice((g0 + j) * CH, (g0 + j + 1) * CH)
                    pj = slice(j * 512, j * 512 + CH)
                    nc.tensor.matmul(zps[HID:128, pj], SB2, T[:, ch],
                                     start=True, stop=False,
                                     tile_position=(0, 64))
                for j in range(ng):
                    c4 = slice(4 * (g0 + j), 4 * (g0 + j) + 4)
                    pj = slice(j * 512, j * 512 + CH)
                    dind = W["I116"][:, c4].unsqueeze(2).broadcast_to(
                        [N_ROI, 4, N_ROI])
                    nc.tensor.matmul(zps[HID:128, pj], XrB, dind,
                                     start=False, stop=True,
                                     tile_position=(0, 64))
                src = zps[:, :].rearrange("p (g c) -> p g c",
                                          c=512)[:, 0:ng, 0:CH]
                dst = u2[:, g0 * CH:(g0 + ng) * CH].rearrange(
                    "p (g c) -> p g c", c=CH)
                nc.scalar.activation(dst, src, AF.Prelu,
                                     alpha=alpha02[:])

            # attention scores: 4 col-tiled streams -> rows 0,32,64,96
            scAB_c = scAB[:, :].rearrange("r (cc c) -> r cc c", c=CH)
            for base0 in range(0, NCH, 12):
                n = min(12, NCH - base0)
                npad = (n + 3) // 4 * 4
                nslot = npad // 4
                sps = psA.tile([128, GRP * 512], F32, tag="zps")
                for idx in range(npad):
                    c = base0 + min(idx, n - 1)
                    k, j = idx % 4, idx // 4
                    nc.tensor.matmul(
                        sps[32 * k:32 * k + 32, j * 512:j * 512 + CH],
                        attd[l], u2[:, c * CH:(c + 1) * CH],
                        start=True, stop=True,
                        tile_position=(0, 32 * k))
                scc = spool.tile([128, GRP * 512], F32, tag="scc", bufs=2)
                ssrc = sps[:, :].rearrange("p (j c) -> p j c",
                                           c=512)[:, 0:nslot, 0:CH]
                sdst = scc[:, 0:nslot * CH].rearrange(
                    "p (j c) -> p j c", c=CH)
                nc.scalar.activation(sdst, ssrc, AF.Copy)
                for k in range(4):
                    nk = len([i for i in range(n) if i % 4 == k])
                    if nk == 0:
                        continue
                    src3 = scc[32 * k:32 * k + 2, 0:nslot * CH].rearrange(
                        "p (j c) -> p j c", c=CH)[:, 0:nk, :]
                    dst3 = scAB_c[:, base0 + k:base0 + n:4, :]
                    eng = nc.sync if k % 2 == 0 else nc.gpsimd
                    eng.dma_start(out=dst3, in_=src3)
            return XlbA, XlbB

        def gat_tail(p, l, XlbA, XlbB):
            """per-graph softmax + alpha-weighted aggregation."""
            gA, gB = 2 * p, 2 * p + 1
            cA = slice(gA * N_ROI, (gA + 1) * N_ROI)
            cB = slice(gB * N_ROI, (gB + 1) * N_ROI)
            # dst-major: scAB rows reload directly as [dst, src]
            scAB_m = scAB[:, :].rearrange("r (dd s) -> r dd s",
                                          s=N_ROI)
            for (g, rr, Xlb, cg) in ((gA, 0, XlbA, cA),
                                     (gB, 1, XlbB, cB)):
                epT = spool.tile([N_ROI, N_ROI], F32, tag="epT")
                nc.sync.dma_start(out=epT[:], in_=scAB_m[rr])
                mrow = spool.tile([N_ROI, 1], F32, tag="mrow")
                nc.vector.tensor_reduce(mrow[:], epT[:, :], axis=AX.X,
                                        op=OP.max)
                mneg = spool.tile([N_ROI, 1], F32, tag="mneg")
                nc.vector.tensor_scalar_mul(mneg[:], mrow[:], -1.0)
                ehT = spool.tile([N_ROI, N_ROI], F32, tag="ehT")
                nc.scalar.activation(ehT[:], epT[:], AF.Exp,
                                     bias=mneg[:])
                srow = spool.tile([N_ROI, 1], F32, tag="srow")
                nc.vector.tensor_reduce(srow[:], ehT[:, :], axis=AX.X,
                                        op=OP.add)
                rrow = spool.tile([N_ROI, 1], F32, tag="rrow")
                nc.vector.reciprocal(rrow[:], srow[:])
                adT = spool.tile([N_ROI, N_ROI], F32, tag="adT")
                nc.vector.tensor_scalar_mul(adT[:], ehT[:], rrow[:])
                as_ps = psB.tile([N_ROI, N_ROI], F32, tag="mm_ps")
                nc.tensor.transpose(as_ps[:], adT[:],
                                    identF[0:N_ROI, 0:N_ROI])
                aS = spool.tile([N_ROI, N_ROI], F32, tag="aS")
                nc.vector.tensor_copy(aS[:], as_ps[:])
                am = psB.tile([N_ROI, HID], F32, tag="mm_ps")
                nc.tensor.matmul(am[:], aS[:], Xlb[:],
                                 start=True, stop=True)
                hnF = spool.tile([N_ROI, HID], F32, tag="hnF")
                nc.scalar.activation(hnF[:], am[:], AF.Relu)
                nc.vector.tensor_copy(hnew[g][:], hnF[:])
                htp = psB.tile([HID, N_ROI], F32, tag="mm_ps")
                nc.tensor.transpose(htp[:], hnF[:],
                                    identF[0:N_ROI, 0:N_ROI])
                nc.vector.tensor_copy(hT[:, cg], htp[:])

        def pool_scores(p):
            gA, gB = 2 * p, 2 * p + 1
            cP = slice(gA * N_ROI, (gB + 1) * N_ROI)
            pps = psB.tile([HID, 2 * N_ROI], F32, tag="mm_ps")
            nc.tensor.matmul(pps[:], W["poolW1"][:], hT[:, cP],
                             start=True, stop=True)
            nc.scalar.activation(tanT[:, cP], pps[:], AF.Tanh,
                                 bias=W["pool_b1"][:])
            scp = psB.tile([1, 2 * N_ROI], F32, tag="mm_ps")
            nc.tensor.matmul(scp[:], W["pool_w2"][:], tanT[:, cP],
                             start=True, stop=True)
            scs = spool.tile([1, 2 * N_ROI], F32, tag="scs")
            nc.vector.tensor_copy(scs[:], scp[:])
            nc.sync.dma_start(
                out=psc_row[0, p * 2 * N_ROI:(p + 1) * 2 * N_ROI],
                in_=scs[:])

        # pipeline: pair p+1's GINE head fills the PE while pair p's
        # second GAT layer finishes its softmax/aggregation tail
        agg_next = gine_head(0)
        for p in range(PAIRS):
            gine_rest(p, agg_next)
            Xlb0 = gat_edges(p, 0)
            gat_tail(p, 0, *Xlb0)
            Xlb1 = gat_edges(p, 1)
            if p + 1 < PAIRS:
                agg_next = gine_head(p + 1)
            gat_tail(p, 1, *Xlb1)
            pool_scores(p)

        # ---------- phase C: pooling + head ----------
        nc.gpsimd.dma_start(
            out=eMat[:], in_=psc_row[0, :].rearrange("(g r) -> r g", r=N_ROI))
        eMb = spool.tile([N_ROI, G_C], BF16, tag="eMb")
        nc.scalar.activation(eMb[:], eMat[:], AF.Exp)
        ssum_ps = psB.tile([1, G_C], F32, tag="mm_ps")
        nc.tensor.matmul(ssum_ps[:], ones116[:], eMb[:],
                         start=True, stop=True)
        ssum = spool.tile([1, 1], F32, tag="ssum")
        nc.vector.tensor_reduce(ssum[:], ssum_ps[0:1, :], axis=AX.X,
                                op=OP.add)
        nc.gpsimd.dma_start(out=s_in[:], in_=ssum[:])
        nc.gpsimd.collective_compute(
            "AllReduce", OP.add, replica_groups=[list(range(n_cores))],
            ins=[s_in[:]], outs=[s_out[:]])
        sS64 = spool.tile([HID, 1], F32, tag="sS64")
        nc.gpsimd.dma_start(out=sS64[:], in_=s_out[:].broadcast_to([HID, 1]))
        nc.vector.reciprocal(sS64[:], sS64[:])

        pool_ps = psB.tile([HID, G_C], F32, tag="mm_ps")
        for g in range(G_C):
            nc.tensor.matmul(pool_ps[:, g:g + 1], hnew[g][:],
                             eMb[:, g:g + 1], start=True, stop=True)
        pooledT = spool.tile([HID, G_C], BF16, tag="pooledT")
        nc.scalar.activation(pooledT[:], pool_ps[:], AF.Identity,
                             scale=sS64[:])
        o1ps = psB.tile([N_ROI, G_C], F32, tag="mm_ps")
        nc.tensor.matmul(o1ps[:], W["lin1W"][:], pooledT[:],
                         start=True, stop=True)
        o1 = spool.tile([N_ROI, G_C], BF16, tag="o1")
        nc.scalar.activation(o1[:], o1ps[:], AF.Relu, bias=W["lin1_b"][:])
        o2ps = psB.tile([2, G_C], F32, tag="mm_ps")
        nc.tensor.matmul(o2ps[:], W["lin2W"][:], o1[:], start=True, stop=True)
        oflast = spool.tile([2, G_C], F32, tag="oflast")
        nc.scalar.activation(oflast[:], o2ps[:], AF.Identity,
                             bias=W["lin2_b"][:])
        nc.gpsimd.dma_start(out=outT[:], in_=oflast[:])


# ---------------------------------------------------------------------------
_NC_CACHE = {}


def get_nc():
    if "nc" not in _NC_CACHE:
        _NC_CACHE["nc"] = build_nc()
    return _NC_CACHE["nc"]


def host_prep(x, edge_attr, emb, enc_W, enc_b, bn_g, bn_b,
              gine_We, gine_be, gine_W1, gine_b1, gine_W2, gine_b2,
              gat_Wl, gat_bl, gat_Wr, gat_br, gat_att, gat_We, gat_bias,
              pool_W1, pool_b1, pool_w2, lin1_W, lin1_b, lin2_W, lin2_b,
              group_ids):
    bf = ml_dtypes.bfloat16
    f32 = np.float32

    def col(v):
        return np.ascontiguousarray(np.asarray(v, f32).reshape(-1, 1))

    base = {
        "embT": np.ascontiguousarray(
            np.asarray(emb, f32).T[:, np.asarray(group_ids[:N_ROI])]
        ).astype(bf),
        "I116": np.eye(N_ROI).astype(bf),
        "encW1": np.ascontiguousarray(np.asarray(enc_W, f32)[:N_ROI]
                                      ).astype(bf),
        "encW2": np.ascontiguousarray(np.asarray(enc_W, f32)[N_ROI:]
                                      ).astype(bf),
        "enc_b": col(enc_b), "bn_g": col(bn_g), "bn_b": col(bn_b),
        "gineWeZA": np.vstack([np.asarray(gine_We, f32),
                               np.zeros((EDIM, HID), f32)]).astype(bf),
        "gineWeZB": np.vstack([np.zeros((EDIM, HID), f32),
                               np.asarray(gine_We, f32)]).astype(bf),
        "gine_be": col(gine_be),
        "gineW1": np.asarray(gine_W1, f32),
        "gine_b1": col(gine_b1),
        "gineW2": np.asarray(gine_W2, f32),
        "gine_b2": col(gine_b2),
        "poolW1": np.asarray(pool_W1, f32),
        "pool_b1": col(pool_b1),
        "pool_w2": col(pool_w2).astype(bf),
        "lin1W": np.asarray(lin1_W, f32).astype(bf),
        "lin1_b": col(lin1_b),
        "lin2W": np.asarray(lin2_W, f32).astype(bf),
        "lin2_b": col(lin2_b),
        "ones2": np.ones((2, E_G)).astype(bf),
    }
    for l in range(2):
        base[f"gatWl{l}"] = np.asarray(gat_Wl[l], f32)
        base[f"gat_bl{l}"] = col(gat_bl[l])
        base[f"gatWr{l}"] = np.asarray(gat_Wr[l], f32)
        base[f"gat_br{l}"] = col(gat_br[l])
        base[f"gat_blb{l}"] = col(np.asarray(gat_bl[l], f32) +
                                  np.asarray(gat_bias[l], f32))
        base[f"gatWeZA{l}"] = np.vstack([np.asarray(gat_We[l], f32),
                                         np.zeros((EDIM, HID), f32)
                                         ]).astype(bf)
        base[f"gatWeZB{l}"] = np.vstack([np.zeros((EDIM, HID), f32),
                                         np.asarray(gat_We[l], f32)
                                         ]).astype(bf)
        base[f"att{l}"] = col(np.concatenate([np.asarray(gat_att[l], f32), np.asarray(gat_att[l], f32)]))

    xT = np.ascontiguousarray(np.asarray(x, f32).T).astype(bf)
    # reorder edges to dst-major within each graph: edge = dst*116 + src
    ea4 = np.asarray(edge_attr, f32).reshape(N_GRAPHS, N_ROI, N_ROI, EDIM)
    ea_dm = np.ascontiguousarray(ea4.transpose(0, 2, 1, 3)).reshape(-1, EDIM)
    eaT = np.ascontiguousarray(ea_dm.T).astype(bf)

    in_maps = []
    for c in range(N_CORES):
        m = dict(base)
        m["xT"] = np.ascontiguousarray(xT[:, c * N_C:(c + 1) * N_C])
        m["eaT"] = np.ascontiguousarray(eaT[:, c * E_C:(c + 1) * E_C])
        in_maps.append(m)
    return in_maps


def assemble_out(results):
    return np.concatenate([np.asarray(r["outT"], np.float32).T
                           for r in results], axis=0)


# ===========================================================================
# SPMD runner (replicates bass2jax.run_bass_via_pjrt, but reusable + timeable)
# ===========================================================================
def _make_runner(nc, n_cores=N_CORES):
    import jax
    import jax.numpy as jnp
    from jax.sharding import Mesh, PartitionSpec
    from jax.experimental.shard_map import shard_map
    from concourse import bass2jax
    from concourse.bass2jax import _bass_exec_p, partition_id_tensor
    import concourse.mybir as mb

    bass2jax.install_neuronx_cc_hook()
    partition_name = (nc.partition_id_tensor.name
                      if nc.partition_id_tensor else None)
    in_names, out_names, out_avals, zero_outs = [], [], [], []
    for alloc in nc.m.functions[0].allocations:
        if not isinstance(alloc, mb.MemoryLocationSet):
            continue
        name = alloc.memorylocations[0].name
        if alloc.kind == "ExternalInput":
            if name != partition_name:
                in_names.append(name)
        elif alloc.kind == "ExternalOutput":
            out_names.append(name)
            shape = tuple(alloc.tensor_shape)
            dtype = mb.dt.np(alloc.dtype)
            out_avals.append(jax.core.ShapedArray(shape, dtype))
            zero_outs.append(np.zeros(shape, dtype))
    n_params = len(in_names)
    all_in = in_names + out_names
    if partition_name is not None:
        all_in = all_in + [partition_name]

    def _body(*args):
        operands = list(args)
        if partition_name is not None:
            operands.append(partition_id_tensor())
        outs = _bass_exec_p.bind(
            *operands, out_avals=tuple(out_avals), in_names=tuple(all_in),
            out_names=tuple(out_names), lowering_input_output_aliases=(),
            sim_require_finite=False, sim_require_nnan=False, nc=nc)
        return tuple(outs)

    devices = jax.devices()[:n_cores]
    mesh = Mesh(np.asarray(devices), ("core",))
    nin = n_params + len(zero_outs)
    sharded = jax.jit(shard_map(
        _body, mesh=mesh, in_specs=(PartitionSpec("core"),) * nin,
        out_specs=(PartitionSpec("core"),) * len(out_names),
        check_rep=False), keep_unused=True)

    def run(in_maps):
        per_core = [[np.asarray(m[name]) for name in in_names]
                    for m in in_maps]
        concat_in = [np.concatenate([per_core[c][i] for c in range(n_cores)],
                                    axis=0) for i in range(n_params)]
        concat_zeros = [np.zeros((n_cores * z.shape[0], *z.shape[1:]),
                                 z.dtype) for z in zero_outs]
        out_arrs = sharded(*concat_in, *concat_zeros)
        jax.block_until_ready(out_arrs)
        return [{name: np.asarray(out_arrs[i]).reshape(
                    n_cores, *out_avals[i].shape)[c]
                 for i, name in enumerate(out_names)}
                for c in range(n_cores)]

    def run_device(dev_in, fresh_zero_arrs):
        out_arrs = sharded(*dev_in, *fresh_zero_arrs)
        jax.block_until_ready(out_arrs)
        return out_arrs

    runner = dict(run=run, run_device=run_device, sharded=sharded,
                  in_names=in_names, out_names=out_names,
                  out_avals=out_avals, zero_outs=zero_outs,
                  n_params=n_params, mesh=mesh, n_cores=n_cores)
    return runner


_RUNNER_CACHE = {}


def _get_runner():
    if "r" not in _RUNNER_CACHE:
        _RUNNER_CACHE["r"] = _make_runner(get_nc(), N_CORES)
    return _RUNNER_CACHE["r"]


# ===========================================================================
# structured-input check + numpy fallback
# ===========================================================================
def _is_structured(edge_index, batch, group_ids, num_graphs, N, E):
    ng = int(np.asarray(num_graphs))
    if ng != N_GRAPHS or N != ng * N_ROI or E != ng * E_G:
        return False
    src = np.asarray(edge_index[0])
    dst = np.asarray(edge_index[1])
    idx = np.arange(N_ROI)
    s = np.repeat(idx, N_ROI)
    dd = np.tile(idx, N_ROI)
    off = (np.arange(ng) * N_ROI)[:, None]
    if not np.array_equal(src.reshape(ng, E_G), s[None, :] + off):
        return False
    if not np.array_equal(dst.reshape(ng, E_G), dd[None, :] + off):
        return False
    if not np.array_equal(np.asarray(batch),
                          np.repeat(np.arange(ng), N_ROI)):
        return False
    gi = np.asarray(group_ids)
    if not np.array_equal(gi, np.tile(gi[:N_ROI], ng)):
        return False
    return True


def _numpy_fallback(x, edge_attr, emb, enc_W, enc_b, bn_g, bn_b,
                    gine_We, gine_be, gine_W1, gine_b1, gine_W2, gine_b2,
                    gat_Wl, gat_bl, gat_Wr, gat_br, gat_att, gat_We,
                    gat_bias, pool_W1, pool_b1, pool_w2, lin1_W, lin1_b,
                    lin2_W, lin2_b, edge_index, batch, group_ids,
                    num_graphs):
    f32 = np.float32
    x = np.asarray(x, f32)
    edge_attr = np.asarray(edge_attr, f32)
    src = np.asarray(edge_index[0]).astype(np.int64)
    dst = np.asarray(edge_index[1]).astype(np.int64)
    batch = np.asarray(batch).astype(np.int64)
    ng = int(np.asarray(num_graphs))
    N = x.shape[0]
    h = np.concatenate([x, np.asarray(emb, f32)[np.asarray(group_ids)]], 1)
    h = h @ np.asarray(enc_W, f32) + np.asarray(enc_b, f32)
    h = np.maximum(h, 0)
    mu = h.mean(0)
    var = h.var(0)
    h = (h - mu) / np.sqrt(var + 1e-5) * np.asarray(bn_g, f32) + \
        np.asarray(bn_b, f32)
    e = edge_attr @ np.asarray(gine_We, f32) + np.asarray(gine_be, f32)
    msg = np.maximum(h[src] + e, 0)
    agg = np.zeros_like(h)
    np.add.at(agg, dst, msg)
    h = h + agg
    h = np.maximum(h @ np.asarray(gine_W1, f32) +
                   np.asarray(gine_b1, f32), 0)
    h = h @ np.asarray(gine_W2, f32) + np.asarray(gine_b2, f32)
    h = np.maximum(h, 0)
    for l in range(2):
        xl = h @ np.asarray(gat_Wl, f32)[l] + np.asarray(gat_bl, f32)[l]
        xr = h @ np.asarray(gat_Wr, f32)[l] + np.asarray(gat_br, f32)[l]
        z = xl[src] + xr[dst] + edge_attr @ np.asarray(gat_We, f32)[l]
        z = np.where(z > 0, z, 0.2 * z)
        sc = z @ np.asarray(gat_att, f32)[l]
        m = np.full(N, -np.inf, f32)
        np.maximum.at(m, dst, sc)
        ex = np.exp(sc - m[dst])
        ssum = np.zeros(N, f32)
        np.add.at(ssum, dst, ex)
        alpha = ex / (ssum[dst] + np.float32(1e-16))
        acc = np.zeros_like(h)
        np.add.at(acc, dst, xl[src] * alpha[:, None])
        h = np.maximum(acc + np.asarray(gat_bias, f32)[l], 0)
    sc = np.tanh(h @ np.asarray(pool_W1, f32) + np.asarray(pool_b1, f32))
    sc = sc @ np.asarray(pool_w2, f32)
    ex = np.exp(sc - sc.max())
    w = ex / ex.sum()
    hw = h * w[:, None]
    pooled = np.zeros((ng, HID), f32)
    np.add.at(pooled, batch, hw)
    o = np.maximum(pooled @ np.asarray(lin1_W, f32) +
                   np.asarray(lin1_b, f32), 0)
    return (o @ np.asarray(lin2_W, f32) + np.asarray(lin2_b, f32)).astype(f32)


def kernel(x, edge_attr, emb, enc_W, enc_b, bn_g, bn_b,
           gine_We, gine_be, gine_W1, gine_b1, gine_W2, gine_b2,
           gat_Wl, gat_bl, gat_Wr, gat_br, gat_att, gat_We, gat_bias,
           pool_W1, pool_b1, pool_w2, lin1_W, lin1_b, lin2_W, lin2_b,
           edge_index, batch, group_ids, num_graphs):
    N = np.asarray(x).shape[0]
    E = np.asarray(edge_attr).shape[0]
    if not _is_structured(edge_index, batch, group_ids, num_graphs, N, E):
        return _numpy_fallback(
            x, edge_attr, emb, enc_W, enc_b, bn_g, bn_b, gine_We, gine_be,
            gine_W1, gine_b1, gine_W2, gine_b2, gat_Wl, gat_bl, gat_Wr,
            gat_br, gat_att, gat_We, gat_bias, pool_W1, pool_b1, pool_w2,
            lin1_W, lin1_b, lin2_W, lin2_b, edge_index, batch, group_ids,
            num_graphs)
    in_maps = host_prep(x, edge_attr, emb, enc_W, enc_b, bn_g, bn_b,
                        gine_We, gine_be, gine_W1, gine_b1, gine_W2,
                        gine_b2, gat_Wl, gat_bl, gat_Wr, gat_br, gat_att,
                        gat_We, gat_bias, pool_W1, pool_b1, pool_w2,
                        lin1_W, lin1_b, lin2_W, lin2_b, group_ids)
    runner = _get_runner()
    results = runner["run"](in_maps)
    return assemble_out(results)



# revision 39
# speedup vs baseline: 2.3136x; 1.1588x over previous
"""BrainNetGAT Bass/Tile kernel for 8 Trainium2 NeuronCores.

Graph-level data parallelism: 16 graphs per core, processed as 8 pairs with
two concurrent PE column-tiled streams. Edge message passing is dense
augmented matmuls over each graph's 116x116 edge grid, with edges in
DST-MAJOR order (edge = dst*116 + src):
  moving tile T = [one-hot src-index (116); ea^T (5); ones (2)]  [123, 13456]
  stationary   = [Xsrc (116); We (5); K (2)]                     [123, 64]
so one matmul yields ea@We + x_src[src] for every edge; a second matmul with
a per-chunk sliced broadcast-AP identity adds x_dst[dst]. Dst-major makes
the GINE segment-sum a contiguous-axis DVE reduce and makes the attention
score matrix load back from DRAM directly as [dst, src] with no transposes.
GAT attention scores are computed by a 4-stream block-diagonal att matmul
(bf16), bounced through DRAM. GINE relu runs on the Vector engine to keep
the Scalar/ACT engine for the GAT leaky-relu. Most small DMAs are issued
from the otherwise-idle Sync engine. BatchNorm stats and the global pooling
softmax sum use two small AllReduces.
"""
import contextlib

import numpy as np
import ml_dtypes

import concourse.bacc as bacc
import concourse.mybir as mybir
import concourse.tile as tile

F32 = mybir.dt.float32
BF16 = mybir.dt.bfloat16
AF = mybir.ActivationFunctionType
OP = mybir.AluOpType
AX = mybir.AxisListType

N_ROI = 116
HID = 64
EDIM = 5
N_GRAPHS = 128
N_CORES = 8
G_C = N_GRAPHS // N_CORES          # 16 graphs per core
PAIRS = G_C // 2
N_C = G_C * N_ROI                  # 1856 nodes per core
E_G = N_ROI * N_ROI                # 13456 edges per graph
E_C = G_C * E_G
N_TOTAL = N_GRAPHS * N_ROI         # 14848
# shared moving tile rows: 0:116 src-onehot, 116:121 eaA, 121:126 eaB,
# 126:128 ones (for the per-graph bf16-centering K rows)
EA_A = N_ROI                       # 116
EA_B = N_ROI + EDIM                # 121
ONES_R = N_ROI + 2 * EDIM          # 126
KAUG = ONES_R + 2                  # 128
KGINE = ONES_R                     # 126 rows for the GINE matmuls
CH = 4 * N_ROI                     # 464-col edge chunk (4 dst blocks)
NCH = E_G // CH                    # 29
GRP = 3                            # z-chunks per psum group
NODE_CH = 4 * N_ROI                # 464 node cols (4 graphs)


def build_nc(n_cores=N_CORES):
    nc = bacc.Bacc()
    d = {}

    def inp(name, shape, dt):
        d[name] = nc.declare_dram_parameter(name, list(shape), dt,
                                            isOutput=False)

    inp("xT", (N_ROI, N_C), BF16)
    inp("eaT", (EDIM, E_C), BF16)
    inp("embT", (16, N_ROI), BF16)
    inp("I116", (N_ROI, N_ROI), BF16)
    inp("encW1", (N_ROI, HID), BF16)
    inp("encW2", (16, HID), BF16)
    inp("enc_b", (HID, 1), F32)
    inp("bn_g", (HID, 1), F32)
    inp("bn_b", (HID, 1), F32)
    inp("gineWeZA", (2 * EDIM, HID), BF16)
    inp("gineWeZB", (2 * EDIM, HID), BF16)
    inp("gine_be", (HID, 1), F32)
    inp("gineW1", (HID, HID), F32)
    inp("gine_b1", (HID, 1), F32)
    inp("gineW2", (HID, HID), F32)
    inp("gine_b2", (HID, 1), F32)
    for l in range(2):
        inp(f"gatWl{l}", (HID, HID), F32)
        inp(f"gat_bl{l}", (HID, 1), F32)
        inp(f"gatWr{l}", (HID, HID), F32)
        inp(f"gat_br{l}", (HID, 1), F32)
        inp(f"gat_blb{l}", (HID, 1), F32)
        inp(f"gatWeZA{l}", (2 * EDIM, HID), BF16)
        inp(f"gatWeZB{l}", (2 * EDIM, HID), BF16)
        inp(f"att{l}", (128, 1), F32)
    inp("poolW1", (HID, HID), F32)
    inp("pool_b1", (HID, 1), F32)
    inp("pool_w2", (HID, 1), BF16)
    inp("lin1W", (HID, N_ROI), BF16)
    inp("lin1_b", (N_ROI, 1), F32)
    inp("lin2W", (N_ROI, 2), BF16)
    inp("lin2_b", (2, 1), F32)
    inp("ones2", (2, E_G), BF16)
    outT = nc.declare_dram_parameter("outT", [2, G_C], F32, isOutput=True)

    with tile.TileContext(nc) as tc:
        _body(nc, tc, d, outT, n_cores)
    nc.finalize()
    return nc


def _body(nc, tc, d, outT, n_cores=N_CORES):
    ctx = contextlib.ExitStack()
    with ctx:
        wpool = ctx.enter_context(tc.tile_pool(name="weights", bufs=1))
        state = ctx.enter_context(tc.tile_pool(name="state", bufs=1))
        tpool = ctx.enter_context(tc.tile_pool(name="tmoving", bufs=1))
        upool = ctx.enter_context(tc.tile_pool(name="u", bufs=1))
        spool = ctx.enter_context(tc.tile_pool(name="smalls", bufs=3))
        station = ctx.enter_context(tc.tile_pool(name="station", bufs=2))
        psA = ctx.enter_context(tc.tile_pool(name="psA", bufs=2, space="PSUM"))
        psB = ctx.enter_context(tc.tile_pool(name="psB", bufs=2, space="PSUM"))
        dpool = ctx.enter_context(tc.tile_pool(name="dram", bufs=2,
                                               space="DRAM"))
        bn_in = dpool.tile([HID, 2], F32, tag="bn_in", bufs=1)
        bn_out = dpool.tile([HID, 2], F32, tag="bn_out", bufs=1)
        s_in = dpool.tile([1, 1], F32, tag="s_in", bufs=1)
        s_out = dpool.tile([1, 1], F32, tag="s_out", bufs=1)
        psc_row = dpool.tile([1, N_C], F32, tag="psc_row", bufs=1)
        scAB0 = dpool.tile([2, E_G], F32, tag="scAB0", bufs=1)
        scAB1 = dpool.tile([2, E_G], F32, tag="scAB1", bufs=1)
        scABs = (scAB0, scAB1)

        # ---------- weights / constants ----------
        W = {}
        for name, h in d.items():
            if name in ("eaT", "ones2"):
                continue
            W[name] = wpool.tile(list(h.shape), h.dtype, tag=name, name=name)
            nc.gpsimd.dma_start(out=W[name][:], in_=h[:])

        ident = wpool.tile([128, 128], BF16, tag="ident")
        nc.vector.memset(ident[:], 0.0)
        nc.gpsimd.affine_select(out=ident[:], in_=ident[:],
                                compare_op=OP.not_equal, fill=1.0, base=0,
                                pattern=[[-1, 128]], channel_multiplier=1)
        identF = wpool.tile([128, 128], F32, tag="identF")
        nc.vector.memset(identF[:], 0.0)
        nc.gpsimd.affine_select(out=identF[:], in_=identF[:],
                                compare_op=OP.not_equal, fill=1.0, base=0,
                                pattern=[[-1, 128]], channel_multiplier=1)
        alpha02 = wpool.tile([128, 1], F32, tag="alpha02")
        nc.vector.memset(alpha02[:], 0.2)
        eps6 = wpool.tile([N_ROI, 1], F32, tag="eps6")
        nc.vector.memset(eps6[:], 1e-6)
        eps5 = wpool.tile([HID, 1], F32, tag="eps5")
        nc.vector.memset(eps5[:], 1e-5)
        ones116 = wpool.tile([N_ROI, 1], BF16, tag="ones116")
        nc.vector.memset(ones116[:], 1.0)

        attd = []
        for l in range(2):
            t = wpool.tile([128, 32], BF16, tag=f"attd{l}")
            nc.vector.memset(t[:], 0.0)
            nc.vector.tensor_copy(t[0:HID, 0:1], W[f"att{l}"][0:HID, :])
            nc.vector.tensor_copy(t[HID:128, 1:2], W[f"att{l}"][HID:128, :])
            attd.append(t)

        Bbe = wpool.tile([HID, 1], F32, tag="Bbe")

        # one shared moving tile (dst-major edges) for BOTH graphs of a pair:
        # rows 0:116 = s-onehot, 116:121 = eaA, 121:126 = eaB, 126:128 = ones
        Tb0 = tpool.tile([KAUG, E_G], BF16, tag="Tb0")
        Tb1 = tpool.tile([KAUG, E_G], BF16, tag="Tb1")
        T_bufs = (Tb0, Tb1)
        sind_src = W["I116"][:, :].unsqueeze(1).broadcast_to(
            [N_ROI, N_ROI, N_ROI])
        for Tt in T_bufs:
            nc.vector.tensor_copy(
                Tt[0:N_ROI, :].rearrange("p (dd s) -> p dd s", s=N_ROI),
                sind_src)
            nc.gpsimd.dma_start(out=Tt[ONES_R:KAUG, :], in_=d["ones2"][:])

        hT = state.tile([HID, N_C], F32, tag="hT")
        hbeT = state.tile([HID, N_C], BF16, tag="hbeT")
        h0T = state.tile([HID, N_C], BF16, tag="h0T")
        tanT = state.tile([HID, N_C], BF16, tag="tanT")
        eMat = state.tile([N_ROI, G_C], F32, tag="eMat")
        hnew = []
        for g in range(G_C):
            hn_t = state.tile([N_ROI, HID], BF16, tag=f"hnew{g}",
                              name=f"hnew{g}")
            hnew.append(hn_t)

        # ---------- phase A: encoder + BN ----------
        emb_b = W["embT"][:, :].unsqueeze(1).broadcast_to([16, 4, N_ROI])
        for k in range(N_C // NODE_CH):
            sl = slice(k * NODE_CH, (k + 1) * NODE_CH)
            ps = psB.tile([HID, NODE_CH], F32, tag="mm_ps")
            nc.tensor.matmul(ps[:], W["encW1"][:], W["xT"][:, sl],
                             start=True, stop=False)
            nc.tensor.matmul(ps[:], W["encW2"][:], emb_b,
                             start=False, stop=True)
            nc.scalar.activation(h0T[:, sl], ps[:], AF.Relu,
                                 bias=W["enc_b"][:])

        st = spool.tile([HID, 2], F32, tag="bn_st")
        sq = upool.tile([HID, N_C], BF16, tag="sq")
        nc.vector.tensor_reduce(st[:, 0:1], h0T[:, :], axis=AX.X, op=OP.add)
        nc.vector.tensor_tensor(sq[:], h0T[:], h0T[:], op=OP.mult)
        nc.vector.tensor_reduce(st[:, 1:2], sq[:, :], axis=AX.X, op=OP.add)
        nc.gpsimd.dma_start(out=bn_in[:], in_=st[:])
        nc.gpsimd.collective_compute(
            "AllReduce", OP.add, replica_groups=[list(range(n_cores))],
            ins=[bn_in[:]], outs=[bn_out[:]])
        stg = spool.tile([HID, 2], F32, tag="bn_stg")
        nc.gpsimd.dma_start(out=stg[:], in_=bn_out[:])

        mu = spool.tile([HID, 1], F32, tag="mu")
        var = spool.tile([HID, 1], F32, tag="var")
        sd = spool.tile([HID, 1], F32, tag="sd")
        A = spool.tile([HID, 1], F32, tag="A")
        B = spool.tile([HID, 1], F32, tag="B")
        t3 = spool.tile([HID, 1], F32, tag="t3")
        nc.vector.tensor_scalar_mul(mu[:], stg[:, 0:1], 1.0 / N_TOTAL)
        nc.vector.tensor_scalar_mul(var[:], stg[:, 1:2], 1.0 / N_TOTAL)
        nc.vector.tensor_tensor(t3[:], mu[:], mu[:], op=OP.mult)
        nc.vector.tensor_tensor(var[:], var[:], t3[:], op=OP.subtract)
        nc.scalar.activation(sd[:], var[:], AF.Sqrt, bias=eps5[:])
        nc.vector.reciprocal(sd[:], sd[:])
        nc.vector.tensor_tensor(A[:], sd[:], W["bn_g"][:], op=OP.mult)
        nc.vector.tensor_tensor(t3[:], mu[:], A[:], op=OP.mult)
        nc.vector.tensor_tensor(B[:], W["bn_b"][:], t3[:], op=OP.subtract)
        nc.vector.tensor_tensor(Bbe[:], B[:], W["gine_be"][:], op=OP.add)
        nc.scalar.activation(hT[:, :], h0T[:, :], AF.Identity,
                             bias=B[:], scale=A[:])
        nc.scalar.activation(hbeT[:, :], h0T[:, :], AF.Identity,
                             bias=Bbe[:], scale=A[:])

        # ---------- phase B: software-pipelined pairs ----------
        def gine_head(p):
            """ea load + GINE stationaries + edge matmuls + relu/segsum."""
            gA, gB = 2 * p, 2 * p + 1
            cA = slice(gA * N_ROI, (gA + 1) * N_ROI)
            cB = slice(gB * N_ROI, (gB + 1) * N_ROI)
            T = T_bufs[p % 2]
            nc.gpsimd.dma_start(out=T[EA_A:EA_A + EDIM, :],
                                in_=d["eaT"][:, gA * E_G:(gA + 1) * E_G])
            nc.gpsimd.dma_start(out=T[EA_B:EA_B + EDIM, :],
                                in_=d["eaT"][:, gB * E_G:(gB + 1) * E_G])
            SA = station.tile([KGINE, HID], BF16, tag="SA")
            SB = station.tile([KGINE, HID], BF16, tag="SB")
            nc.gpsimd.dma_start(out=SA[EA_A:KGINE, :],
                                in_=d["gineWeZA"][:])
            nc.gpsimd.dma_start(out=SB[EA_A:KGINE, :],
                                in_=d["gineWeZB"][:])
            for (S, cg) in ((SA, cA), (SB, cB)):
                trp = psB.tile([N_ROI, HID], BF16, tag="mm_ps")
                nc.tensor.transpose(trp[:], hbeT[:, cg], ident[0:HID, 0:HID])
                nc.vector.tensor_copy(S[0:N_ROI, :], trp[:])

            agg = spool.tile([128, N_ROI], F32, tag="agg",
                             name=f"agg{p}")
            for g0 in range(0, NCH, GRP):
                ng = min(GRP, NCH - g0)
                zps = psA.tile([128, GRP * 512], F32, tag="zps")
                for j in range(ng):
                    ch = slice((g0 + j) * CH, (g0 + j + 1) * CH)
                    pj = slice(j * 512, j * 512 + CH)
                    nc.tensor.matmul(zps[0:HID, pj], SA,
                                     T[0:KGINE, ch],
                                     start=True, stop=True)
                for j in range(ng):
                    ch = slice((g0 + j) * CH, (g0 + j + 1) * CH)
                    pj = slice(j * 512, j * 512 + CH)
                    nc.tensor.matmul(zps[HID:128, pj], SB,
                                     T[0:KGINE, ch],
                                     start=True, stop=True,
                                     tile_position=(0, 64))
                # relu into a small scratch, then contiguous segment-sum
                u1g = spool.tile([128, GRP * CH], BF16, tag="u1g")
                srcv = zps[:, :].rearrange("p (g c) -> p g c",
                                           c=512)[:, 0:ng, 0:CH]
                dstv = u1g[:, 0:ng * CH].rearrange("p (g c) -> p g c", c=CH)
                nc.vector.tensor_scalar_max(dstv, srcv, 0.0)
                rv = u1g[:, 0:ng * CH].rearrange("p (dd s) -> p dd s",
                                                 s=N_ROI)
                nc.vector.tensor_reduce(agg[:, 4 * g0:4 * (g0 + ng)], rv,
                                        axis=AX.X, op=OP.add)
            return agg

        def gine_rest(p, agg):
            gA, gB = 2 * p, 2 * p + 1
            cA = slice(gA * N_ROI, (gA + 1) * N_ROI)
            cB = slice(gB * N_ROI, (gB + 1) * N_ROI)
            cP = slice(gA * N_ROI, (gB + 1) * N_ROI)
            nc.vector.tensor_tensor(hT[:, cA], hT[:, cA], agg[0:HID, :],
                                    op=OP.add)
            aggB = spool.tile([HID, N_ROI], F32, tag="aggB")
            nc.gpsimd.dma_start(out=aggB[:], in_=agg[HID:128, :])
            nc.vector.tensor_tensor(hT[:, cB], hT[:, cB], aggB[:],
                                    op=OP.add)
            mp1 = psB.tile([HID, 2 * N_ROI], F32, tag="mm_ps")
            nc.tensor.matmul(mp1[:], W["gineW1"][:], hT[:, cP],
                             start=True, stop=True)
            mt = spool.tile([HID, 2 * N_ROI], F32, tag="mt")
            nc.scalar.activation(mt[:], mp1[:], AF.Relu, bias=W["gine_b1"][:])
            mp2 = psB.tile([HID, 2 * N_ROI], F32, tag="mm_ps")
            nc.tensor.matmul(mp2[:], W["gineW2"][:], mt[:],
                             start=True, stop=True)
            nc.scalar.activation(hT[:, cP], mp2[:], AF.Relu,
                                 bias=W["gine_b2"][:])

        def gat_edges(p, l):
            """xl/xr projections, stationaries, edge matmuls + prelu,
            attention-score matmuls with PSUM-direct extraction."""
            gA, gB = 2 * p, 2 * p + 1
            cP = slice(gA * N_ROI, (gB + 1) * N_ROI)
            lA = slice(0, N_ROI)
            lB = slice(N_ROI, 2 * N_ROI)
            T = T_bufs[p % 2]
            scAB = scABs[l]
            xps = psB.tile([HID, 2 * N_ROI], F32, tag="mm_ps")
            nc.tensor.matmul(xps[:], W[f"gatWl{l}"][:], hT[:, cP],
                             start=True, stop=True)
            xlT = spool.tile([HID, 2 * N_ROI], F32, tag="xlT")
            xlbT = spool.tile([HID, 2 * N_ROI], F32, tag="xlbT")
            nc.scalar.activation(xlT[:], xps[:], AF.Identity,
                                 bias=W[f"gat_bl{l}"][:])
            nc.scalar.activation(xlbT[:], xps[:], AF.Identity,
                                 bias=W[f"gat_blb{l}"][:])
            xps2 = psB.tile([HID, 2 * N_ROI], F32, tag="mm_ps")
            nc.tensor.matmul(xps2[:], W[f"gatWr{l}"][:], hT[:, cP],
                             start=True, stop=True)
            xrT = spool.tile([HID, 2 * N_ROI], F32, tag="xrT")
            nc.scalar.activation(xrT[:], xps2[:], AF.Identity,
                                 bias=W[f"gat_br{l}"][:])

            SA2 = station.tile([KAUG, HID], BF16, tag="SA2")
            SB2 = station.tile([KAUG, HID], BF16, tag="SB2")
            XrA = station.tile([N_ROI, HID], BF16, tag="XrA")
            XrB = station.tile([N_ROI, HID], BF16, tag="XrB")
            XlbA = station.tile([N_ROI, HID], F32, tag="XlbA")
            XlbB = station.tile([N_ROI, HID], F32, tag="XlbB")
            nc.sync.dma_start(out=SA2[EA_A:ONES_R, :],
                              in_=d[f"gatWeZA{l}"][:])
            nc.sync.dma_start(out=SB2[EA_A:ONES_R, :],
                              in_=d[f"gatWeZB{l}"][:])
            for (S, Xr, Xlb, lg) in ((SA2, XrA, XlbA, lA),
                                     (SB2, XrB, XlbB, lB)):
                # per-graph centering of xl/xr; exact offset via 2 rows
                mL = spool.tile([HID, 1], F32, tag="mL")
                mR = spool.tile([HID, 1], F32, tag="mR")
                nc.vector.tensor_reduce(mL[:], xlT[:, lg], axis=AX.X,
                                        op=OP.add)
                nc.vector.tensor_scalar_mul(mL[:], mL[:], 1.0 / N_ROI)
                nc.vector.tensor_reduce(mR[:], xrT[:, lg], axis=AX.X,
                                        op=OP.add)
                nc.vector.tensor_scalar_mul(mR[:], mR[:], 1.0 / N_ROI)
                xlc = spool.tile([HID, N_ROI], BF16, tag="xlc")
                xrc = spool.tile([HID, N_ROI], BF16, tag="xrc")
                nc.vector.tensor_scalar(xlc[:], xlT[:, lg], mL[:],
                                        scalar2=None,
                                        op0=OP.subtract)
                nc.vector.tensor_scalar(xrc[:], xrT[:, lg], mR[:],
                                        scalar2=None,
                                        op0=OP.subtract)
                Ksum = spool.tile([HID, 1], F32, tag="Ksum")
                nc.vector.tensor_tensor(Ksum[:], mL[:], mR[:], op=OP.add)
                K2 = spool.tile([HID, 2], BF16, tag="K2")
                nc.vector.tensor_copy(K2[:, 0:1], Ksum[:])
                Klo = spool.tile([HID, 1], F32, tag="Klo")
                nc.vector.tensor_tensor(Klo[:], Ksum[:], K2[:, 0:1],
                                        op=OP.subtract)
                nc.vector.tensor_copy(K2[:, 1:2], Klo[:])
                k2p = psB.tile([2, HID], BF16, tag="mm_ps")
                nc.tensor.transpose(k2p[:], K2[:], ident[0:HID, 0:HID])
                k2s = spool.tile([2, HID], BF16, tag="k2s")
                nc.vector.tensor_copy(k2s[:], k2p[:])
                nc.gpsimd.dma_start(out=S[ONES_R:KAUG, :],
                                    in_=k2s[:])
                t1p = psB.tile([N_ROI, HID], BF16, tag="mm_ps")
                nc.tensor.transpose(t1p[:], xlc[:], ident[0:HID, 0:HID])
                nc.vector.tensor_copy(S[0:N_ROI, :], t1p[:])
                t2p = psB.tile([N_ROI, HID], BF16, tag="mm_ps")
                nc.tensor.transpose(t2p[:], xrc[:], ident[0:HID, 0:HID])
                nc.vector.tensor_copy(Xr[:], t2p[:])
                t3p = psB.tile([N_ROI, HID], F32, tag="mm_ps")
                nc.tensor.transpose(t3p[:], xlbT[:, lg],
                                    identF[0:HID, 0:HID])
                nc.vector.tensor_copy(Xlb[:], t3p[:])

            u2 = upool.tile([128, E_G], BF16, tag="u")
            for g0 in range(0, NCH, GRP):
                ng = min(GRP, NCH - g0)
                zps = psA.tile([128, GRP * 512], F32, tag="zps")
                for j in range(ng):
                    ch = slice((g0 + j) * CH, (g0 + j + 1) * CH)
                    pj = slice(j * 512, j * 512 + CH)
                    nc.tensor.matmul(zps[0:HID, pj], SA2, T[:, ch],
                                     start=True, stop=False)
                for j in range(ng):
                    c4 = slice(4 * (g0 + j), 4 * (g0 + j) + 4)
                    pj = slice(j * 512, j * 512 + CH)
                    dind = W["I116"][:, c4].unsqueeze(2).broadcast_to(
                        [N_ROI, 4, N_ROI])
                    nc.tensor.matmul(zps[0:HID, pj], XrA, dind,
                                     start=False, stop=True)
                for j in range(ng):
                    ch = slice((g0 + j) * CH, (g0 + j + 1) * CH)
                    pj = slice(j * 512, j * 512 + CH)
                    nc.tensor.matmul(zps[HID:128, pj], SB2, T[:, ch],
                                     start=True, stop=False,
                                     tile_position=(0, 64))
                for j in range(ng):
                    c4 = slice(4 * (g0 + j), 4 * (g0 + j) + 4)
                    pj = slice(j * 512, j * 512 + CH)
                    dind = W["I116"][:, c4].unsqueeze(2).broadcast_to(
                        [N_ROI, 4, N_ROI])
                    nc.tensor.matmul(zps[HID:128, pj], XrB, dind,
                                     start=False, stop=True,
                                     tile_position=(0, 64))
                src = zps[:, :].rearrange("p (g c) -> p g c",
                                          c=512)[:, 0:ng, 0:CH]
                dst = u2[:, g0 * CH:(g0 + ng) * CH].rearrange(
                    "p (g c) -> p g c", c=CH)
                nc.scalar.activation(dst, src, AF.Prelu,
                                     alpha=alpha02[:])

            # attention scores: 4 col-tiled streams -> rows 0,32,64,96
            scAB_c = scAB[:, :].rearrange("r (cc c) -> r cc c", c=CH)
            for base0 in range(0, NCH, 12):
                n = min(12, NCH - base0)
                npad = (n + 3) // 4 * 4
                nslot = npad // 4
                sps = psA.tile([128, GRP * 512], F32, tag="zps")
                for idx in range(npad):
                    c = base0 + min(idx, n - 1)
                    k, j = idx % 4, idx // 4
                    nc.tensor.matmul(
                        sps[32 * k:32 * k + 32, j * 512:j * 512 + CH],
                        attd[l], u2[:, c * CH:(c + 1) * CH],
                        start=True, stop=True,
                        tile_position=(0, 32 * k))
                scc = spool.tile([128, GRP * 512], F32, tag="scc", bufs=2)
                ssrc = sps[:, :].rearrange("p (j c) -> p j c",
                                           c=512)[:, 0:nslot, 0:CH]
                sdst = scc[:, 0:nslot * CH].rearrange(
                    "p (j c) -> p j c", c=CH)
                nc.scalar.activation(sdst, ssrc, AF.Copy)
                for k in range(4):
                    nk = len([i for i in range(n) if i % 4 == k])
                    if nk == 0:
                        continue
                    src3 = scc[32 * k:32 * k + 2, 0:nslot * CH].rearrange(
                        "p (j c) -> p j c", c=CH)[:, 0:nk, :]
                    dst3 = scAB_c[:, base0 + k:base0 + n:4, :]
                    nc.sync.dma_start(out=dst3, in_=src3)
            return XlbA, XlbB

        def gat_tail(p, l, XlbA, XlbB):
            """per-graph softmax + alpha-weighted aggregation."""
            gA, gB = 2 * p, 2 * p + 1
            cA = slice(gA * N_ROI, (gA + 1) * N_ROI)
            cB = slice(gB * N_ROI, (gB + 1) * N_ROI)
            # dst-major: scAB rows reload directly as [dst, src]
            scAB_m = scABs[l][:, :].rearrange("r (dd s) -> r dd s",
                                              s=N_ROI)
            for (g, rr, Xlb, cg) in ((gA, 0, XlbA, cA),
                                     (gB, 1, XlbB, cB)):
                epT = spool.tile([N_ROI, N_ROI], F32, tag="epT")
                nc.sync.dma_start(out=epT[:], in_=scAB_m[rr])
                mrow = spool.tile([N_ROI, 1], F32, tag="mrow")
                nc.vector.tensor_reduce(mrow[:], epT[:, :], axis=AX.X,
                                        op=OP.max)
                mneg = spool.tile([N_ROI, 1], F32, tag="mneg")
                nc.vector.tensor_scalar_mul(mneg[:], mrow[:], -1.0)
                ehT = spool.tile([N_ROI, N_ROI], F32, tag="ehT")
                nc.scalar.activation(ehT[:], epT[:], AF.Exp,
                                     bias=mneg[:])
                srow = spool.tile([N_ROI, 1], F32, tag="srow")
                nc.vector.tensor_reduce(srow[:], ehT[:, :], axis=AX.X,
                                        op=OP.add)
                rrow = spool.tile([N_ROI, 1], F32, tag="rrow")
                nc.vector.reciprocal(rrow[:], srow[:])
                adT = spool.tile([N_ROI, N_ROI], F32, tag="adT")
                nc.vector.tensor_scalar_mul(adT[:], ehT[:], rrow[:])
                as_ps = psB.tile([N_ROI, N_ROI], F32, tag="mm_ps")
                nc.tensor.transpose(as_ps[:], adT[:],
                                    identF[0:N_ROI, 0:N_ROI])
                aS = spool.tile([N_ROI, N_ROI], F32, tag="aS")
                nc.vector.tensor_copy(aS[:], as_ps[:])
                am = psB.tile([N_ROI, HID], F32, tag="mm_ps")
                nc.tensor.matmul(am[:], aS[:], Xlb[:],
                                 start=True, stop=True)
                hnF = spool.tile([N_ROI, HID], F32, tag="hnF")
                nc.scalar.activation(hnF[:], am[:], AF.Relu)
                nc.vector.tensor_copy(hnew[g][:], hnF[:])
                htp = psB.tile([HID, N_ROI], F32, tag="mm_ps")
                nc.tensor.transpose(htp[:], hnF[:],
                                    identF[0:N_ROI, 0:N_ROI])
                nc.vector.tensor_copy(hT[:, cg], htp[:])

        def pool_scores(p):
            gA, gB = 2 * p, 2 * p + 1
            cP = slice(gA * N_ROI, (gB + 1) * N_ROI)
            pps = psB.tile([HID, 2 * N_ROI], F32, tag="mm_ps")
            nc.tensor.matmul(pps[:], W["poolW1"][:], hT[:, cP],
                             start=True, stop=True)
            nc.scalar.activation(tanT[:, cP], pps[:], AF.Tanh,
                                 bias=W["pool_b1"][:])
            scp = psB.tile([1, 2 * N_ROI], F32, tag="mm_ps")
            nc.tensor.matmul(scp[:], W["pool_w2"][:], tanT[:, cP],
                             start=True, stop=True)
            scs = spool.tile([1, 2 * N_ROI], F32, tag="scs")
            nc.vector.tensor_copy(scs[:], scp[:])
            nc.gpsimd.dma_start(
                out=psc_row[0, p * 2 * N_ROI:(p + 1) * 2 * N_ROI],
                in_=scs[:])

        # 2-deep software pipeline: the next pair's GINE head and first
        # GAT edge phase fill the PE while this pair's softmax tails run
        agg0 = gine_head(0)
        gine_rest(0, agg0)
        E0 = gat_edges(0, 0)
        for p in range(PAIRS):
            if p + 1 < PAIRS:
                agg_n = gine_head(p + 1)
            gat_tail(p, 0, *E0)
            E1 = gat_edges(p, 1)
            if p + 1 < PAIRS:
                gine_rest(p + 1, agg_n)
                E0 = gat_edges(p + 1, 0)
            gat_tail(p, 1, *E1)
            pool_scores(p)

        # ---------- phase C: pooling + head ----------
        nc.gpsimd.dma_start(
            out=eMat[:], in_=psc_row[0, :].rearrange("(g r) -> r g", r=N_ROI))
        eMb = spool.tile([N_ROI, G_C], BF16, tag="eMb")
        nc.scalar.activation(eMb[:], eMat[:], AF.Exp)
        ssum_ps = psB.tile([1, G_C], F32, tag="mm_ps")
        nc.tensor.matmul(ssum_ps[:], ones116[:], eMb[:],
                         start=True, stop=True)
        ssum = spool.tile([1, 1], F32, tag="ssum")
        nc.vector.tensor_reduce(ssum[:], ssum_ps[0:1, :], axis=AX.X,
                                op=OP.add)
        nc.gpsimd.dma_start(out=s_in[:], in_=ssum[:])
        nc.gpsimd.collective_compute(
            "AllReduce", OP.add, replica_groups=[list(range(n_cores))],
            ins=[s_in[:]], outs=[s_out[:]])
        sS64 = spool.tile([HID, 1], F32, tag="sS64")
        nc.gpsimd.dma_start(out=sS64[:], in_=s_out[:].broadcast_to([HID, 1]))
        nc.vector.reciprocal(sS64[:], sS64[:])

        pool_ps = psB.tile([HID, G_C], F32, tag="mm_ps")
        for g in range(G_C):
            nc.tensor.matmul(pool_ps[:, g:g + 1], hnew[g][:],
                             eMb[:, g:g + 1], start=True, stop=True)
        pooledT = spool.tile([HID, G_C], BF16, tag="pooledT")
        nc.scalar.activation(pooledT[:], pool_ps[:], AF.Identity,
                             scale=sS64[:])
        o1ps = psB.tile([N_ROI, G_C], F32, tag="mm_ps")
        nc.tensor.matmul(o1ps[:], W["lin1W"][:], pooledT[:],
                         start=True, stop=True)
        o1 = spool.tile([N_ROI, G_C], BF16, tag="o1")
        nc.scalar.activation(o1[:], o1ps[:], AF.Relu, bias=W["lin1_b"][:])
        o2ps = psB.tile([2, G_C], F32, tag="mm_ps")
        nc.tensor.matmul(o2ps[:], W["lin2W"][:], o1[:], start=True, stop=True)
        oflast = spool.tile([2, G_C], F32, tag="oflast")
        nc.scalar.activation(oflast[:], o2ps[:], AF.Identity,
                             bias=W["lin2_b"][:])
        nc.gpsimd.dma_start(out=outT[:], in_=oflast[:])


# ---------------------------------------------------------------------------
_NC_CACHE = {}


def get_nc():
    if "nc" not in _NC_CACHE:
        _NC_CACHE["nc"] = build_nc()
    return _NC_CACHE["nc"]


def host_prep(x, edge_attr, emb, enc_W, enc_b, bn_g, bn_b,
              gine_We, gine_be, gine_W1, gine_b1, gine_W2, gine_b2,
              gat_Wl, gat_bl, gat_Wr, gat_br, gat_att, gat_We, gat_bias,
              pool_W1, pool_b1, pool_w2, lin1_W, lin1_b, lin2_W, lin2_b,
              group_ids):
    bf = ml_dtypes.bfloat16
    f32 = np.float32

    def col(v):
        return np.ascontiguousarray(np.asarray(v, f32).reshape(-1, 1))

    base = {
        "embT": np.ascontiguousarray(
            np.asarray(emb, f32).T[:, np.asarray(group_ids[:N_ROI])]
        ).astype(bf),
        "I116": np.eye(N_ROI).astype(bf),
        "encW1": np.ascontiguousarray(np.asarray(enc_W, f32)[:N_ROI]
                                      ).astype(bf),
        "encW2": np.ascontiguousarray(np.asarray(enc_W, f32)[N_ROI:]
                                      ).astype(bf),
        "enc_b": col(enc_b), "bn_g": col(bn_g), "bn_b": col(bn_b),
        "gineWeZA": np.vstack([np.asarray(gine_We, f32),
                               np.zeros((EDIM, HID), f32)]).astype(bf),
        "gineWeZB": np.vstack([np.zeros((EDIM, HID), f32),
                               np.asarray(gine_We, f32)]).astype(bf),
        "gine_be": col(gine_be),
        "gineW1": np.asarray(gine_W1, f32),
        "gine_b1": col(gine_b1),
        "gineW2": np.asarray(gine_W2, f32),
        "gine_b2": col(gine_b2),
        "poolW1": np.asarray(pool_W1, f32),
        "pool_b1": col(pool_b1),
        "pool_w2": col(pool_w2).astype(bf),
        "lin1W": np.asarray(lin1_W, f32).astype(bf),
        "lin1_b": col(lin1_b),
        "lin2W": np.asarray(lin2_W, f32).astype(bf),
        "lin2_b": col(lin2_b),
        "ones2": np.ones((2, E_G)).astype(bf),
    }
    for l in range(2):
        base[f"gatWl{l}"] = np.asarray(gat_Wl[l], f32)
        base[f"gat_bl{l}"] = col(gat_bl[l])
        base[f"gatWr{l}"] = np.asarray(gat_Wr[l], f32)
        base[f"gat_br{l}"] = col(gat_br[l])
        base[f"gat_blb{l}"] = col(np.asarray(gat_bl[l], f32) +
                                  np.asarray(gat_bias[l], f32))
        base[f"gatWeZA{l}"] = np.vstack([np.asarray(gat_We[l], f32),
                                         np.zeros((EDIM, HID), f32)
                                         ]).astype(bf)
        base[f"gatWeZB{l}"] = np.vstack([np.zeros((EDIM, HID), f32),
                                         np.asarray(gat_We[l], f32)
                                         ]).astype(bf)
        base[f"att{l}"] = col(np.concatenate([np.asarray(gat_att[l], f32), np.asarray(gat_att[l], f32)]))

    xT = np.ascontiguousarray(np.asarray(x, f32).T).astype(bf)
    # reorder edges to dst-major within each graph: edge = dst*116 + src
    ea4 = np.asarray(edge_attr, f32).reshape(N_GRAPHS, N_ROI, N_ROI, EDIM)
    ea_dm = np.ascontiguousarray(ea4.transpose(0, 2, 1, 3)).reshape(-1, EDIM)
    eaT = np.ascontiguousarray(ea_dm.T).astype(bf)

    in_maps = []
    for c in range(N_CORES):
        m = dict(base)
        m["xT"] = np.ascontiguousarray(xT[:, c * N_C:(c + 1) * N_C])
        m["eaT"] = np.ascontiguousarray(eaT[:, c * E_C:(c + 1) * E_C])
        in_maps.append(m)
    return in_maps


def assemble_out(results):
    return np.concatenate([np.asarray(r["outT"], np.float32).T
                           for r in results], axis=0)


# ===========================================================================
# SPMD runner (replicates bass2jax.run_bass_via_pjrt, but reusable + timeable)
# ===========================================================================
def _make_runner(nc, n_cores=N_CORES):
    import jax
    import jax.numpy as jnp
    from jax.sharding import Mesh, PartitionSpec
    from jax.experimental.shard_map import shard_map
    from concourse import bass2jax
    from concourse.bass2jax import _bass_exec_p, partition_id_tensor
    import concourse.mybir as mb

    bass2jax.install_neuronx_cc_hook()
    partition_name = (nc.partition_id_tensor.name
                      if nc.partition_id_tensor else None)
    in_names, out_names, out_avals, zero_outs = [], [], [], []
    for alloc in nc.m.functions[0].allocations:
        if not isinstance(alloc, mb.MemoryLocationSet):
            continue
        name = alloc.memorylocations[0].name
        if alloc.kind == "ExternalInput":
            if name != partition_name:
                in_names.append(name)
        elif alloc.kind == "ExternalOutput":
            out_names.append(name)
            shape = tuple(alloc.tensor_shape)
            dtype = mb.dt.np(alloc.dtype)
            out_avals.append(jax.core.ShapedArray(shape, dtype))
            zero_outs.append(np.zeros(shape, dtype))
    n_params = len(in_names)
    all_in = in_names + out_names
    if partition_name is not None:
        all_in = all_in + [partition_name]

    def _body(*args):
        operands = list(args)
        if partition_name is not None:
            operands.append(partition_id_tensor())
        outs = _bass_exec_p.bind(
            *operands, out_avals=tuple(out_avals), in_names=tuple(all_in),
            out_names=tuple(out_names), lowering_input_output_aliases=(),
            sim_require_finite=False, sim_require_nnan=False, nc=nc)
        return tuple(outs)

    devices = jax.devices()[:n_cores]
    mesh = Mesh(np.asarray(devices), ("core",))
    nin = n_params + len(zero_outs)
    sharded = jax.jit(shard_map(
        _body, mesh=mesh, in_specs=(PartitionSpec("core"),) * nin,
        out_specs=(PartitionSpec("core"),) * len(out_names),
        check_rep=False), keep_unused=True)

    def run(in_maps):
        per_core = [[np.asarray(m[name]) for name in in_names]
                    for m in in_maps]
        concat_in = [np.concatenate([per_core[c][i] for c in range(n_cores)],
                                    axis=0) for i in range(n_params)]
        concat_zeros = [np.zeros((n_cores * z.shape[0], *z.shape[1:]),
                                 z.dtype) for z in zero_outs]
        out_arrs = sharded(*concat_in, *concat_zeros)
        jax.block_until_ready(out_arrs)
        return [{name: np.asarray(out_arrs[i]).reshape(
                    n_cores, *out_avals[i].shape)[c]
                 for i, name in enumerate(out_names)}
                for c in range(n_cores)]

    def run_device(dev_in, fresh_zero_arrs):
        out_arrs = sharded(*dev_in, *fresh_zero_arrs)
        jax.block_until_ready(out_arrs)
        return out_arrs

    runner = dict(run=run, run_device=run_device, sharded=sharded,
                  in_names=in_names, out_names=out_names,
                  out_avals=out_avals, zero_outs=zero_outs,
                  n_params=n_params, mesh=mesh, n_cores=n_cores)
    return runner


_RUNNER_CACHE = {}


def _get_runner():
    if "r" not in _RUNNER_CACHE:
        _RUNNER_CACHE["r"] = _make_runner(get_nc(), N_CORES)
    return _RUNNER_CACHE["r"]


# ===========================================================================
# structured-input check + numpy fallback
# ===========================================================================
def _is_structured(edge_index, batch, group_ids, num_graphs, N, E):
    ng = int(np.asarray(num_graphs))
    if ng != N_GRAPHS or N != ng * N_ROI or E != ng * E_G:
        return False
    src = np.asarray(edge_index[0])
    dst = np.asarray(edge_index[1])
    idx = np.arange(N_ROI)
    s = np.repeat(idx, N_ROI)
    dd = np.tile(idx, N_ROI)
    off = (np.arange(ng) * N_ROI)[:, None]
    if not np.array_equal(src.reshape(ng, E_G), s[None, :] + off):
        return False
    if not np.array_equal(dst.reshape(ng, E_G), dd[None, :] + off):
        return False
    if not np.array_equal(np.asarray(batch),
                          np.repeat(np.arange(ng), N_ROI)):
        return False
    gi = np.asarray(group_ids)
    if not np.array_equal(gi, np.tile(gi[:N_ROI], ng)):
        return False
    return True


def _numpy_fallback(x, edge_attr, emb, enc_W, enc_b, bn_g, bn_b,
                    gine_We, gine_be, gine_W1, gine_b1, gine_W2, gine_b2,
                    gat_Wl, gat_bl, gat_Wr, gat_br, gat_att, gat_We,
                    gat_bias, pool_W1, pool_b1, pool_w2, lin1_W, lin1_b,
                    lin2_W, lin2_b, edge_index, batch, group_ids,
                    num_graphs):
    f32 = np.float32
    x = np.asarray(x, f32)
    edge_attr = np.asarray(edge_attr, f32)
    src = np.asarray(edge_index[0]).astype(np.int64)
    dst = np.asarray(edge_index[1]).astype(np.int64)
    batch = np.asarray(batch).astype(np.int64)
    ng = int(np.asarray(num_graphs))
    N = x.shape[0]
    h = np.concatenate([x, np.asarray(emb, f32)[np.asarray(group_ids)]], 1)
    h = h @ np.asarray(enc_W, f32) + np.asarray(enc_b, f32)
    h = np.maximum(h, 0)
    mu = h.mean(0)
    var = h.var(0)
    h = (h - mu) / np.sqrt(var + 1e-5) * np.asarray(bn_g, f32) + \
        np.asarray(bn_b, f32)
    e = edge_attr @ np.asarray(gine_We, f32) + np.asarray(gine_be, f32)
    msg = np.maximum(h[src] + e, 0)
    agg = np.zeros_like(h)
    np.add.at(agg, dst, msg)
    h = h + agg
    h = np.maximum(h @ np.asarray(gine_W1, f32) +
                   np.asarray(gine_b1, f32), 0)
    h = h @ np.asarray(gine_W2, f32) + np.asarray(gine_b2, f32)
    h = np.maximum(h, 0)
    for l in range(2):
        xl = h @ np.asarray(gat_Wl, f32)[l] + np.asarray(gat_bl, f32)[l]
        xr = h @ np.asarray(gat_Wr, f32)[l] + np.asarray(gat_br, f32)[l]
        z = xl[src] + xr[dst] + edge_attr @ np.asarray(gat_We, f32)[l]
        z = np.where(z > 0, z, 0.2 * z)
        sc = z @ np.asarray(gat_att, f32)[l]
        m = np.full(N, -np.inf, f32)
        np.maximum.at(m, dst, sc)
        ex = np.exp(sc - m[dst])
        ssum = np.zeros(N, f32)
        np.add.at(ssum, dst, ex)
        alpha = ex / (ssum[dst] + np.float32(1e-16))
        acc = np.zeros_like(h)
        np.add.at(acc, dst, xl[src] * alpha[:, None])
        h = np.maximum(acc + np.asarray(gat_bias, f32)[l], 0)
    sc = np.tanh(h @ np.asarray(pool_W1, f32) + np.asarray(pool_b1, f32))
    sc = sc @ np.asarray(pool_w2, f32)
    ex = np.exp(sc - sc.max())
    w = ex / ex.sum()
    hw = h * w[:, None]
    pooled = np.zeros((ng, HID), f32)
    np.add.at(pooled, batch, hw)
    o = np.maximum(pooled @ np.asarray(lin1_W, f32) +
                   np.asarray(lin1_b, f32), 0)
    return (o @ np.asarray(lin2_W, f32) + np.asarray(lin2_b, f32)).astype(f32)


def kernel(x, edge_attr, emb, enc_W, enc_b, bn_g, bn_b,
           gine_We, gine_be, gine_W1, gine_b1, gine_W2, gine_b2,
           gat_Wl, gat_bl, gat_Wr, gat_br, gat_att, gat_We, gat_bias,
           pool_W1, pool_b1, pool_w2, lin1_W, lin1_b, lin2_W, lin2_b,
           edge_index, batch, group_ids, num_graphs):
    N = np.asarray(x).shape[0]
    E = np.asarray(edge_attr).shape[0]
    if not _is_structured(edge_index, batch, group_ids, num_graphs, N, E):
        return _numpy_fallback(
            x, edge_attr, emb, enc_W, enc_b, bn_g, bn_b, gine_We, gine_be,
            gine_W1, gine_b1, gine_W2, gine_b2, gat_Wl, gat_bl, gat_Wr,
            gat_br, gat_att, gat_We, gat_bias, pool_W1, pool_b1, pool_w2,
            lin1_W, lin1_b, lin2_W, lin2_b, edge_index, batch, group_ids,
            num_graphs)
    in_maps = host_prep(x, edge_attr, emb, enc_W, enc_b, bn_g, bn_b,
                        gine_We, gine_be, gine_W1, gine_b1, gine_W2,
                        gine_b2, gat_Wl, gat_bl, gat_Wr, gat_br, gat_att,
                        gat_We, gat_bias, pool_W1, pool_b1, pool_w2,
                        lin1_W, lin1_b, lin2_W, lin2_b, group_ids)
    runner = _get_runner()
    results = runner["run"](in_maps)
    return assemble_out(results)



# revision 40
# speedup vs baseline: 2.4549x; 1.0611x over previous
"""BrainNetGAT Bass/Tile kernel for 8 Trainium2 NeuronCores.

Graph-level data parallelism: 16 graphs per core, processed as 8 pairs with
two concurrent PE column-tiled streams. Edge message passing is dense
augmented matmuls over each graph's 116x116 edge grid, with edges in
DST-MAJOR order (edge = dst*116 + src):
  moving tile T = [one-hot src-index (116); ea^T (5); ones (2)]  [123, 13456]
  stationary   = [Xsrc (116); We (5); K (2)]                     [123, 64]
so one matmul yields ea@We + x_src[src] for every edge; a second matmul with
a per-chunk sliced broadcast-AP identity adds x_dst[dst]. Dst-major makes
the GINE segment-sum a contiguous-axis DVE reduce and makes the attention
score matrix load back from DRAM directly as [dst, src] with no transposes.
GAT attention scores are computed by a 4-stream block-diagonal att matmul
(bf16), bounced through DRAM. GINE relu runs on the Vector engine to keep
the Scalar/ACT engine for the GAT leaky-relu. Most small DMAs are issued
from the otherwise-idle Sync engine. BatchNorm stats and the global pooling
softmax sum use two small AllReduces.
"""
import contextlib

import numpy as np
import ml_dtypes

import concourse.bacc as bacc
import concourse.mybir as mybir
import concourse.tile as tile

F32 = mybir.dt.float32
BF16 = mybir.dt.bfloat16
AF = mybir.ActivationFunctionType
OP = mybir.AluOpType
AX = mybir.AxisListType

N_ROI = 116
HID = 64
EDIM = 5
N_GRAPHS = 128
N_CORES = 8
G_C = N_GRAPHS // N_CORES          # 16 graphs per core
PAIRS = G_C // 2
N_C = G_C * N_ROI                  # 1856 nodes per core
E_G = N_ROI * N_ROI                # 13456 edges per graph
E_C = G_C * E_G
N_TOTAL = N_GRAPHS * N_ROI         # 14848
# shared moving tile rows: 0:116 src-onehot, 116:121 eaA, 121:126 eaB,
# 126:128 ones (for the per-graph bf16-centering K rows)
EA_A = N_ROI                       # 116
EA_B = N_ROI + EDIM                # 121
ONES_R = N_ROI + 2 * EDIM          # 126
KAUG = ONES_R + 2                  # 128
KGINE = ONES_R                     # 126 rows for the GINE matmuls
CH = 4 * N_ROI                     # 464-col edge chunk (4 dst blocks)
NCH = E_G // CH                    # 29
GRP = 3                            # z-chunks per psum group
NODE_CH = 4 * N_ROI                # 464 node cols (4 graphs)


def build_nc(n_cores=N_CORES):
    nc = bacc.Bacc()
    d = {}

    def inp(name, shape, dt):
        d[name] = nc.declare_dram_parameter(name, list(shape), dt,
                                            isOutput=False)

    inp("xT", (N_ROI, N_C), BF16)
    inp("eaT", (EDIM, E_C), BF16)
    inp("embT", (16, N_ROI), BF16)
    inp("I116", (N_ROI, N_ROI), BF16)
    inp("encW1", (N_ROI, HID), BF16)
    inp("encW2", (16, HID), BF16)
    inp("enc_b", (HID, 1), F32)
    inp("bn_g", (HID, 1), F32)
    inp("bn_b", (HID, 1), F32)
    inp("gineWeZA", (2 * EDIM, HID), BF16)
    inp("gineWeZB", (2 * EDIM, HID), BF16)
    inp("gine_be", (HID, 1), F32)
    inp("gineW1", (HID, HID), F32)
    inp("gine_b1", (HID, 1), F32)
    inp("gineW2", (HID, HID), F32)
    inp("gine_b2", (HID, 1), F32)
    for l in range(2):
        inp(f"gatWl{l}", (HID, HID), F32)
        inp(f"gat_bl{l}", (HID, 1), F32)
        inp(f"gatWr{l}", (HID, HID), F32)
        inp(f"gat_br{l}", (HID, 1), F32)
        inp(f"gat_blb{l}", (HID, 1), F32)
        inp(f"gatWeZA{l}", (2 * EDIM, HID), BF16)
        inp(f"gatWeZB{l}", (2 * EDIM, HID), BF16)
        inp(f"att{l}", (128, 1), F32)
    inp("poolW1", (HID, HID), F32)
    inp("pool_b1", (HID, 1), F32)
    inp("pool_w2", (HID, 1), BF16)
    inp("lin1W", (HID, N_ROI), BF16)
    inp("lin1_b", (N_ROI, 1), F32)
    inp("lin2W", (N_ROI, 2), BF16)
    inp("lin2_b", (2, 1), F32)
    inp("ones2", (2, E_G), BF16)
    outT = nc.declare_dram_parameter("outT", [2, G_C], F32, isOutput=True)

    with tile.TileContext(nc) as tc:
        _body(nc, tc, d, outT, n_cores)
    nc.finalize()
    return nc


def _body(nc, tc, d, outT, n_cores=N_CORES):
    ctx = contextlib.ExitStack()
    with ctx:
        wpool = ctx.enter_context(tc.tile_pool(name="weights", bufs=1))
        state = ctx.enter_context(tc.tile_pool(name="state", bufs=1))
        tpool = ctx.enter_context(tc.tile_pool(name="tmoving", bufs=1))
        upool = ctx.enter_context(tc.tile_pool(name="u", bufs=1))
        spool = ctx.enter_context(tc.tile_pool(name="smalls", bufs=3))
        station = ctx.enter_context(tc.tile_pool(name="station", bufs=2))
        psA = ctx.enter_context(tc.tile_pool(name="psA", bufs=2, space="PSUM"))
        psB = ctx.enter_context(tc.tile_pool(name="psB", bufs=2, space="PSUM"))
        dpool = ctx.enter_context(tc.tile_pool(name="dram", bufs=2,
                                               space="DRAM"))
        bn_in = dpool.tile([HID, 2], F32, tag="bn_in", bufs=1)
        bn_out = dpool.tile([HID, 2], F32, tag="bn_out", bufs=1)
        s_in = dpool.tile([1, 1], F32, tag="s_in", bufs=1)
        s_out = dpool.tile([1, 1], F32, tag="s_out", bufs=1)
        psc_row = dpool.tile([1, N_C], F32, tag="psc_row", bufs=1)
        scAB0 = dpool.tile([2, E_G], F32, tag="scAB0", bufs=1)
        scAB1 = dpool.tile([2, E_G], F32, tag="scAB1", bufs=1)
        scABs = (scAB0, scAB1)

        # ---------- weights / constants ----------
        W = {}
        for name, h in d.items():
            if name in ("eaT", "ones2"):
                continue
            W[name] = wpool.tile(list(h.shape), h.dtype, tag=name, name=name)
            nc.gpsimd.dma_start(out=W[name][:], in_=h[:])

        ident = wpool.tile([128, 128], BF16, tag="ident")
        nc.vector.memset(ident[:], 0.0)
        nc.gpsimd.affine_select(out=ident[:], in_=ident[:],
                                compare_op=OP.not_equal, fill=1.0, base=0,
                                pattern=[[-1, 128]], channel_multiplier=1)
        identF = wpool.tile([128, 128], F32, tag="identF")
        nc.vector.memset(identF[:], 0.0)
        nc.gpsimd.affine_select(out=identF[:], in_=identF[:],
                                compare_op=OP.not_equal, fill=1.0, base=0,
                                pattern=[[-1, 128]], channel_multiplier=1)
        alpha02 = wpool.tile([128, 1], F32, tag="alpha02")
        nc.vector.memset(alpha02[:], 0.2)
        eps6 = wpool.tile([N_ROI, 1], F32, tag="eps6")
        nc.vector.memset(eps6[:], 1e-6)
        eps5 = wpool.tile([HID, 1], F32, tag="eps5")
        nc.vector.memset(eps5[:], 1e-5)
        ones116 = wpool.tile([N_ROI, 1], BF16, tag="ones116")
        nc.vector.memset(ones116[:], 1.0)

        attd = []
        for l in range(2):
            t = wpool.tile([128, 32], BF16, tag=f"attd{l}")
            nc.vector.memset(t[:], 0.0)
            nc.vector.tensor_copy(t[0:HID, 0:1], W[f"att{l}"][0:HID, :])
            nc.vector.tensor_copy(t[HID:128, 1:2], W[f"att{l}"][HID:128, :])
            attd.append(t)

        Bbe = wpool.tile([HID, 1], F32, tag="Bbe")

        # one shared moving tile (dst-major edges) for BOTH graphs of a pair:
        # rows 0:116 = s-onehot, 116:121 = eaA, 121:126 = eaB, 126:128 = ones
        Tb0 = tpool.tile([KAUG, E_G], BF16, tag="Tb0")
        Tb1 = tpool.tile([KAUG, E_G], BF16, tag="Tb1")
        T_bufs = (Tb0, Tb1)
        sind_src = W["I116"][:, :].unsqueeze(1).broadcast_to(
            [N_ROI, N_ROI, N_ROI])
        for Tt in T_bufs:
            nc.vector.tensor_copy(
                Tt[0:N_ROI, :].rearrange("p (dd s) -> p dd s", s=N_ROI),
                sind_src)
            nc.gpsimd.dma_start(out=Tt[ONES_R:KAUG, :], in_=d["ones2"][:])

        hT = state.tile([HID, N_C], F32, tag="hT")
        hbeT = state.tile([HID, N_C], BF16, tag="hbeT")
        h0T = state.tile([HID, N_C], BF16, tag="h0T")
        tanT = state.tile([HID, N_C], BF16, tag="tanT")
        eMat = state.tile([N_ROI, G_C], F32, tag="eMat")
        hnew = []
        for g in range(G_C):
            hn_t = state.tile([N_ROI, HID], BF16, tag=f"hnew{g}",
                              name=f"hnew{g}")
            hnew.append(hn_t)

        # ---------- phase A: encoder + BN ----------
        emb_b = W["embT"][:, :].unsqueeze(1).broadcast_to([16, 4, N_ROI])
        for k in range(N_C // NODE_CH):
            sl = slice(k * NODE_CH, (k + 1) * NODE_CH)
            ps = psB.tile([HID, NODE_CH], F32, tag="mm_ps")
            nc.tensor.matmul(ps[:], W["encW1"][:], W["xT"][:, sl],
                             start=True, stop=False)
            nc.tensor.matmul(ps[:], W["encW2"][:], emb_b,
                             start=False, stop=True)
            nc.scalar.activation(h0T[:, sl], ps[:], AF.Relu,
                                 bias=W["enc_b"][:])

        st = spool.tile([HID, 2], F32, tag="bn_st")
        sq = upool.tile([HID, N_C], BF16, tag="sq")
        nc.vector.tensor_reduce(st[:, 0:1], h0T[:, :], axis=AX.X, op=OP.add)
        nc.vector.tensor_tensor(sq[:], h0T[:], h0T[:], op=OP.mult)
        nc.vector.tensor_reduce(st[:, 1:2], sq[:, :], axis=AX.X, op=OP.add)
        nc.gpsimd.dma_start(out=bn_in[:], in_=st[:])
        nc.gpsimd.collective_compute(
            "AllReduce", OP.add, replica_groups=[list(range(n_cores))],
            ins=[bn_in[:]], outs=[bn_out[:]])
        stg = spool.tile([HID, 2], F32, tag="bn_stg")
        nc.gpsimd.dma_start(out=stg[:], in_=bn_out[:])

        mu = spool.tile([HID, 1], F32, tag="mu")
        var = spool.tile([HID, 1], F32, tag="var")
        sd = spool.tile([HID, 1], F32, tag="sd")
        A = spool.tile([HID, 1], F32, tag="A")
        B = spool.tile([HID, 1], F32, tag="B")
        t3 = spool.tile([HID, 1], F32, tag="t3")
        nc.vector.tensor_scalar_mul(mu[:], stg[:, 0:1], 1.0 / N_TOTAL)
        nc.vector.tensor_scalar_mul(var[:], stg[:, 1:2], 1.0 / N_TOTAL)
        nc.vector.tensor_tensor(t3[:], mu[:], mu[:], op=OP.mult)
        nc.vector.tensor_tensor(var[:], var[:], t3[:], op=OP.subtract)
        nc.scalar.activation(sd[:], var[:], AF.Sqrt, bias=eps5[:])
        nc.vector.reciprocal(sd[:], sd[:])
        nc.vector.tensor_tensor(A[:], sd[:], W["bn_g"][:], op=OP.mult)
        nc.vector.tensor_tensor(t3[:], mu[:], A[:], op=OP.mult)
        nc.vector.tensor_tensor(B[:], W["bn_b"][:], t3[:], op=OP.subtract)
        nc.vector.tensor_tensor(Bbe[:], B[:], W["gine_be"][:], op=OP.add)
        nc.scalar.activation(hT[:, :], h0T[:, :], AF.Identity,
                             bias=B[:], scale=A[:])
        nc.scalar.activation(hbeT[:, :], h0T[:, :], AF.Identity,
                             bias=Bbe[:], scale=A[:])

        # ---------- phase B: software-pipelined pairs ----------
        def gine_head(p):
            """ea load + GINE stationaries + edge matmuls + relu/segsum."""
            gA, gB = 2 * p, 2 * p + 1
            cA = slice(gA * N_ROI, (gA + 1) * N_ROI)
            cB = slice(gB * N_ROI, (gB + 1) * N_ROI)
            T = T_bufs[p % 2]
            nc.gpsimd.dma_start(out=T[EA_A:EA_A + EDIM, :],
                                in_=d["eaT"][:, gA * E_G:(gA + 1) * E_G])
            nc.gpsimd.dma_start(out=T[EA_B:EA_B + EDIM, :],
                                in_=d["eaT"][:, gB * E_G:(gB + 1) * E_G])
            SA = station.tile([KGINE, HID], BF16, tag="SA")
            SB = station.tile([KGINE, HID], BF16, tag="SB")
            nc.gpsimd.dma_start(out=SA[EA_A:KGINE, :],
                                in_=d["gineWeZA"][:])
            nc.gpsimd.dma_start(out=SB[EA_A:KGINE, :],
                                in_=d["gineWeZB"][:])
            for (S, cg) in ((SA, cA), (SB, cB)):
                trp = psB.tile([N_ROI, HID], BF16, tag="mm_ps")
                nc.tensor.transpose(trp[:], hbeT[:, cg], ident[0:HID, 0:HID])
                nc.vector.tensor_copy(S[0:N_ROI, :], trp[:])

            agg = spool.tile([128, N_ROI], F32, tag="agg",
                             name=f"agg{p}")
            for g0 in range(0, NCH, GRP):
                ng = min(GRP, NCH - g0)
                zps = psA.tile([128, GRP * 512], F32, tag="zps")
                for j in range(ng):
                    ch = slice((g0 + j) * CH, (g0 + j + 1) * CH)
                    pj = slice(j * 512, j * 512 + CH)
                    nc.tensor.matmul(zps[0:HID, pj], SA,
                                     T[0:KGINE, ch],
                                     start=True, stop=True)
                for j in range(ng):
                    ch = slice((g0 + j) * CH, (g0 + j + 1) * CH)
                    pj = slice(j * 512, j * 512 + CH)
                    nc.tensor.matmul(zps[HID:128, pj], SB,
                                     T[0:KGINE, ch],
                                     start=True, stop=True,
                                     tile_position=(0, 64))
                # relu into a small scratch, then contiguous segment-sum;
                # alternate relu between Scalar and Vector so neither
                # engine paces the GINE group pipeline alone
                u1g = spool.tile([128, GRP * CH], BF16, tag="u1g")
                srcv = zps[:, :].rearrange("p (g c) -> p g c",
                                           c=512)[:, 0:ng, 0:CH]
                dstv = u1g[:, 0:ng * CH].rearrange("p (g c) -> p g c", c=CH)
                if (g0 // GRP) % 2 == 0:
                    nc.scalar.activation(dstv, srcv, AF.Relu)
                else:
                    nc.vector.tensor_scalar_max(dstv, srcv, 0.0)
                rv = u1g[:, 0:ng * CH].rearrange("p (dd s) -> p dd s",
                                                 s=N_ROI)
                nc.vector.tensor_reduce(agg[:, 4 * g0:4 * (g0 + ng)], rv,
                                        axis=AX.X, op=OP.add)
            return agg

        def gine_rest(p, agg):
            gA, gB = 2 * p, 2 * p + 1
            cA = slice(gA * N_ROI, (gA + 1) * N_ROI)
            cB = slice(gB * N_ROI, (gB + 1) * N_ROI)
            cP = slice(gA * N_ROI, (gB + 1) * N_ROI)
            nc.vector.tensor_tensor(hT[:, cA], hT[:, cA], agg[0:HID, :],
                                    op=OP.add)
            aggB = spool.tile([HID, N_ROI], F32, tag="aggB")
            nc.gpsimd.dma_start(out=aggB[:], in_=agg[HID:128, :])
            nc.vector.tensor_tensor(hT[:, cB], hT[:, cB], aggB[:],
                                    op=OP.add)
            mp1 = psB.tile([HID, 2 * N_ROI], F32, tag="mm_ps")
            nc.tensor.matmul(mp1[:], W["gineW1"][:], hT[:, cP],
                             start=True, stop=True)
            mt = spool.tile([HID, 2 * N_ROI], F32, tag="mt")
            nc.scalar.activation(mt[:], mp1[:], AF.Relu, bias=W["gine_b1"][:])
            mp2 = psB.tile([HID, 2 * N_ROI], F32, tag="mm_ps")
            nc.tensor.matmul(mp2[:], W["gineW2"][:], mt[:],
                             start=True, stop=True)
            nc.scalar.activation(hT[:, cP], mp2[:], AF.Relu,
                                 bias=W["gine_b2"][:])

        def gat_edges(p, l):
            """xl/xr projections, stationaries, edge matmuls + prelu,
            attention-score matmuls with PSUM-direct extraction."""
            gA, gB = 2 * p, 2 * p + 1
            cP = slice(gA * N_ROI, (gB + 1) * N_ROI)
            lA = slice(0, N_ROI)
            lB = slice(N_ROI, 2 * N_ROI)
            T = T_bufs[p % 2]
            scAB = scABs[l]
            xps = psB.tile([HID, 2 * N_ROI], F32, tag="mm_ps")
            nc.tensor.matmul(xps[:], W[f"gatWl{l}"][:], hT[:, cP],
                             start=True, stop=True)
            xlT = spool.tile([HID, 2 * N_ROI], F32, tag="xlT")
            xlbT = spool.tile([HID, 2 * N_ROI], F32, tag="xlbT")
            nc.scalar.activation(xlT[:], xps[:], AF.Identity,
                                 bias=W[f"gat_bl{l}"][:])
            nc.scalar.activation(xlbT[:], xps[:], AF.Identity,
                                 bias=W[f"gat_blb{l}"][:])
            xps2 = psB.tile([HID, 2 * N_ROI], F32, tag="mm_ps")
            nc.tensor.matmul(xps2[:], W[f"gatWr{l}"][:], hT[:, cP],
                             start=True, stop=True)
            xrT = spool.tile([HID, 2 * N_ROI], F32, tag="xrT")
            nc.scalar.activation(xrT[:], xps2[:], AF.Identity,
                                 bias=W[f"gat_br{l}"][:])

            SA2 = station.tile([KAUG, HID], BF16, tag="SA2")
            SB2 = station.tile([KAUG, HID], BF16, tag="SB2")
            XrA = station.tile([N_ROI, HID], BF16, tag="XrA")
            XrB = station.tile([N_ROI, HID], BF16, tag="XrB")
            XlbA = station.tile([N_ROI, HID], F32, tag="XlbA")
            XlbB = station.tile([N_ROI, HID], F32, tag="XlbB")
            nc.sync.dma_start(out=SA2[EA_A:ONES_R, :],
                              in_=d[f"gatWeZA{l}"][:])
            nc.sync.dma_start(out=SB2[EA_A:ONES_R, :],
                              in_=d[f"gatWeZB{l}"][:])
            for (S, Xr, Xlb, lg) in ((SA2, XrA, XlbA, lA),
                                     (SB2, XrB, XlbB, lB)):
                # per-graph centering of xl/xr; exact offset via 2 rows
                mL = spool.tile([HID, 1], F32, tag="mL")
                mR = spool.tile([HID, 1], F32, tag="mR")
                nc.vector.tensor_reduce(mL[:], xlT[:, lg], axis=AX.X,
                                        op=OP.add)
                nc.vector.tensor_scalar_mul(mL[:], mL[:], 1.0 / N_ROI)
                nc.vector.tensor_reduce(mR[:], xrT[:, lg], axis=AX.X,
                                        op=OP.add)
                nc.vector.tensor_scalar_mul(mR[:], mR[:], 1.0 / N_ROI)
                xlc = spool.tile([HID, N_ROI], BF16, tag="xlc")
                xrc = spool.tile([HID, N_ROI], BF16, tag="xrc")
                nc.vector.tensor_scalar(xlc[:], xlT[:, lg], mL[:],
                                        scalar2=None,
                                        op0=OP.subtract)
                nc.vector.tensor_scalar(xrc[:], xrT[:, lg], mR[:],
                                        scalar2=None,
                                        op0=OP.subtract)
                Ksum = spool.tile([HID, 1], F32, tag="Ksum")
                nc.vector.tensor_tensor(Ksum[:], mL[:], mR[:], op=OP.add)
                K2 = spool.tile([HID, 2], BF16, tag="K2")
                nc.vector.tensor_copy(K2[:, 0:1], Ksum[:])
                Klo = spool.tile([HID, 1], F32, tag="Klo")
                nc.vector.tensor_tensor(Klo[:], Ksum[:], K2[:, 0:1],
                                        op=OP.subtract)
                nc.vector.tensor_copy(K2[:, 1:2], Klo[:])
                k2p = psB.tile([2, HID], BF16, tag="mm_ps")
                nc.tensor.transpose(k2p[:], K2[:], ident[0:HID, 0:HID])
                k2s = spool.tile([2, HID], BF16, tag="k2s")
                nc.vector.tensor_copy(k2s[:], k2p[:])
                nc.gpsimd.dma_start(out=S[ONES_R:KAUG, :],
                                    in_=k2s[:])
                t1p = psB.tile([N_ROI, HID], BF16, tag="mm_ps")
                nc.tensor.transpose(t1p[:], xlc[:], ident[0:HID, 0:HID])
                nc.vector.tensor_copy(S[0:N_ROI, :], t1p[:])
                t2p = psB.tile([N_ROI, HID], BF16, tag="mm_ps")
                nc.tensor.transpose(t2p[:], xrc[:], ident[0:HID, 0:HID])
                nc.vector.tensor_copy(Xr[:], t2p[:])
                t3p = psB.tile([N_ROI, HID], F32, tag="mm_ps")
                nc.tensor.transpose(t3p[:], xlbT[:, lg],
                                    identF[0:HID, 0:HID])
                nc.vector.tensor_copy(Xlb[:], t3p[:])

            u2 = upool.tile([128, E_G], BF16, tag="u")
            for g0 in range(0, NCH, GRP):
                ng = min(GRP, NCH - g0)
                zps = psA.tile([128, GRP * 512], F32, tag="zps")
                for j in range(ng):
                    ch = slice((g0 + j) * CH, (g0 + j + 1) * CH)
                    pj = slice(j * 512, j * 512 + CH)
                    nc.tensor.matmul(zps[0:HID, pj], SA2, T[:, ch],
                                     start=True, stop=False)
                for j in range(ng):
                    c4 = slice(4 * (g0 + j), 4 * (g0 + j) + 4)
                    pj = slice(j * 512, j * 512 + CH)
                    dind = W["I116"][:, c4].unsqueeze(2).broadcast_to(
                        [N_ROI, 4, N_ROI])
                    nc.tensor.matmul(zps[0:HID, pj], XrA, dind,
                                     start=False, stop=True)
                for j in range(ng):
                    ch = slice((g0 + j) * CH, (g0 + j + 1) * CH)
                    pj = slice(j * 512, j * 512 + CH)
                    nc.tensor.matmul(zps[HID:128, pj], SB2, T[:, ch],
                                     start=True, stop=False,
                                     tile_position=(0, 64))
                for j in range(ng):
                    c4 = slice(4 * (g0 + j), 4 * (g0 + j) + 4)
                    pj = slice(j * 512, j * 512 + CH)
                    dind = W["I116"][:, c4].unsqueeze(2).broadcast_to(
                        [N_ROI, 4, N_ROI])
                    nc.tensor.matmul(zps[HID:128, pj], XrB, dind,
                                     start=False, stop=True,
                                     tile_position=(0, 64))
                src = zps[:, :].rearrange("p (g c) -> p g c",
                                          c=512)[:, 0:ng, 0:CH]
                dst = u2[:, g0 * CH:(g0 + ng) * CH].rearrange(
                    "p (g c) -> p g c", c=CH)
                nc.scalar.activation(dst, src, AF.Prelu,
                                     alpha=alpha02[:])

            # attention scores: 4 col-tiled streams -> rows 0,32,64,96
            scAB_c = scAB[:, :].rearrange("r (cc c) -> r cc c", c=CH)
            for base0 in range(0, NCH, 12):
                n = min(12, NCH - base0)
                npad = (n + 3) // 4 * 4
                nslot = npad // 4
                sps = psA.tile([128, GRP * 512], F32, tag="zps")
                for idx in range(npad):
                    c = base0 + min(idx, n - 1)
                    k, j = idx % 4, idx // 4
                    nc.tensor.matmul(
                        sps[32 * k:32 * k + 32, j * 512:j * 512 + CH],
                        attd[l], u2[:, c * CH:(c + 1) * CH],
                        start=True, stop=True,
                        tile_position=(0, 32 * k))
                scc = spool.tile([128, GRP * 512], F32, tag="scc", bufs=2)
                ssrc = sps[:, :].rearrange("p (j c) -> p j c",
                                           c=512)[:, 0:nslot, 0:CH]
                sdst = scc[:, 0:nslot * CH].rearrange(
                    "p (j c) -> p j c", c=CH)
                nc.scalar.activation(sdst, ssrc, AF.Copy)
                for k in range(4):
                    nk = len([i for i in range(n) if i % 4 == k])
                    if nk == 0:
                        continue
                    src3 = scc[32 * k:32 * k + 2, 0:nslot * CH].rearrange(
                        "p (j c) -> p j c", c=CH)[:, 0:nk, :]
                    dst3 = scAB_c[:, base0 + k:base0 + n:4, :]
                    nc.sync.dma_start(out=dst3, in_=src3)
            return XlbA, XlbB

        def gat_tail(p, l, XlbA, XlbB):
            """per-graph softmax + alpha-weighted aggregation."""
            gA, gB = 2 * p, 2 * p + 1
            cA = slice(gA * N_ROI, (gA + 1) * N_ROI)
            cB = slice(gB * N_ROI, (gB + 1) * N_ROI)
            # dst-major: scAB rows reload directly as [dst, src]
            scAB_m = scABs[l][:, :].rearrange("r (dd s) -> r dd s",
                                              s=N_ROI)
            for (g, rr, Xlb, cg) in ((gA, 0, XlbA, cA),
                                     (gB, 1, XlbB, cB)):
                epT = spool.tile([N_ROI, N_ROI], F32, tag="epT")
                nc.sync.dma_start(out=epT[:], in_=scAB_m[rr])
                mrow = spool.tile([N_ROI, 1], F32, tag="mrow")
                nc.vector.tensor_reduce(mrow[:], epT[:, :], axis=AX.X,
                                        op=OP.max)
                mneg = spool.tile([N_ROI, 1], F32, tag="mneg")
                nc.vector.tensor_scalar_mul(mneg[:], mrow[:], -1.0)
                ehT = spool.tile([N_ROI, N_ROI], F32, tag="ehT")
                nc.scalar.activation(ehT[:], epT[:], AF.Exp,
                                     bias=mneg[:])
                srow = spool.tile([N_ROI, 1], F32, tag="srow")
                nc.vector.tensor_reduce(srow[:], ehT[:, :], axis=AX.X,
                                        op=OP.add)
                rrow = spool.tile([N_ROI, 1], F32, tag="rrow")
                nc.vector.reciprocal(rrow[:], srow[:])
                adT = spool.tile([N_ROI, N_ROI], F32, tag="adT")
                nc.vector.tensor_scalar_mul(adT[:], ehT[:], rrow[:])
                as_ps = psB.tile([N_ROI, N_ROI], F32, tag="mm_ps")
                nc.tensor.transpose(as_ps[:], adT[:],
                                    identF[0:N_ROI, 0:N_ROI])
                aS = spool.tile([N_ROI, N_ROI], F32, tag="aS")
                nc.vector.tensor_copy(aS[:], as_ps[:])
                am = psB.tile([N_ROI, HID], F32, tag="mm_ps")
                nc.tensor.matmul(am[:], aS[:], Xlb[:],
                                 start=True, stop=True)
                hnF = spool.tile([N_ROI, HID], F32, tag="hnF")
                nc.scalar.activation(hnF[:], am[:], AF.Relu)
                nc.vector.tensor_copy(hnew[g][:], hnF[:])
                htp = psB.tile([HID, N_ROI], F32, tag="mm_ps")
                nc.tensor.transpose(htp[:], hnF[:],
                                    identF[0:N_ROI, 0:N_ROI])
                nc.vector.tensor_copy(hT[:, cg], htp[:])

        def pool_scores(p):
            gA, gB = 2 * p, 2 * p + 1
            cP = slice(gA * N_ROI, (gB + 1) * N_ROI)
            pps = psB.tile([HID, 2 * N_ROI], F32, tag="mm_ps")
            nc.tensor.matmul(pps[:], W["poolW1"][:], hT[:, cP],
                             start=True, stop=True)
            nc.scalar.activation(tanT[:, cP], pps[:], AF.Tanh,
                                 bias=W["pool_b1"][:])
            scp = psB.tile([1, 2 * N_ROI], F32, tag="mm_ps")
            nc.tensor.matmul(scp[:], W["pool_w2"][:], tanT[:, cP],
                             start=True, stop=True)
            scs = spool.tile([1, 2 * N_ROI], F32, tag="scs")
            nc.vector.tensor_copy(scs[:], scp[:])
            nc.gpsimd.dma_start(
                out=psc_row[0, p * 2 * N_ROI:(p + 1) * 2 * N_ROI],
                in_=scs[:])

        # 2-deep software pipeline: the next pair's GINE head and first
        # GAT edge phase fill the PE while this pair's softmax tails run
        agg0 = gine_head(0)
        gine_rest(0, agg0)
        E0 = gat_edges(0, 0)
        for p in range(PAIRS):
            if p + 1 < PAIRS:
                agg_n = gine_head(p + 1)
            gat_tail(p, 0, *E0)
            E1 = gat_edges(p, 1)
            if p + 1 < PAIRS:
                gine_rest(p + 1, agg_n)
                E0 = gat_edges(p + 1, 0)
            gat_tail(p, 1, *E1)
            pool_scores(p)

        # ---------- phase C: pooling + head ----------
        nc.gpsimd.dma_start(
            out=eMat[:], in_=psc_row[0, :].rearrange("(g r) -> r g", r=N_ROI))
        eMb = spool.tile([N_ROI, G_C], BF16, tag="eMb")
        nc.scalar.activation(eMb[:], eMat[:], AF.Exp)
        ssum_ps = psB.tile([1, G_C], F32, tag="mm_ps")
        nc.tensor.matmul(ssum_ps[:], ones116[:], eMb[:],
                         start=True, stop=True)
        ssum = spool.tile([1, 1], F32, tag="ssum")
        nc.vector.tensor_reduce(ssum[:], ssum_ps[0:1, :], axis=AX.X,
                                op=OP.add)
        nc.gpsimd.dma_start(out=s_in[:], in_=ssum[:])
        nc.gpsimd.collective_compute(
            "AllReduce", OP.add, replica_groups=[list(range(n_cores))],
            ins=[s_in[:]], outs=[s_out[:]])
        sS64 = spool.tile([HID, 1], F32, tag="sS64")
        nc.gpsimd.dma_start(out=sS64[:], in_=s_out[:].broadcast_to([HID, 1]))
        nc.vector.reciprocal(sS64[:], sS64[:])

        pool_ps = psB.tile([HID, G_C], F32, tag="mm_ps")
        for g in range(G_C):
            nc.tensor.matmul(pool_ps[:, g:g + 1], hnew[g][:],
                             eMb[:, g:g + 1], start=True, stop=True)
        pooledT = spool.tile([HID, G_C], BF16, tag="pooledT")
        nc.scalar.activation(pooledT[:], pool_ps[:], AF.Identity,
                             scale=sS64[:])
        o1ps = psB.tile([N_ROI, G_C], F32, tag="mm_ps")
        nc.tensor.matmul(o1ps[:], W["lin1W"][:], pooledT[:],
                         start=True, stop=True)
        o1 = spool.tile([N_ROI, G_C], BF16, tag="o1")
        nc.scalar.activation(o1[:], o1ps[:], AF.Relu, bias=W["lin1_b"][:])
        o2ps = psB.tile([2, G_C], F32, tag="mm_ps")
        nc.tensor.matmul(o2ps[:], W["lin2W"][:], o1[:], start=True, stop=True)
        oflast = spool.tile([2, G_C], F32, tag="oflast")
        nc.scalar.activation(oflast[:], o2ps[:], AF.Identity,
                             bias=W["lin2_b"][:])
        nc.gpsimd.dma_start(out=outT[:], in_=oflast[:])


# ---------------------------------------------------------------------------
_NC_CACHE = {}


def get_nc():
    if "nc" not in _NC_CACHE:
        _NC_CACHE["nc"] = build_nc()
    return _NC_CACHE["nc"]


def host_prep(x, edge_attr, emb, enc_W, enc_b, bn_g, bn_b,
              gine_We, gine_be, gine_W1, gine_b1, gine_W2, gine_b2,
              gat_Wl, gat_bl, gat_Wr, gat_br, gat_att, gat_We, gat_bias,
              pool_W1, pool_b1, pool_w2, lin1_W, lin1_b, lin2_W, lin2_b,
              group_ids):
    bf = ml_dtypes.bfloat16
    f32 = np.float32

    def col(v):
        return np.ascontiguousarray(np.asarray(v, f32).reshape(-1, 1))

    base = {
        "embT": np.ascontiguousarray(
            np.asarray(emb, f32).T[:, np.asarray(group_ids[:N_ROI])]
        ).astype(bf),
        "I116": np.eye(N_ROI).astype(bf),
        "encW1": np.ascontiguousarray(np.asarray(enc_W, f32)[:N_ROI]
                                      ).astype(bf),
        "encW2": np.ascontiguousarray(np.asarray(enc_W, f32)[N_ROI:]
                                      ).astype(bf),
        "enc_b": col(enc_b), "bn_g": col(bn_g), "bn_b": col(bn_b),
        "gineWeZA": np.vstack([np.asarray(gine_We, f32),
                               np.zeros((EDIM, HID), f32)]).astype(bf),
        "gineWeZB": np.vstack([np.zeros((EDIM, HID), f32),
                               np.asarray(gine_We, f32)]).astype(bf),
        "gine_be": col(gine_be),
        "gineW1": np.asarray(gine_W1, f32),
        "gine_b1": col(gine_b1),
        "gineW2": np.asarray(gine_W2, f32),
        "gine_b2": col(gine_b2),
        "poolW1": np.asarray(pool_W1, f32),
        "pool_b1": col(pool_b1),
        "pool_w2": col(pool_w2).astype(bf),
        "lin1W": np.asarray(lin1_W, f32).astype(bf),
        "lin1_b": col(lin1_b),
        "lin2W": np.asarray(lin2_W, f32).astype(bf),
        "lin2_b": col(lin2_b),
        "ones2": np.ones((2, E_G)).astype(bf),
    }
    for l in range(2):
        base[f"gatWl{l}"] = np.asarray(gat_Wl[l], f32)
        base[f"gat_bl{l}"] = col(gat_bl[l])
        base[f"gatWr{l}"] = np.asarray(gat_Wr[l], f32)
        base[f"gat_br{l}"] = col(gat_br[l])
        base[f"gat_blb{l}"] = col(np.asarray(gat_bl[l], f32) +
                                  np.asarray(gat_bias[l], f32))
        base[f"gatWeZA{l}"] = np.vstack([np.asarray(gat_We[l], f32),
                                         np.zeros((EDIM, HID), f32)
                                         ]).astype(bf)
        base[f"gatWeZB{l}"] = np.vstack([np.zeros((EDIM, HID), f32),
                                         np.asarray(gat_We[l], f32)
                                         ]).astype(bf)
        base[f"att{l}"] = col(np.concatenate([np.asarray(gat_att[l], f32), np.asarray(gat_att[l], f32)]))

    xT = np.ascontiguousarray(np.asarray(x, f32).T).astype(bf)
    # reorder edges to dst-major within each graph: edge = dst*116 + src
    ea4 = np.asarray(edge_attr, f32).reshape(N_GRAPHS, N_ROI, N_ROI, EDIM)
    ea_dm = np.ascontiguousarray(ea4.transpose(0, 2, 1, 3)).reshape(-1, EDIM)
    eaT = np.ascontiguousarray(ea_dm.T).astype(bf)

    in_maps = []
    for c in range(N_CORES):
        m = dict(base)
        m["xT"] = np.ascontiguousarray(xT[:, c * N_C:(c + 1) * N_C])
        m["eaT"] = np.ascontiguousarray(eaT[:, c * E_C:(c + 1) * E_C])
        in_maps.append(m)
    return in_maps


def assemble_out(results):
    return np.concatenate([np.asarray(r["outT"], np.float32).T
                           for r in results], axis=0)


# ===========================================================================
# SPMD runner (replicates bass2jax.run_bass_via_pjrt, but reusable + timeable)
# ===========================================================================
def _make_runner(nc, n_cores=N_CORES):
    import jax
    import jax.numpy as jnp
    from jax.sharding import Mesh, PartitionSpec
    from jax.experimental.shard_map import shard_map
    from concourse import bass2jax
    from concourse.bass2jax import _bass_exec_p, partition_id_tensor
    import concourse.mybir as mb

    bass2jax.install_neuronx_cc_hook()
    partition_name = (nc.partition_id_tensor.name
                      if nc.partition_id_tensor else None)
    in_names, out_names, out_avals, zero_outs = [], [], [], []
    for alloc in nc.m.functions[0].allocations:
        if not isinstance(alloc, mb.MemoryLocationSet):
            continue
        name = alloc.memorylocations[0].name
        if alloc.kind == "ExternalInput":
            if name != partition_name:
                in_names.append(name)
        elif alloc.kind == "ExternalOutput":
            out_names.append(name)
            shape = tuple(alloc.tensor_shape)
            dtype = mb.dt.np(alloc.dtype)
            out_avals.append(jax.core.ShapedArray(shape, dtype))
            zero_outs.append(np.zeros(shape, dtype))
    n_params = len(in_names)
    all_in = in_names + out_names
    if partition_name is not None:
        all_in = all_in + [partition_name]

    def _body(*args):
        operands = list(args)
        if partition_name is not None:
            operands.append(partition_id_tensor())
        outs = _bass_exec_p.bind(
            *operands, out_avals=tuple(out_avals), in_names=tuple(all_in),
            out_names=tuple(out_names), lowering_input_output_aliases=(),
            sim_require_finite=False, sim_require_nnan=False, nc=nc)
        return tuple(outs)

    devices = jax.devices()[:n_cores]
    mesh = Mesh(np.asarray(devices), ("core",))
    nin = n_params + len(zero_outs)
    sharded = jax.jit(shard_map(
        _body, mesh=mesh, in_specs=(PartitionSpec("core"),) * nin,
        out_specs=(PartitionSpec("core"),) * len(out_names),
        check_rep=False), keep_unused=True)

    def run(in_maps):
        per_core = [[np.asarray(m[name]) for name in in_names]
                    for m in in_maps]
        concat_in = [np.concatenate([per_core[c][i] for c in range(n_cores)],
                                    axis=0) for i in range(n_params)]
        concat_zeros = [np.zeros((n_cores * z.shape[0], *z.shape[1:]),
                                 z.dtype) for z in zero_outs]
        out_arrs = sharded(*concat_in, *concat_zeros)
        jax.block_until_ready(out_arrs)
        return [{name: np.asarray(out_arrs[i]).reshape(
                    n_cores, *out_avals[i].shape)[c]
                 for i, name in enumerate(out_names)}
                for c in range(n_cores)]

    def run_device(dev_in, fresh_zero_arrs):
        out_arrs = sharded(*dev_in, *fresh_zero_arrs)
        jax.block_until_ready(out_arrs)
        return out_arrs

    runner = dict(run=run, run_device=run_device, sharded=sharded,
                  in_names=in_names, out_names=out_names,
                  out_avals=out_avals, zero_outs=zero_outs,
                  n_params=n_params, mesh=mesh, n_cores=n_cores)
    return runner


_RUNNER_CACHE = {}


def _get_runner():
    if "r" not in _RUNNER_CACHE:
        _RUNNER_CACHE["r"] = _make_runner(get_nc(), N_CORES)
    return _RUNNER_CACHE["r"]


# ===========================================================================
# structured-input check + numpy fallback
# ===========================================================================
def _is_structured(edge_index, batch, group_ids, num_graphs, N, E):
    ng = int(np.asarray(num_graphs))
    if ng != N_GRAPHS or N != ng * N_ROI or E != ng * E_G:
        return False
    src = np.asarray(edge_index[0])
    dst = np.asarray(edge_index[1])
    idx = np.arange(N_ROI)
    s = np.repeat(idx, N_ROI)
    dd = np.tile(idx, N_ROI)
    off = (np.arange(ng) * N_ROI)[:, None]
    if not np.array_equal(src.reshape(ng, E_G), s[None, :] + off):
        return False
    if not np.array_equal(dst.reshape(ng, E_G), dd[None, :] + off):
        return False
    if not np.array_equal(np.asarray(batch),
                          np.repeat(np.arange(ng), N_ROI)):
        return False
    gi = np.asarray(group_ids)
    if not np.array_equal(gi, np.tile(gi[:N_ROI], ng)):
        return False
    return True


def _numpy_fallback(x, edge_attr, emb, enc_W, enc_b, bn_g, bn_b,
                    gine_We, gine_be, gine_W1, gine_b1, gine_W2, gine_b2,
                    gat_Wl, gat_bl, gat_Wr, gat_br, gat_att, gat_We,
                    gat_bias, pool_W1, pool_b1, pool_w2, lin1_W, lin1_b,
                    lin2_W, lin2_b, edge_index, batch, group_ids,
                    num_graphs):
    f32 = np.float32
    x = np.asarray(x, f32)
    edge_attr = np.asarray(edge_attr, f32)
    src = np.asarray(edge_index[0]).astype(np.int64)
    dst = np.asarray(edge_index[1]).astype(np.int64)
    batch = np.asarray(batch).astype(np.int64)
    ng = int(np.asarray(num_graphs))
    N = x.shape[0]
    h = np.concatenate([x, np.asarray(emb, f32)[np.asarray(group_ids)]], 1)
    h = h @ np.asarray(enc_W, f32) + np.asarray(enc_b, f32)
    h = np.maximum(h, 0)
    mu = h.mean(0)
    var = h.var(0)
    h = (h - mu) / np.sqrt(var + 1e-5) * np.asarray(bn_g, f32) + \
        np.asarray(bn_b, f32)
    e = edge_attr @ np.asarray(gine_We, f32) + np.asarray(gine_be, f32)
    msg = np.maximum(h[src] + e, 0)
    agg = np.zeros_like(h)
    np.add.at(agg, dst, msg)
    h = h + agg
    h = np.maximum(h @ np.asarray(gine_W1, f32) +
                   np.asarray(gine_b1, f32), 0)
    h = h @ np.asarray(gine_W2, f32) + np.asarray(gine_b2, f32)
    h = np.maximum(h, 0)
    for l in range(2):
        xl = h @ np.asarray(gat_Wl, f32)[l] + np.asarray(gat_bl, f32)[l]
        xr = h @ np.asarray(gat_Wr, f32)[l] + np.asarray(gat_br, f32)[l]
        z = xl[src] + xr[dst] + edge_attr @ np.asarray(gat_We, f32)[l]
        z = np.where(z > 0, z, 0.2 * z)
        sc = z @ np.asarray(gat_att, f32)[l]
        m = np.full(N, -np.inf, f32)
        np.maximum.at(m, dst, sc)
        ex = np.exp(sc - m[dst])
        ssum = np.zeros(N, f32)
        np.add.at(ssum, dst, ex)
        alpha = ex / (ssum[dst] + np.float32(1e-16))
        acc = np.zeros_like(h)
        np.add.at(acc, dst, xl[src] * alpha[:, None])
        h = np.maximum(acc + np.asarray(gat_bias, f32)[l], 0)
    sc = np.tanh(h @ np.asarray(pool_W1, f32) + np.asarray(pool_b1, f32))
    sc = sc @ np.asarray(pool_w2, f32)
    ex = np.exp(sc - sc.max())
    w = ex / ex.sum()
    hw = h * w[:, None]
    pooled = np.zeros((ng, HID), f32)
    np.add.at(pooled, batch, hw)
    o = np.maximum(pooled @ np.asarray(lin1_W, f32) +
                   np.asarray(lin1_b, f32), 0)
    return (o @ np.asarray(lin2_W, f32) + np.asarray(lin2_b, f32)).astype(f32)


def kernel(x, edge_attr, emb, enc_W, enc_b, bn_g, bn_b,
           gine_We, gine_be, gine_W1, gine_b1, gine_W2, gine_b2,
           gat_Wl, gat_bl, gat_Wr, gat_br, gat_att, gat_We, gat_bias,
           pool_W1, pool_b1, pool_w2, lin1_W, lin1_b, lin2_W, lin2_b,
           edge_index, batch, group_ids, num_graphs):
    N = np.asarray(x).shape[0]
    E = np.asarray(edge_attr).shape[0]
    if not _is_structured(edge_index, batch, group_ids, num_graphs, N, E):
        return _numpy_fallback(
            x, edge_attr, emb, enc_W, enc_b, bn_g, bn_b, gine_We, gine_be,
            gine_W1, gine_b1, gine_W2, gine_b2, gat_Wl, gat_bl, gat_Wr,
            gat_br, gat_att, gat_We, gat_bias, pool_W1, pool_b1, pool_w2,
            lin1_W, lin1_b, lin2_W, lin2_b, edge_index, batch, group_ids,
            num_graphs)
    in_maps = host_prep(x, edge_attr, emb, enc_W, enc_b, bn_g, bn_b,
                        gine_We, gine_be, gine_W1, gine_b1, gine_W2,
                        gine_b2, gat_Wl, gat_bl, gat_Wr, gat_br, gat_att,
                        gat_We, gat_bias, pool_W1, pool_b1, pool_w2,
                        lin1_W, lin1_b, lin2_W, lin2_b, group_ids)
    runner = _get_runner()
    results = runner["run"](in_maps)
    return assemble_out(results)

